# revision 22
# baseline (speedup 1.0000x reference)
"""CaptionNet Trainium2 kernel (Bass/Tile, 8-core SPMD, batch-sharded).

Strategy:
- Batch-parallel over 8 NeuronCores (8 batches/core). No collectives.
- LSTM state kept transposed ([feature-part, batch-free]) so pointwise ops
  use all 128 lanes and the recurrent matmuls run weight-stationary.
- Attention scores via 4-way column-tiled PE matmuls (M=1 per batch).
- Matmuls run float32r (fp32 data, ~1e-4 error, 4x faster than fp32) except
  the weight-heavy gates / attention-value / logits matmuls which use fp16
  weights+activations (~5e-4) to halve PE ingest bytes.
- Softmax: exp(scale*x) with fused accumulate, no max-subtraction (scores
  are small by construction), normalization folded in as per-row scalars.
"""
import numpy as np
import ml_dtypes

import concourse.bass as bass
import concourse.tile as tile
from concourse import bacc, mybir
from concourse.bass_utils import run_bass_kernel_spmd
from concourse.masks import make_identity
from contextlib import ExitStack

F32 = mybir.dt.float32
F32R = mybir.dt.float32r
F16 = mybir.dt.float16
I32 = mybir.dt.int32
AF = mybir.ActivationFunctionType
OP = mybir.AluOpType

B, T, C, HW, V, E, U = 64, 20, 512, 196, 10000, 256, 512
NCORES = 8
BL = B // NCORES          # 8 batches per core
KP = 256                  # per-batch padded attention length (HW=196 -> 256)
G4 = 4 * U                # 2048 gate width
OD = U + C + E            # 1280 outs feature dim
VT = (V + 127) // 128     # 79 vocab M-tiles
VP = VT * 128             # 10112 padded vocab
INV_SCALE = 1.0 / float(U) ** 0.5

_CACHE = {}


def _build(t_steps=T):
    """Build + compile the per-core Bass program. Returns nc."""
    nc = bacc.Bacc("TRN2", target_bir_lowering=False, debug=False,
                   num_devices=NCORES)
    TB = t_steps * BL

    # ---- DRAM I/O ----
    d_featsT = nc.dram_tensor("featsT", [C, BL * HW], F32R, kind="ExternalInput")
    d_feats16 = nc.dram_tensor("feats16", [BL * KP, C], F16, kind="ExternalInput")
    d_emb = nc.dram_tensor("emb", [V, E], F32, kind="ExternalInput")
    d_capidx = nc.dram_tensor("capidx", [TB, 1], I32, kind="ExternalInput")
    d_WkT = nc.dram_tensor("WkT", [C, U], F32R, kind="ExternalInput")
    d_bkT = nc.dram_tensor("bkT", [128, 4], F32, kind="ExternalInput")
    d_Wh0T = nc.dram_tensor("Wh0T", [C, U], F32R, kind="ExternalInput")
    d_bh0T = nc.dram_tensor("bh0T", [128, 4], F32, kind="ExternalInput")
    d_Wc0T = nc.dram_tensor("Wc0T", [C, U], F32R, kind="ExternalInput")
    d_bc0T = nc.dram_tensor("bc0T", [128, 4], F32, kind="ExternalInput")
    d_WieT = nc.dram_tensor("WieT", [E, G4], F16, kind="ExternalInput")
    d_WahT = nc.dram_tensor("WahT", [C + U, G4], F16, kind="ExternalInput")
    d_bgT = nc.dram_tensor("bgT", [128, 16], F32, kind="ExternalInput")
    d_WoT = nc.dram_tensor("WoT", [OD, VP], F16, kind="ExternalInput")
    d_boT = nc.dram_tensor("boT", [128, VT], F32, kind="ExternalInput")

    d_logitsT = nc.dram_tensor("logitsT", [VT, 128, TB], F32, kind="ExternalOutput")
    d_attn = nc.dram_tensor("attn", [BL, t_steps, HW], F32, kind="ExternalOutput")

    with tile.TileContext(nc) as tc, ExitStack() as ctx:
        ctx.enter_context(nc.allow_low_precision(
            reason="float32r stores feed f32r matmuls by design"))
        per = ctx.enter_context(tc.tile_pool(name="per", bufs=1))

        # ---------- persistent SBUF ----------
        ident = per.tile([128, 128], F32)
        make_identity(nc, ident[:])

        featsT_sb = per.tile([128, 4, BL * HW], F32R)
        nc.sync.dma_start(featsT_sb[:],
                          d_featsT.ap().rearrange("(cc p) n -> p cc n", p=128))
        feats16_sb = per.tile([128, 2 * BL, C], F16)
        nc.sync.dma_start(feats16_sb[:],
                          d_feats16.ap().rearrange("(bh p) c -> p bh c", p=128))
        WkT_sb = per.tile([128, 4, U], F32R)
        nc.sync.dma_start(WkT_sb[:],
                          d_WkT.ap().rearrange("(cc p) u -> p cc u", p=128))
        Wh0T_sb = per.tile([128, 4, U], F32R)
        nc.sync.dma_start(Wh0T_sb[:],
                          d_Wh0T.ap().rearrange("(cc p) u -> p cc u", p=128))
        Wc0T_sb = per.tile([128, 4, U], F32R)
        nc.sync.dma_start(Wc0T_sb[:],
                          d_Wc0T.ap().rearrange("(cc p) u -> p cc u", p=128))
        WieT_sb = per.tile([128, 2, G4], F16)
        nc.sync.dma_start(WieT_sb[:],
                          d_WieT.ap().rearrange("(ee p) j -> p ee j", p=128))
        WahT_sb = per.tile([128, 8, G4], F16)
        nc.sync.dma_start(WahT_sb[:],
                          d_WahT.ap().rearrange("(kc p) j -> p kc j", p=128))
        bkT_sb = per.tile([128, 4], F32)
        nc.sync.dma_start(bkT_sb[:], d_bkT.ap())
        bh0T_sb = per.tile([128, 4], F32)
        nc.sync.dma_start(bh0T_sb[:], d_bh0T.ap())
        bc0T_sb = per.tile([128, 4], F32)
        nc.sync.dma_start(bc0T_sb[:], d_bc0T.ap())
        bgT_sb = per.tile([128, 16], F32)
        nc.sync.dma_start(bgT_sb[:], d_bgT.ap())
        boT_sb = per.tile([128, VT], F32)
        nc.sync.dma_start(boT_sb[:], d_boT.ap())

        keysT_sb = per.tile([128, 4, BL, KP], F16)   # padded [u, b, k]
        ge_sb = per.tile([128, 16, TB], F32)         # emb-part of gates + bias
        embT_sb = per.tile([128, 2, TB], F32)
        outsT16 = per.tile([128, 10, t_steps, BL], F16)

        hidT_sb = per.tile([128, 4, BL], F32)
        # 32-col padded so col-tiled score matmuls can write a full 32-row
        # block (cols 8..39 stay zero; junk rows land on unused partitions)
        hidT_r = per.tile([128, 4, BL + 32], F16)
        cellT_sb = per.tile([128, 4, BL], F32)
        xT16 = per.tile([128, 8, BL], F16)           # kc 0-3 = aT, 4-7 = hidT

        # =========== Phase A: embedding gather + renorm + transpose =========
        with tc.tile_pool(name="embp", bufs=1) as embp, \
             tc.tile_pool(name="embps", bufs=1, space="PSUM") as embps:
            segs = [(0, min(128, TB))]
            if TB > 128:
                segs.append((128, TB))
            seg_idx = []
            for si, (r0, r1) in enumerate(segs):
                ti = embp.tile([r1 - r0, 1], I32, tag=f"ix{si}")
                nc.sync.dma_start(ti[:], d_capidx.ap()[r0:r1, :])
                seg_idx.append((r1 - r0, ti, r0))
            ps_e = embps.tile([128, 4, 128], F32)
            for si, (npart, t_idx, off) in enumerate(seg_idx):
                g = embp.tile([npart, E], F32, tag=f"eg{si}")
                nc.gpsimd.indirect_dma_start(
                    out=g[:], out_offset=None, in_=d_emb.ap(),
                    in_offset=bass.IndirectOffsetOnAxis(ap=t_idx[:, :1], axis=0))
                sq = embp.tile([npart, E], F32, tag=f"sq{si}")
                n2 = embp.tile([npart, 1], F32, tag=f"n2{si}")
                nc.scalar.activation(sq[:], g[:], AF.Square,
                                     accum_out=n2[:, :1])
                nrm = embp.tile([npart, 1], F32, tag=f"nr{si}")
                nc.scalar.sqrt(nrm[:], n2[:])
                nc.vector.tensor_scalar_max(nrm[:], nrm[:], 1e-12)
                inv = embp.tile([npart, 1], F32, tag=f"iv{si}")
                nc.vector.reciprocal(inv[:], nrm[:])
                nc.vector.tensor_scalar_mul(inv[:], inv[:], 5.0)
                nc.vector.tensor_scalar_min(inv[:], inv[:], 1.0)
                nc.vector.tensor_scalar_mul(g[:], g[:], inv[:, :1])
                # transpose [npart, 256] -> embT [256, npart]
                for ee in range(2):
                    nc.tensor.transpose(
                        ps_e[:, 2 * si + ee, 0:npart],
                        g[:, ee * 128:(ee + 1) * 128],
                        ident[0:npart, 0:npart])
                    nc.vector.tensor_copy(
                        embT_sb[:, ee, off:off + npart],
                        ps_e[:, 2 * si + ee, 0:npart])
            # fp16 copy into outsT16 emb rows (kc 8,9)
            nc.vector.tensor_copy(
                outsT16[:, 8:10, :, :],
                embT_sb[:].rearrange("p ee (t b) -> p ee t b", b=BL))
            emb16 = embp.tile([128, 2, TB], F16)
            nc.vector.tensor_copy(emb16[:], embT_sb[:])

            # ======= Phase B: gates_e = W_ihe @ embT (+bias), fp16 =======
            with tc.tile_pool(name="geps", bufs=1, space="PSUM") as geps:
                for half in range(2):
                    ps_ge = geps.tile([128, 8, 256], F32, tag="ge")
                    for jt8 in range(8):
                        jt = half * 8 + jt8
                        for ee in range(2):
                            nc.tensor.matmul(
                                ps_ge[:, jt8, 0:TB],
                                WieT_sb[:, ee, jt * 128:(jt + 1) * 128],
                                emb16[:, ee, :],
                                start=(ee == 0), stop=(ee == 1))
                    for jt8 in range(8):
                        jt = half * 8 + jt8
                        eng = nc.vector if jt8 % 2 == 0 else nc.scalar
                        if eng is nc.vector:
                            nc.vector.tensor_scalar_add(
                                ge_sb[:, jt, :], ps_ge[:, jt8, 0:TB],
                                bgT_sb[:, jt:jt + 1])
                        else:
                            nc.scalar.activation(
                                ge_sb[:, jt, :], ps_ge[:, jt8, 0:TB],
                                AF.Identity, bias=bgT_sb[:, jt:jt + 1])

        # =========== Phase C: feats_mean + keys + initial state ===========
        with tc.tile_pool(name="inip", bufs=1) as inip, \
             tc.tile_pool(name="inips", bufs=1, space="PSUM") as inips, \
             tc.tile_pool(name="keyps", bufs=2, space="PSUM") as keyps:
            fsum = inip.tile([128, 4, BL], F32R)
            for cc in range(4):
                nc.vector.tensor_reduce(
                    fsum[:, cc, :],
                    featsT_sb[:, cc, :].rearrange("p (b k) -> p b k", b=BL),
                    axis=mybir.AxisListType.X, op=OP.add)
            # keys: out keysT [u-tile, (b,k)] ; evac into padded [u, b, KP]
            HHW = 4 * HW  # half the (b,k) range = 784
            for jt in range(4):
                for half in range(2):
                    ps_k = keyps.tile([128, HHW], F32, tag="k")
                    for cc in range(4):
                        for (n0, n1) in [(0, 512), (512, HHW)]:
                            nc.tensor.matmul(
                                ps_k[:, n0:n1],
                                WkT_sb[:, cc, jt * 128:(jt + 1) * 128],
                                featsT_sb[:, cc,
                                          half * HHW + n0:half * HHW + n1],
                                start=(cc == 0), stop=(cc == 3))
                    out_ap = keysT_sb[:, jt, half * 4:(half + 1) * 4, 0:HW]
                    in_ap = ps_k[:].rearrange("p (b k) -> p b k", b=4)
                    if (2 * jt + half) % 2 == 0:
                        nc.vector.tensor_scalar_add(out_ap, in_ap,
                                                    bkT_sb[:, jt:jt + 1])
                    else:
                        nc.scalar.activation(out_ap, in_ap, AF.Identity,
                                             bias=bkT_sb[:, jt:jt + 1])
                    # zero the k-padding (196..256) with a rounding store
                    nc.vector.tensor_scalar_mul(
                        keysT_sb[:, jt, half * 4:(half + 1) * 4, HW:KP],
                        ps_k[:, 0:4 * (KP - HW)]
                        .rearrange("p (b k) -> p b k", b=4),
                        0.0)
            # hid0 / cell0 (M-orientation, f32r)
            ps_i = inips.tile([128, 4, 128], F32, tag="i0")
            ps_c = inips.tile([128, 4, 128], F32, tag="c0")
            for jt in range(4):
                for cc in range(4):
                    nc.tensor.matmul(
                        ps_i[:, jt, 0:BL],
                        Wh0T_sb[:, cc, jt * 128:(jt + 1) * 128],
                        fsum[:, cc, :],
                        start=(cc == 0), stop=(cc == 3))
                    nc.tensor.matmul(
                        ps_c[:, jt, 0:BL],
                        Wc0T_sb[:, cc, jt * 128:(jt + 1) * 128],
                        fsum[:, cc, :],
                        start=(cc == 0), stop=(cc == 3))
            for jt in range(4):
                nc.vector.tensor_scalar_add(hidT_sb[:, jt, :],
                                            ps_i[:, jt, 0:BL],
                                            bh0T_sb[:, jt:jt + 1])
                nc.vector.tensor_scalar_add(cellT_sb[:, jt, :],
                                            ps_c[:, jt, 0:BL],
                                            bc0T_sb[:, jt:jt + 1])
            nc.vector.tensor_copy(xT16[:, 4:8, :], hidT_sb[:])
            nc.vector.tensor_copy(hidT_r[:, :, 0:BL], hidT_sb[:])
            nc.vector.tensor_scalar_mul(
                hidT_r[:, :, BL:], bh0T_sb[:].unsqueeze(-1)
                .to_broadcast([128, 4, BL + 32 - BL]), 0.0)

        # ======================= Phase D: the scan =======================
        with tc.tile_pool(name="scps", bufs=1, space="PSUM") as scps, \
             tc.tile_pool(name="scsb", bufs=2) as scsb:
            ps_sc2 = [scps.tile([128, 2, KP], F32, tag=f"sc{i}", name=f"ps_sc{i}")
                      for i in range(2)]
            nc.vector.memset(ps_sc2[0][:], 0.0)
            nc.vector.memset(ps_sc2[1][:], 0.0)
            ps_tr = scps.tile([128, 4, 128], F32)
            ps_aT = scps.tile([128, 4, BL], F32)
            ps_g2 = [scps.tile([128, 2, 16, BL], F32, tag=f"g{i}",
                               name=f"ps_g{i}") for i in range(2)]
            w_sb2 = [scsb.tile([128, 2, KP], F32, tag=f"w{i}", name=f"w_sb{i}")
                     for i in range(2)]
            nc.gpsimd.memset(w_sb2[0][:], 0.0)
            nc.gpsimd.memset(w_sb2[1][:], 0.0)

            for t in range(t_steps):
                ps_sc = ps_sc2[t % 2]
                ps_g = ps_g2[t % 2]

                # 1. gates hid-part (can overlap with attention)
                for jt in range(16):
                    for kc in range(4, 8):
                        nc.tensor.matmul(
                            ps_g[:, 0, jt, :],
                            WahT_sb[:, kc, jt * 128:(jt + 1) * 128],
                            xT16[:, kc, :],
                            start=(kc == 4), stop=(kc == 7))

                # 2. scores (col-tiled, f32r): batch b=g*4+c -> psum part 32c
                for g in range(2):
                    for c in range(4):
                        b = g * 4 + c
                        for jj in range(4):
                            nc.tensor.matmul(
                                ps_sc[32 * c:32 * c + 32, g, :],
                                hidT_r[:, jj, b:b + 32],
                                keysT_sb[:, jj, b, :],
                                start=(jj == 0), stop=(jj == 3),
                                tile_position=(0, 32 * c))

                # 3-5. softmax (no max-subtract; normalize via row scalars)
                w_sb = w_sb2[t % 2]
                sume = scsb.tile([128, 2], F32, tag="sume")
                rinv = scsb.tile([128, 2], F32, tag="rinv")
                for g in range(2):
                    nc.scalar.activation(
                        w_sb[:, g, 0:HW], ps_sc[:, g, 0:HW], AF.Exp,
                        scale=INV_SCALE, accum_out=sume[:, g:g + 1])
                nc.vector.reciprocal(rinv[:], sume[:])
                for g in range(2):
                    nc.vector.tensor_scalar_mul(
                        w_sb[:, g, 0:HW], w_sb[:, g, 0:HW], rinv[:, g:g + 1])
                # attn output rows b=g*4+c live at partition 32c
                for g in range(2):
                    nc.sync.dma_start(
                        d_attn.ap()[g * 4:(g + 1) * 4, t, :],
                        w_sb[0:128:32, g, 0:HW])

                # 6. transpose w -> wT (cols = batches at 32c)
                for hh in range(4):
                    g, h = hh // 2, hh % 2
                    nc.tensor.transpose(
                        ps_tr[:, hh, :],
                        w_sb[:, g, h * 128:(h + 1) * 128],
                        ident[:])
                eT16 = scsb.tile([128, 4, 4], F16, tag="eT")
                for hh in range(4):
                    nc.vector.tensor_copy(
                        eT16[:, hh, :], ps_tr[:, hh, 0:128:32])

                # 7. attention output aT[c,b] (fp16 feats as weights)
                for b in range(BL):
                    g, c = b // 4, b % 4
                    for cm in range(4):
                        for h in range(2):
                            nc.tensor.matmul(
                                ps_aT[:, cm, b:b + 1],
                                feats16_sb[:, b * 2 + h,
                                           cm * 128:(cm + 1) * 128],
                                eT16[:, g * 2 + h, c:c + 1],
                                start=(h == 0), stop=(h == 1))
                nc.vector.tensor_copy(xT16[:, 0:4, :], ps_aT[:])
                nc.vector.tensor_copy(outsT16[:, 4:8, t, :], ps_aT[:])

                # 8. gates a-part
                for jt in range(16):
                    for kc in range(0, 4):
                        nc.tensor.matmul(
                            ps_g[:, 1, jt, :],
                            WahT_sb[:, kc, jt * 128:(jt + 1) * 128],
                            xT16[:, kc, :],
                            start=(kc == 0), stop=(kc == 3))

                # 9. add hid-part + emb-part(with bias) + a-part
                gf = scsb.tile([128, 16, BL], F32, tag="gf")
                nc.vector.tensor_tensor(
                    out=gf[:], in0=ps_g[:, 0, :, :],
                    in1=ge_sb[:, :, t * BL:(t + 1) * BL], op=OP.add)
                nc.vector.tensor_tensor(
                    out=gf[:], in0=ps_g[:, 1, :, :], in1=gf[:], op=OP.add)

                # 10. LSTM pointwise on [128, (16jj, 8b)]
                gff = gf[:].rearrange("p jj b -> p (jj b)")
                pw = scsb.tile([128, 128], F32, tag="pw")
                pwf = pw[:]
                nc.scalar.activation(pwf[:, 0:64], gff[:, 0:64], AF.Sigmoid)
                nc.scalar.activation(pwf[:, 64:96], gff[:, 64:96], AF.Tanh)
                nc.scalar.activation(pwf[:, 96:128], gff[:, 96:128], AF.Sigmoid)
                m1 = scsb.tile([128, 32], F32, tag="m1")
                m2 = scsb.tile([128, 32], F32, tag="m2")
                cellf = cellT_sb[:].rearrange("p jj b -> p (jj b)")
                hidf = hidT_sb[:].rearrange("p jj b -> p (jj b)")
                nc.vector.tensor_tensor(out=m1[:], in0=pwf[:, 32:64],
                                        in1=cellf, op=OP.mult)
                nc.vector.tensor_tensor(out=m2[:], in0=pwf[:, 0:32],
                                        in1=pwf[:, 64:96], op=OP.mult)
                nc.vector.tensor_tensor(out=cellf, in0=m1[:], in1=m2[:],
                                        op=OP.add)
                tc_ = scsb.tile([128, 32], F32, tag="tc")
                nc.scalar.activation(tc_[:], cellf, AF.Tanh)
                nc.vector.tensor_tensor(out=hidf, in0=pwf[:, 96:128],
                                        in1=tc_[:], op=OP.mult)
                nc.vector.tensor_copy(xT16[:, 4:8, :], hidT_sb[:])
                nc.vector.tensor_copy(hidT_r[:, :, 0:BL], hidT_sb[:])
                nc.vector.tensor_copy(outsT16[:, 0:4, t, :], hidT_sb[:])

        # ======================= Phase E: logits =======================
        with tc.tile_pool(name="lw", bufs=4) as lwp, \
             tc.tile_pool(name="lo", bufs=4) as lop, \
             tc.tile_pool(name="lps", bufs=4, space="PSUM") as lps:
            for mt in range(VT):
                wt = lwp.tile([128, 10, 128], F16, tag="wt")
                nc.sync.dma_start(
                    wt[:],
                    d_WoT.ap()[:, mt * 128:(mt + 1) * 128]
                    .rearrange("(kc p) m -> p kc m", p=128))
                ps_l = lps.tile([128, 256], F32, tag="l")
                for kc in range(10):
                    nc.tensor.matmul(
                        ps_l[:, 0:TB], wt[:, kc, :],
                        outsT16[:, kc, :, :],
                        start=(kc == 0), stop=(kc == 9))
                lo = lop.tile([128, TB], F32, tag="lo")
                if mt % 2 == 0:
                    nc.vector.tensor_scalar_add(lo[:], ps_l[:, 0:TB],
                                                boT_sb[:, mt:mt + 1])
                else:
                    nc.scalar.activation(lo[:], ps_l[:, 0:TB], AF.Identity,
                                         bias=boT_sb[:, mt:mt + 1])
                nc.sync.dma_start(d_logitsT.ap()[mt, :, :], lo[:])

    nc.compile()
    return nc


def _prep_inputs(image_features, captions_ix, W_h0, b_h0, W_c0, b_c0, emb,
                 W_key, b_key, W_ih, b_ih, W_hh, b_hh, W_out, b_out,
                 t_steps=T):
    """Host-side sharding/layout. Returns list of per-core in_maps."""
    f32 = np.float32
    f16 = np.float16
    img = np.ascontiguousarray(np.asarray(image_features, dtype=f32))
    cap = np.asarray(captions_ix).astype(np.int32)[:, :t_steps]

    WkT = np.ascontiguousarray(np.asarray(W_key, f32).T)
    bkT = np.ascontiguousarray(np.asarray(b_key, f32).reshape(4, 128).T)
    Wh0T = np.ascontiguousarray(np.asarray(W_h0, f32).T / float(HW))
    bh0T = np.ascontiguousarray(np.asarray(b_h0, f32).reshape(4, 128).T)
    Wc0T = np.ascontiguousarray(np.asarray(W_c0, f32).T / float(HW))
    bc0T = np.ascontiguousarray(np.asarray(b_c0, f32).reshape(4, 128).T)
    W_ih = np.asarray(W_ih, f32)
    W_hh = np.asarray(W_hh, f32)
    WieT = np.ascontiguousarray(W_ih[:, :E].T.astype(f16))
    WahT = np.ascontiguousarray(
        np.concatenate([W_ih[:, E:].T, np.asarray(W_hh, f32).T], axis=0)
        .astype(f16))
    bgT = np.ascontiguousarray(
        (np.asarray(b_ih, f32) + np.asarray(b_hh, f32)).reshape(16, 128).T)
    WoT = np.zeros((OD, VP), f16)
    WoT[:, :V] = np.asarray(W_out, f32).T.astype(f16)
    boT = np.zeros((VP,), f32)
    boT[:V] = np.asarray(b_out, f32)
    boT = np.ascontiguousarray(boT.reshape(VT, 128).T)
    embf = np.ascontiguousarray(np.asarray(emb, f32))

    in_maps = []
    for ci in range(NCORES):
        sl = slice(ci * BL, (ci + 1) * BL)
        img_l = img[sl]                                   # [BL, C, HW]
        featsT = np.ascontiguousarray(
            img_l.transpose(1, 0, 2).reshape(C, BL * HW))
        fp = np.zeros((BL, KP, C), f32)
        fp[:, :HW, :] = img_l.transpose(0, 2, 1)
        feats16 = np.ascontiguousarray(fp.reshape(BL * KP, C).astype(f16))
        capidx = np.ascontiguousarray(
            cap[sl].T.reshape(t_steps * BL, 1))           # row r = t*BL+b
        in_maps.append({
            "featsT": featsT, "feats16": feats16, "emb": embf,
            "capidx": capidx, "WkT": WkT, "bkT": bkT,
            "Wh0T": Wh0T, "bh0T": bh0T, "Wc0T": Wc0T, "bc0T": bc0T,
            "WieT": WieT, "WahT": WahT, "bgT": bgT,
            "WoT": WoT, "boT": boT,
        })
    return in_maps


def _assemble(results, t_steps=T):
    logits = np.empty((B, t_steps, V), np.float32)
    attn = np.empty((B, t_steps, HW), np.float32)
    for ci, r in enumerate(results):
        lt = r["logitsT"].reshape(VP, t_steps * BL)[:V]   # [V, (t,b)]
        logits[ci * BL:(ci + 1) * BL] = (
            lt.reshape(V, t_steps, BL).transpose(2, 1, 0))
        attn[ci * BL:(ci + 1) * BL] = r["attn"]
    return logits, attn


def kernel(**inputs):
    if "nc" not in _CACHE:
        _CACHE["nc"] = _build(T)
    nc = _CACHE["nc"]
    in_maps = _prep_inputs(**inputs)
    res = run_bass_kernel_spmd(nc, in_maps, core_ids=list(range(NCORES)))
    return _assemble(res.results)


# revision 24
# speedup vs baseline: 1.0162x; 1.0162x over previous
"""CaptionNet Trainium2 kernel (Bass/Tile, 8-core SPMD, batch-sharded).

Strategy:
- Batch-parallel over 8 NeuronCores (8 batches/core). No collectives.
- LSTM state kept transposed ([feature-part, batch-free]) so pointwise ops
  use all 128 lanes and the recurrent matmuls run weight-stationary.
- Attention scores via 4-way column-tiled PE matmuls (M=1 per batch).
- Matmuls run float32r (fp32 data, ~1e-4 error, 4x faster than fp32) except
  the weight-heavy gates / attention-value / logits matmuls which use fp16
  weights+activations (~5e-4) to halve PE ingest bytes.
- Softmax: exp(scale*x) with fused accumulate, no max-subtraction (scores
  are small by construction), normalization folded in as per-row scalars.
"""
import numpy as np
import ml_dtypes

import concourse.bass as bass
import concourse.tile as tile
from concourse import bacc, mybir
from concourse.bass_utils import run_bass_kernel_spmd
from concourse.masks import make_identity
from contextlib import ExitStack

F32 = mybir.dt.float32
F32R = mybir.dt.float32r
F16 = mybir.dt.float16
I32 = mybir.dt.int32
AF = mybir.ActivationFunctionType
OP = mybir.AluOpType

B, T, C, HW, V, E, U = 64, 20, 512, 196, 10000, 256, 512
NCORES = 8
BL = B // NCORES          # 8 batches per core
KP = 256                  # per-batch padded attention length (HW=196 -> 256)
G4 = 4 * U                # 2048 gate width
OD = U + C + E            # 1280 outs feature dim
VT = (V + 127) // 128     # 79 vocab M-tiles
VP = VT * 128             # 10112 padded vocab
INV_SCALE = 1.0 / float(U) ** 0.5

_CACHE = {}


def _build(t_steps=T):
    """Build + compile the per-core Bass program. Returns nc."""
    nc = bacc.Bacc("TRN2", target_bir_lowering=False, debug=False,
                   num_devices=NCORES)
    TB = t_steps * BL

    # ---- DRAM I/O ----
    d_featsT = nc.dram_tensor("featsT", [C, BL * HW], F16, kind="ExternalInput")
    d_feats16 = nc.dram_tensor("feats16", [BL * KP, C], F16, kind="ExternalInput")
    d_emb = nc.dram_tensor("emb", [TB, E], F32, kind="ExternalInput")
    d_WkT = nc.dram_tensor("WkT", [C, U], F16, kind="ExternalInput")
    d_bkT = nc.dram_tensor("bkT", [128, 4], F32, kind="ExternalInput")
    d_Wh0T = nc.dram_tensor("Wh0T", [C, U], F32R, kind="ExternalInput")
    d_bh0T = nc.dram_tensor("bh0T", [128, 4], F32, kind="ExternalInput")
    d_Wc0T = nc.dram_tensor("Wc0T", [C, U], F32R, kind="ExternalInput")
    d_bc0T = nc.dram_tensor("bc0T", [128, 4], F32, kind="ExternalInput")
    d_WieT = nc.dram_tensor("WieT", [E, G4], F16, kind="ExternalInput")
    d_WahT = nc.dram_tensor("WahT", [C + U, G4], F16, kind="ExternalInput")
    d_bgT = nc.dram_tensor("bgT", [128, 16], F32, kind="ExternalInput")
    d_WoT = nc.dram_tensor("WoT", [OD, VP], F16, kind="ExternalInput")
    d_boT = nc.dram_tensor("boT", [128, VT], F32, kind="ExternalInput")

    d_logitsT = nc.dram_tensor("logitsT", [VT, 128, TB], F32, kind="ExternalOutput")
    d_attn = nc.dram_tensor("attn", [BL, t_steps, HW], F32, kind="ExternalOutput")

    with tile.TileContext(nc) as tc, ExitStack() as ctx:
        ctx.enter_context(nc.allow_low_precision(
            reason="float32r stores feed f32r matmuls by design"))
        per = ctx.enter_context(tc.tile_pool(name="per", bufs=1))

        # ---------- persistent SBUF ----------
        ident = per.tile([128, 128], F32)
        make_identity(nc, ident[:])

        featsT_sb = per.tile([128, 4, BL * HW], F16)
        for _cc in range(4):
            nc.sync.dma_start(
                featsT_sb[:, _cc, :],
                d_featsT.ap().rearrange("(cc p) n -> p cc n", p=128)[:, _cc, :])
        feats16_sb = per.tile([128, 2 * BL, C], F16)
        nc.sync.dma_start(feats16_sb[:],
                          d_feats16.ap().rearrange("(bh p) c -> p bh c", p=128))
        WkT_sb = per.tile([128, 4, U], F16)
        nc.sync.dma_start(WkT_sb[:],
                          d_WkT.ap().rearrange("(cc p) u -> p cc u", p=128))
        Wh0T_sb = per.tile([128, 4, U], F32R)
        nc.sync.dma_start(Wh0T_sb[:],
                          d_Wh0T.ap().rearrange("(cc p) u -> p cc u", p=128))
        Wc0T_sb = per.tile([128, 4, U], F32R)
        nc.sync.dma_start(Wc0T_sb[:],
                          d_Wc0T.ap().rearrange("(cc p) u -> p cc u", p=128))
        WieT_sb = per.tile([128, 2, G4], F16)
        nc.sync.dma_start(WieT_sb[:],
                          d_WieT.ap().rearrange("(ee p) j -> p ee j", p=128))
        WahT_sb = per.tile([128, 8, G4], F16)
        nc.sync.dma_start(WahT_sb[:],
                          d_WahT.ap().rearrange("(kc p) j -> p kc j", p=128))
        bkT_sb = per.tile([128, 4], F32)
        nc.sync.dma_start(bkT_sb[:], d_bkT.ap())
        bh0T_sb = per.tile([128, 4], F32)
        nc.sync.dma_start(bh0T_sb[:], d_bh0T.ap())
        bc0T_sb = per.tile([128, 4], F32)
        nc.sync.dma_start(bc0T_sb[:], d_bc0T.ap())
        bgT_sb = per.tile([128, 16], F32)
        nc.sync.dma_start(bgT_sb[:], d_bgT.ap())
        boT_sb = per.tile([128, VT], F32)
        nc.sync.dma_start(boT_sb[:], d_boT.ap())

        keysT_sb = per.tile([128, 4, BL, KP], F16)   # padded [u, b, k]
        ge_sb = per.tile([128, 16, TB], F32)         # emb-part of gates + bias
        embT_sb = per.tile([128, 2, TB], F32)
        outsT16 = per.tile([128, 10, t_steps, BL], F16)

        hidT_sb = per.tile([128, 4, BL], F32)
        # 32-col padded so col-tiled score matmuls can write a full 32-row
        # block (cols 8..39 stay zero; junk rows land on unused partitions)
        hidT_r = per.tile([128, 4, BL + 32], F16)
        cellT_sb = per.tile([128, 4, BL], F32)
        xT16 = per.tile([128, 8, BL], F16)           # kc 0-3 = aT, 4-7 = hidT

        # =========== Phase A: embedding gather + renorm + transpose =========
        with tc.tile_pool(name="embp", bufs=1) as embp, \
             tc.tile_pool(name="embps", bufs=1, space="PSUM") as embps:
            segs = [(0, min(128, TB))]
            if TB > 128:
                segs.append((128, TB))
            ps_e = embps.tile([128, 4, 128], F32)
            for si, (r0, r1) in enumerate(segs):
                npart, off = r1 - r0, r0
                g = embp.tile([npart, E], F32, tag=f"eg{si}")
                nc.sync.dma_start(g[:], d_emb.ap()[r0:r1, :])
                sq = embp.tile([npart, E], F32, tag=f"sq{si}")
                n2 = embp.tile([npart, 1], F32, tag=f"n2{si}")
                nc.scalar.activation(sq[:], g[:], AF.Square,
                                     accum_out=n2[:, :1])
                nrm = embp.tile([npart, 1], F32, tag=f"nr{si}")
                nc.scalar.sqrt(nrm[:], n2[:])
                nc.vector.tensor_scalar_max(nrm[:], nrm[:], 1e-12)
                inv = embp.tile([npart, 1], F32, tag=f"iv{si}")
                nc.vector.reciprocal(inv[:], nrm[:])
                nc.vector.tensor_scalar_mul(inv[:], inv[:], 5.0)
                nc.vector.tensor_scalar_min(inv[:], inv[:], 1.0)
                nc.vector.tensor_scalar_mul(g[:], g[:], inv[:, :1])
                # transpose [npart, 256] -> embT [256, npart]
                for ee in range(2):
                    nc.tensor.transpose(
                        ps_e[:, 2 * si + ee, 0:npart],
                        g[:, ee * 128:(ee + 1) * 128],
                        ident[0:npart, 0:npart])
                    nc.vector.tensor_copy(
                        embT_sb[:, ee, off:off + npart],
                        ps_e[:, 2 * si + ee, 0:npart])
            # fp16 copy into outsT16 emb rows (kc 8,9)
            nc.vector.tensor_copy(
                outsT16[:, 8:10, :, :],
                embT_sb[:].rearrange("p ee (t b) -> p ee t b", b=BL))
            emb16 = embp.tile([128, 2, TB], F16)
            nc.vector.tensor_copy(emb16[:], embT_sb[:])

            # ======= Phase B: gates_e = W_ihe @ embT (+bias), fp16 =======
            with tc.tile_pool(name="geps", bufs=1, space="PSUM") as geps:
                for half in range(2):
                    ps_ge = geps.tile([128, 8, 256], F32, tag="ge")
                    for jt8 in range(8):
                        jt = half * 8 + jt8
                        for ee in range(2):
                            nc.tensor.matmul(
                                ps_ge[:, jt8, 0:TB],
                                WieT_sb[:, ee, jt * 128:(jt + 1) * 128],
                                emb16[:, ee, :],
                                start=(ee == 0), stop=(ee == 1))
                    for jt8 in range(8):
                        jt = half * 8 + jt8
                        eng = nc.vector if jt8 % 2 == 0 else nc.scalar
                        if eng is nc.vector:
                            nc.vector.tensor_scalar_add(
                                ge_sb[:, jt, :], ps_ge[:, jt8, 0:TB],
                                bgT_sb[:, jt:jt + 1])
                        else:
                            nc.scalar.activation(
                                ge_sb[:, jt, :], ps_ge[:, jt8, 0:TB],
                                AF.Identity, bias=bgT_sb[:, jt:jt + 1])

        # =========== Phase C: feats_mean + keys + initial state ===========
        with tc.tile_pool(name="inip", bufs=1) as inip, \
             tc.tile_pool(name="inips", bufs=1, space="PSUM") as inips, \
             tc.tile_pool(name="keyps", bufs=2, space="PSUM") as keyps:
            fsum = inip.tile([128, 4, BL], F32R)
            for cc in range(4):
                nc.vector.tensor_reduce(
                    fsum[:, cc, :],
                    featsT_sb[:, cc, :].rearrange("p (b k) -> p b k", b=BL),
                    axis=mybir.AxisListType.X, op=OP.add)
            # keys: out keysT [u-tile, (b,k)] ; evac into padded [u, b, KP]
            HHW = 4 * HW  # half the (b,k) range = 784
            for jt in range(4):
                for half in range(2):
                    ps_k = keyps.tile([128, HHW], F32, tag="k")
                    for cc in range(4):
                        for (n0, n1) in [(0, 512), (512, HHW)]:
                            nc.tensor.matmul(
                                ps_k[:, n0:n1],
                                WkT_sb[:, cc, jt * 128:(jt + 1) * 128],
                                featsT_sb[:, cc,
                                          half * HHW + n0:half * HHW + n1],
                                start=(cc == 0), stop=(cc == 3))
                    out_ap = keysT_sb[:, jt, half * 4:(half + 1) * 4, 0:HW]
                    in_ap = ps_k[:].rearrange("p (b k) -> p b k", b=4)
                    if (2 * jt + half) % 2 == 0:
                        nc.vector.tensor_scalar_add(out_ap, in_ap,
                                                    bkT_sb[:, jt:jt + 1])
                    else:
                        nc.scalar.activation(out_ap, in_ap, AF.Identity,
                                             bias=bkT_sb[:, jt:jt + 1])
                    # zero the k-padding (196..256) with a rounding store
                    nc.vector.tensor_scalar_mul(
                        keysT_sb[:, jt, half * 4:(half + 1) * 4, HW:KP],
                        ps_k[:, 0:4 * (KP - HW)]
                        .rearrange("p (b k) -> p b k", b=4),
                        0.0)
            # hid0 / cell0 (M-orientation, f32r)
            ps_i = inips.tile([128, 4, 128], F32, tag="i0")
            ps_c = inips.tile([128, 4, 128], F32, tag="c0")
            for jt in range(4):
                for cc in range(4):
                    nc.tensor.matmul(
                        ps_i[:, jt, 0:BL],
                        Wh0T_sb[:, cc, jt * 128:(jt + 1) * 128],
                        fsum[:, cc, :],
                        start=(cc == 0), stop=(cc == 3))
                    nc.tensor.matmul(
                        ps_c[:, jt, 0:BL],
                        Wc0T_sb[:, cc, jt * 128:(jt + 1) * 128],
                        fsum[:, cc, :],
                        start=(cc == 0), stop=(cc == 3))
            for jt in range(4):
                nc.vector.tensor_scalar_add(hidT_sb[:, jt, :],
                                            ps_i[:, jt, 0:BL],
                                            bh0T_sb[:, jt:jt + 1])
                nc.vector.tensor_scalar_add(cellT_sb[:, jt, :],
                                            ps_c[:, jt, 0:BL],
                                            bc0T_sb[:, jt:jt + 1])
            nc.vector.tensor_copy(xT16[:, 4:8, :], hidT_sb[:])
            nc.vector.tensor_copy(hidT_r[:, :, 0:BL], hidT_sb[:])
            nc.vector.tensor_scalar_mul(
                hidT_r[:, :, BL:], bh0T_sb[:].unsqueeze(-1)
                .to_broadcast([128, 4, BL + 32 - BL]), 0.0)

        # ======================= Phase D: the scan =======================
        with tc.tile_pool(name="scps", bufs=1, space="PSUM") as scps, \
             tc.tile_pool(name="scsb", bufs=2) as scsb:
            ps_sc2 = [scps.tile([128, 2, KP], F32, tag=f"sc{i}", name=f"ps_sc{i}")
                      for i in range(2)]
            nc.vector.memset(ps_sc2[0][:], 0.0)
            nc.vector.memset(ps_sc2[1][:], 0.0)
            ps_tr = scps.tile([128, 4, 128], F32)
            ps_aT = scps.tile([128, 4, BL], F32)
            ps_g2 = [scps.tile([128, 2, 16, BL], F32, tag=f"g{i}",
                               name=f"ps_g{i}") for i in range(2)]
            w_sb2 = [scsb.tile([128, 2, KP], F32, tag=f"w{i}", name=f"w_sb{i}")
                     for i in range(2)]
            nc.gpsimd.memset(w_sb2[0][:], 0.0)
            nc.gpsimd.memset(w_sb2[1][:], 0.0)

            for t in range(t_steps):
                ps_sc = ps_sc2[t % 2]
                ps_g = ps_g2[t % 2]

                # 1. gates hid-part (can overlap with attention)
                for jt in range(16):
                    for kc in range(4, 8):
                        nc.tensor.matmul(
                            ps_g[:, 0, jt, :],
                            WahT_sb[:, kc, jt * 128:(jt + 1) * 128],
                            xT16[:, kc, :],
                            start=(kc == 4), stop=(kc == 7))

                # 2. scores (col-tiled, f32r): batch b=g*4+c -> psum part 32c
                for g in range(2):
                    for c in range(4):
                        b = g * 4 + c
                        for jj in range(4):
                            nc.tensor.matmul(
                                ps_sc[32 * c:32 * c + 32, g, :],
                                hidT_r[:, jj, b:b + 32],
                                keysT_sb[:, jj, b, :],
                                start=(jj == 0), stop=(jj == 3),
                                tile_position=(0, 32 * c))

                # 3-5. softmax (no max-subtract; normalize via row scalars)
                w_sb = w_sb2[t % 2]
                sume = scsb.tile([128, 2], F32, tag="sume")
                rinv = scsb.tile([128, 2], F32, tag="rinv")
                for g in range(2):
                    nc.scalar.activation(
                        w_sb[:, g, 0:HW], ps_sc[:, g, 0:HW], AF.Exp,
                        scale=INV_SCALE, accum_out=sume[:, g:g + 1])
                nc.vector.reciprocal(rinv[:], sume[:])
                for g in range(2):
                    nc.vector.tensor_scalar_mul(
                        w_sb[:, g, 0:HW], w_sb[:, g, 0:HW], rinv[:, g:g + 1])
                # attn output rows b=g*4+c live at partition 32c
                for g in range(2):
                    nc.gpsimd.dma_start(
                        d_attn.ap()[g * 4:(g + 1) * 4, t, :],
                        w_sb[0:128:32, g, 0:HW])

                # 6. transpose w -> wT (cols = batches at 32c)
                for hh in range(4):
                    g, h = hh // 2, hh % 2
                    nc.tensor.transpose(
                        ps_tr[:, hh, :],
                        w_sb[:, g, h * 128:(h + 1) * 128],
                        ident[:])
                eT16 = scsb.tile([128, 4, 4], F16, tag="eT")
                for hh in range(4):
                    nc.vector.tensor_copy(
                        eT16[:, hh, :], ps_tr[:, hh, 0:128:32])

                # 7. attention output aT[c,b] (fp16 feats as weights)
                for b in range(BL):
                    g, c = b // 4, b % 4
                    for cm in range(4):
                        for h in range(2):
                            nc.tensor.matmul(
                                ps_aT[:, cm, b:b + 1],
                                feats16_sb[:, b * 2 + h,
                                           cm * 128:(cm + 1) * 128],
                                eT16[:, g * 2 + h, c:c + 1],
                                start=(h == 0), stop=(h == 1))
                nc.vector.tensor_copy(xT16[:, 0:4, :], ps_aT[:])
                nc.vector.tensor_copy(outsT16[:, 4:8, t, :], ps_aT[:])

                # 8. gates a-part
                for jt in range(16):
                    for kc in range(0, 4):
                        nc.tensor.matmul(
                            ps_g[:, 1, jt, :],
                            WahT_sb[:, kc, jt * 128:(jt + 1) * 128],
                            xT16[:, kc, :],
                            start=(kc == 0), stop=(kc == 3))

                # 9. add hid-part + emb-part(with bias) + a-part
                gf = scsb.tile([128, 16, BL], F32, tag="gf")
                nc.vector.tensor_tensor(
                    out=gf[:], in0=ps_g[:, 0, :, :],
                    in1=ge_sb[:, :, t * BL:(t + 1) * BL], op=OP.add)
                nc.vector.tensor_tensor(
                    out=gf[:], in0=ps_g[:, 1, :, :], in1=gf[:], op=OP.add)

                # 10. LSTM pointwise on [128, (16jj, 8b)]
                gff = gf[:].rearrange("p jj b -> p (jj b)")
                pw = scsb.tile([128, 128], F32, tag="pw")
                pwf = pw[:]
                nc.scalar.activation(pwf[:, 0:64], gff[:, 0:64], AF.Sigmoid)
                nc.scalar.activation(pwf[:, 64:96], gff[:, 64:96], AF.Tanh)
                nc.scalar.activation(pwf[:, 96:128], gff[:, 96:128], AF.Sigmoid)
                m1 = scsb.tile([128, 32], F32, tag="m1")
                m2 = scsb.tile([128, 32], F32, tag="m2")
                cellf = cellT_sb[:].rearrange("p jj b -> p (jj b)")
                hidf = hidT_sb[:].rearrange("p jj b -> p (jj b)")
                nc.vector.tensor_tensor(out=m1[:], in0=pwf[:, 32:64],
                                        in1=cellf, op=OP.mult)
                nc.vector.tensor_tensor(out=m2[:], in0=pwf[:, 0:32],
                                        in1=pwf[:, 64:96], op=OP.mult)
                nc.vector.tensor_tensor(out=cellf, in0=m1[:], in1=m2[:],
                                        op=OP.add)
                tc_ = scsb.tile([128, 32], F32, tag="tc")
                nc.scalar.activation(tc_[:], cellf, AF.Tanh)
                nc.vector.tensor_tensor(out=hidf, in0=pwf[:, 96:128],
                                        in1=tc_[:], op=OP.mult)
                nc.vector.tensor_copy(xT16[:, 4:8, :], hidT_sb[:])
                nc.vector.tensor_copy(hidT_r[:, :, 0:BL], hidT_sb[:])
                nc.vector.tensor_copy(outsT16[:, 0:4, t, :], hidT_sb[:])

        # ======================= Phase E: logits =======================
        with tc.tile_pool(name="lw", bufs=24) as lwp, \
             tc.tile_pool(name="lo", bufs=4) as lop, \
             tc.tile_pool(name="lps", bufs=4, space="PSUM") as lps:
            for mt in range(VT):
                wt = lwp.tile([128, 10, 128], F16, tag="wt")
                nc.sync.dma_start(
                    wt[:],
                    d_WoT.ap()[:, mt * 128:(mt + 1) * 128]
                    .rearrange("(kc p) m -> p kc m", p=128))
                ps_l = lps.tile([128, 256], F32, tag="l")
                for kc in range(10):
                    nc.tensor.matmul(
                        ps_l[:, 0:TB], wt[:, kc, :],
                        outsT16[:, kc, :, :],
                        start=(kc == 0), stop=(kc == 9))
                lo = lop.tile([128, TB], F32, tag="lo")
                if mt % 2 == 0:
                    nc.vector.tensor_scalar_add(lo[:], ps_l[:, 0:TB],
                                                boT_sb[:, mt:mt + 1])
                else:
                    nc.scalar.activation(lo[:], ps_l[:, 0:TB], AF.Identity,
                                         bias=boT_sb[:, mt:mt + 1])
                nc.sync.dma_start(d_logitsT.ap()[mt, :, :], lo[:])

    nc.compile()
    return nc


def _prep_inputs(image_features, captions_ix, W_h0, b_h0, W_c0, b_c0, emb,
                 W_key, b_key, W_ih, b_ih, W_hh, b_hh, W_out, b_out,
                 t_steps=T):
    """Host-side sharding/layout. Returns list of per-core in_maps."""
    f32 = np.float32
    f16 = np.float16
    img = np.ascontiguousarray(np.asarray(image_features, dtype=f32))
    cap = np.asarray(captions_ix).astype(np.int32)[:, :t_steps]

    WkT = np.ascontiguousarray(np.asarray(W_key, f32).T.astype(f16))
    bkT = np.ascontiguousarray(np.asarray(b_key, f32).reshape(4, 128).T)
    Wh0T = np.ascontiguousarray(np.asarray(W_h0, f32).T / float(HW))
    bh0T = np.ascontiguousarray(np.asarray(b_h0, f32).reshape(4, 128).T)
    Wc0T = np.ascontiguousarray(np.asarray(W_c0, f32).T / float(HW))
    bc0T = np.ascontiguousarray(np.asarray(b_c0, f32).reshape(4, 128).T)
    W_ih = np.asarray(W_ih, f32)
    W_hh = np.asarray(W_hh, f32)
    WieT = np.ascontiguousarray(W_ih[:, :E].T.astype(f16))
    WahT = np.ascontiguousarray(
        np.concatenate([W_ih[:, E:].T, np.asarray(W_hh, f32).T], axis=0)
        .astype(f16))
    bgT = np.ascontiguousarray(
        (np.asarray(b_ih, f32) + np.asarray(b_hh, f32)).reshape(16, 128).T)
    WoT = np.zeros((OD, VP), f16)
    WoT[:, :V] = np.asarray(W_out, f32).T.astype(f16)
    boT = np.zeros((VP,), f32)
    boT[:V] = np.asarray(b_out, f32)
    boT = np.ascontiguousarray(boT.reshape(VT, 128).T)
    embf = np.ascontiguousarray(np.asarray(emb, f32))

    in_maps = []
    for ci in range(NCORES):
        sl = slice(ci * BL, (ci + 1) * BL)
        img_l = img[sl]                                   # [BL, C, HW]
        featsT = np.ascontiguousarray(
            img_l.transpose(1, 0, 2).reshape(C, BL * HW).astype(f16))
        fp = np.zeros((BL, KP, C), f32)
        fp[:, :HW, :] = img_l.transpose(0, 2, 1)
        feats16 = np.ascontiguousarray(fp.reshape(BL * KP, C).astype(f16))
        emb_rows = np.ascontiguousarray(
            embf[cap[sl].T.reshape(t_steps * BL)])        # row r = t*BL+b
        in_maps.append({
            "featsT": featsT, "feats16": feats16, "emb": emb_rows,
            "WkT": WkT, "bkT": bkT,
            "Wh0T": Wh0T, "bh0T": bh0T, "Wc0T": Wc0T, "bc0T": bc0T,
            "WieT": WieT, "WahT": WahT, "bgT": bgT,
            "WoT": WoT, "boT": boT,
        })
    return in_maps


def _assemble(results, t_steps=T):
    logits = np.empty((B, t_steps, V), np.float32)
    attn = np.empty((B, t_steps, HW), np.float32)
    for ci, r in enumerate(results):
        lt = r["logitsT"].reshape(VP, t_steps * BL)[:V]   # [V, (t,b)]
        logits[ci * BL:(ci + 1) * BL] = (
            lt.reshape(V, t_steps, BL).transpose(2, 1, 0))
        attn[ci * BL:(ci + 1) * BL] = r["attn"]
    return logits, attn


def kernel(**inputs):
    if "nc" not in _CACHE:
        _CACHE["nc"] = _build(T)
    nc = _CACHE["nc"]
    in_maps = _prep_inputs(**inputs)
    res = run_bass_kernel_spmd(nc, in_maps, core_ids=list(range(NCORES)))
    return _assemble(res.results)


# revision 32
# speedup vs baseline: 1.3828x; 1.3607x over previous
"""CaptionNet Trainium2 kernel (Bass/Tile, 8-core SPMD, batch-sharded).

Strategy:
- Batch-parallel over 8 NeuronCores (8 batches/core). No collectives.
- LSTM state kept transposed ([feature-part, batch-free]) so pointwise ops
  use all 128 lanes and the recurrent matmuls run weight-stationary.
- Attention scores via 4-way column-tiled PE matmuls (M=1 per batch).
- Matmuls run float32r (fp32 data, ~1e-4 error, 4x faster than fp32) except
  the weight-heavy gates / attention-value / logits matmuls which use fp16
  weights+activations (~5e-4) to halve PE ingest bytes.
- Softmax: exp(scale*x) with fused accumulate, no max-subtraction (scores
  are small by construction), normalization folded in as per-row scalars.
"""
import numpy as np
import ml_dtypes

import concourse.bass as bass
import concourse.tile as tile
from concourse import bacc, mybir
from concourse.bass_utils import run_bass_kernel_spmd
from concourse.masks import make_identity
from contextlib import ExitStack

F32 = mybir.dt.float32
F32R = mybir.dt.float32r
F16 = mybir.dt.float16
I32 = mybir.dt.int32
AF = mybir.ActivationFunctionType
OP = mybir.AluOpType

B, T, C, HW, V, E, U = 64, 20, 512, 196, 10000, 256, 512
NCORES = 8
BL = B // NCORES          # 8 batches per core
KP = 256                  # per-batch padded attention length (HW=196 -> 256)
G4 = 4 * U                # 2048 gate width
OD = U + C + E            # 1280 outs feature dim
VT = (V + 127) // 128     # 79 vocab M-tiles
VP = VT * 128             # 10112 padded vocab
INV_SCALE = 1.0 / float(U) ** 0.5

_CACHE = {}


def _build(t_steps=T):
    """Build + compile the per-core Bass program. Returns nc."""
    nc = bacc.Bacc("TRN2", target_bir_lowering=False, debug=False,
                   num_devices=NCORES)
    TB = t_steps * BL

    # ---- DRAM I/O ----
    d_featsT = nc.dram_tensor("featsT", [C, BL * HW], F16, kind="ExternalInput")
    d_feats16 = nc.dram_tensor("feats16", [BL * KP, C], F16, kind="ExternalInput")
    d_emb = nc.dram_tensor("emb", [TB, E], F32, kind="ExternalInput")
    d_WkT = nc.dram_tensor("WkT", [C, U], F16, kind="ExternalInput")
    d_bkT = nc.dram_tensor("bkT", [128, 4], F32, kind="ExternalInput")
    d_Wh0T = nc.dram_tensor("Wh0T", [C, U], F32R, kind="ExternalInput")
    d_bh0T = nc.dram_tensor("bh0T", [128, 4], F32, kind="ExternalInput")
    d_Wc0T = nc.dram_tensor("Wc0T", [C, U], F32R, kind="ExternalInput")
    d_bc0T = nc.dram_tensor("bc0T", [128, 4], F32, kind="ExternalInput")
    d_WieT = nc.dram_tensor("WieT", [E, G4], F16, kind="ExternalInput")
    d_WahT = nc.dram_tensor("WahT", [C + U, G4], F16, kind="ExternalInput")
    d_bgT = nc.dram_tensor("bgT", [128, 16], F32, kind="ExternalInput")
    d_WoT = nc.dram_tensor("WoT", [VT, 128, OD], F16, kind="ExternalInput")
    d_boT = nc.dram_tensor("boT", [128, VT], F32, kind="ExternalInput")

    d_logitsT = nc.dram_tensor("logitsT", [VT, 128, TB], F32, kind="ExternalOutput")
    d_attn = nc.dram_tensor("attn", [BL, t_steps, HW], F32, kind="ExternalOutput")

    with tile.TileContext(nc) as tc, ExitStack() as ctx:
        ctx.enter_context(nc.allow_low_precision(
            reason="float32r stores feed f32r matmuls by design"))
        per = ctx.enter_context(tc.tile_pool(name="per", bufs=1))

        # ---------- persistent SBUF ----------
        ident = per.tile([128, 128], F32)
        make_identity(nc, ident[:])

        early = tc.tile_pool(name="early", bufs=1)
        earlyp = early.__enter__()
        featsT_sb = earlyp.tile([128, 4, BL * HW], F16)
        for _cc in range(4):
            nc.sync.dma_start(
                featsT_sb[:, _cc, :],
                d_featsT.ap().rearrange("(cc p) n -> p cc n", p=128)[:, _cc, :])
        feats16_sb = per.tile([128, 2 * BL, C], F16)
        nc.sync.dma_start(feats16_sb[:],
                          d_feats16.ap().rearrange("(bh p) c -> p bh c", p=128))
        WkT_sb = earlyp.tile([128, 4, U], F16)
        nc.sync.dma_start(WkT_sb[:],
                          d_WkT.ap().rearrange("(cc p) u -> p cc u", p=128))
        Wh0T_sb = earlyp.tile([128, 4, U], F32R)
        nc.sync.dma_start(Wh0T_sb[:],
                          d_Wh0T.ap().rearrange("(cc p) u -> p cc u", p=128))
        Wc0T_sb = earlyp.tile([128, 4, U], F32R)
        nc.sync.dma_start(Wc0T_sb[:],
                          d_Wc0T.ap().rearrange("(cc p) u -> p cc u", p=128))
        WieT_sb = earlyp.tile([128, 2, G4], F16)
        nc.sync.dma_start(WieT_sb[:],
                          d_WieT.ap().rearrange("(ee p) j -> p ee j", p=128))
        WahT_sb = per.tile([128, 8, G4], F16)
        nc.sync.dma_start(WahT_sb[:],
                          d_WahT.ap().rearrange("(kc p) j -> p kc j", p=128))
        bkT_sb = per.tile([128, 4], F32)
        nc.sync.dma_start(bkT_sb[:], d_bkT.ap())
        bh0T_sb = per.tile([128, 4], F32)
        nc.sync.dma_start(bh0T_sb[:], d_bh0T.ap())
        bc0T_sb = per.tile([128, 4], F32)
        nc.sync.dma_start(bc0T_sb[:], d_bc0T.ap())
        bgT_sb = per.tile([128, 16], F32)
        nc.sync.dma_start(bgT_sb[:], d_bgT.ap())
        boT_sb = per.tile([128, VT], F32)
        nc.sync.dma_start(boT_sb[:], d_boT.ap())

        keysT_sb = per.tile([128, 4, BL, KP], F16)   # padded [u, b, k]
        ge_sb = per.tile([128, 16, TB], F32)         # emb-part of gates + bias
        embT_sb = per.tile([128, 2, TB], F32)
        outsT16 = per.tile([128, 10, t_steps, BL], F16)

        hidT_sb = per.tile([128, 4, BL], F32)
        cellT_sb = per.tile([128, 4, BL], F32)
        # kc 0-3 = aT, 4-7 = hidT; 40-wide so score matmuls can take 32-col
        # lhsT slices (cols 8..39 zeroed; junk rows land on unused partitions)
        xT16 = per.tile([128, 8, BL + 32], F16)

        # =========== Phase A: embedding gather + renorm + transpose =========
        with tc.tile_pool(name="embp", bufs=1) as embp, \
             tc.tile_pool(name="embps", bufs=1, space="PSUM") as embps:
            segs = [(0, min(128, TB))]
            if TB > 128:
                segs.append((128, TB))
            ps_e = embps.tile([128, 4, 128], F32)
            for si, (r0, r1) in enumerate(segs):
                npart, off = r1 - r0, r0
                g = embp.tile([npart, E], F32, tag=f"eg{si}")
                nc.sync.dma_start(g[:], d_emb.ap()[r0:r1, :])
                sq = embp.tile([npart, E], F32, tag=f"sq{si}")
                n2 = embp.tile([npart, 1], F32, tag=f"n2{si}")
                nc.scalar.activation(sq[:], g[:], AF.Square,
                                     accum_out=n2[:, :1])
                nrm = embp.tile([npart, 1], F32, tag=f"nr{si}")
                nc.scalar.sqrt(nrm[:], n2[:])
                nc.vector.tensor_scalar_max(nrm[:], nrm[:], 1e-12)
                inv = embp.tile([npart, 1], F32, tag=f"iv{si}")
                nc.vector.reciprocal(inv[:], nrm[:])
                nc.vector.tensor_scalar_mul(inv[:], inv[:], 5.0)
                nc.vector.tensor_scalar_min(inv[:], inv[:], 1.0)
                nc.vector.tensor_scalar_mul(g[:], g[:], inv[:, :1])
                # transpose [npart, 256] -> embT [256, npart]
                for ee in range(2):
                    nc.tensor.transpose(
                        ps_e[:, 2 * si + ee, 0:npart],
                        g[:, ee * 128:(ee + 1) * 128],
                        ident[0:npart, 0:npart])
                    nc.vector.tensor_copy(
                        embT_sb[:, ee, off:off + npart],
                        ps_e[:, 2 * si + ee, 0:npart])
            # fp16 copy into outsT16 emb rows (kc 8,9)
            nc.vector.tensor_copy(
                outsT16[:, 8:10, :, :],
                embT_sb[:].rearrange("p ee (t b) -> p ee t b", b=BL))
            emb16 = embp.tile([128, 2, TB], F16)
            nc.vector.tensor_copy(emb16[:], embT_sb[:])

            # ======= Phase B: gates_e = W_ihe @ embT (+bias), fp16 =======
            with tc.tile_pool(name="geps", bufs=1, space="PSUM") as geps:
                for half in range(2):
                    ps_ge = geps.tile([128, 8, 256], F32, tag="ge")
                    for jt8 in range(8):
                        jt = half * 8 + jt8
                        for ee in range(2):
                            nc.tensor.matmul(
                                ps_ge[:, jt8, 0:TB],
                                WieT_sb[:, ee, jt * 128:(jt + 1) * 128],
                                emb16[:, ee, :],
                                start=(ee == 0), stop=(ee == 1))
                    for jt8 in range(8):
                        jt = half * 8 + jt8
                        eng = nc.vector if jt8 % 2 == 0 else nc.scalar
                        if eng is nc.vector:
                            nc.vector.tensor_scalar_add(
                                ge_sb[:, jt, :], ps_ge[:, jt8, 0:TB],
                                bgT_sb[:, jt:jt + 1])
                        else:
                            nc.scalar.activation(
                                ge_sb[:, jt, :], ps_ge[:, jt8, 0:TB],
                                AF.Identity, bias=bgT_sb[:, jt:jt + 1])

        # =========== Phase C: feats_mean + keys + initial state ===========
        with tc.tile_pool(name="inip", bufs=1) as inip, \
             tc.tile_pool(name="inips", bufs=1, space="PSUM") as inips, \
             tc.tile_pool(name="keyps", bufs=2, space="PSUM") as keyps:
            fsum = inip.tile([128, 4, BL], F32R)
            for cc in range(4):
                nc.vector.tensor_reduce(
                    fsum[:, cc, :],
                    featsT_sb[:, cc, :].rearrange("p (b k) -> p b k", b=BL),
                    axis=mybir.AxisListType.X, op=OP.add)
            # keys: out keysT [u-tile, (b,k)] ; evac into padded [u, b, KP]
            HHW = 4 * HW  # half the (b,k) range = 784
            for jt in range(4):
                for half in range(2):
                    ps_k = keyps.tile([128, HHW], F32, tag="k")
                    for cc in range(4):
                        for (n0, n1) in [(0, 512), (512, HHW)]:
                            nc.tensor.matmul(
                                ps_k[:, n0:n1],
                                WkT_sb[:, cc, jt * 128:(jt + 1) * 128],
                                featsT_sb[:, cc,
                                          half * HHW + n0:half * HHW + n1],
                                start=(cc == 0), stop=(cc == 3))
                    out_ap = keysT_sb[:, jt, half * 4:(half + 1) * 4, 0:HW]
                    in_ap = ps_k[:].rearrange("p (b k) -> p b k", b=4)
                    if (2 * jt + half) % 2 == 0:
                        nc.vector.tensor_scalar_add(out_ap, in_ap,
                                                    bkT_sb[:, jt:jt + 1])
                    else:
                        nc.scalar.activation(out_ap, in_ap, AF.Identity,
                                             bias=bkT_sb[:, jt:jt + 1])
                    # zero the k-padding (196..256) with a rounding store
                    nc.vector.tensor_scalar_mul(
                        keysT_sb[:, jt, half * 4:(half + 1) * 4, HW:KP],
                        ps_k[:, 0:4 * (KP - HW)]
                        .rearrange("p (b k) -> p b k", b=4),
                        0.0)
            # hid0 / cell0 (M-orientation, f32r)
            ps_i = inips.tile([128, 4, 128], F32, tag="i0")
            ps_c = inips.tile([128, 4, 128], F32, tag="c0")
            for jt in range(4):
                for cc in range(4):
                    nc.tensor.matmul(
                        ps_i[:, jt, 0:BL],
                        Wh0T_sb[:, cc, jt * 128:(jt + 1) * 128],
                        fsum[:, cc, :],
                        start=(cc == 0), stop=(cc == 3))
                    nc.tensor.matmul(
                        ps_c[:, jt, 0:BL],
                        Wc0T_sb[:, cc, jt * 128:(jt + 1) * 128],
                        fsum[:, cc, :],
                        start=(cc == 0), stop=(cc == 3))
            for jt in range(4):
                nc.vector.tensor_scalar_add(hidT_sb[:, jt, :],
                                            ps_i[:, jt, 0:BL],
                                            bh0T_sb[:, jt:jt + 1])
                nc.vector.tensor_scalar_add(cellT_sb[:, jt, :],
                                            ps_c[:, jt, 0:BL],
                                            bc0T_sb[:, jt:jt + 1])
            nc.vector.tensor_copy(xT16[:, 4:8, 0:BL], hidT_sb[:])
            nc.vector.tensor_scalar_mul(
                xT16[:, :, BL:], bh0T_sb[:, 0:1].unsqueeze(-1)
                .to_broadcast([128, 8, 32]), 0.0)

        early.__exit__(None, None, None)

        # prefetch all logits weight tiles (sync queue stays unblocked;
        # loads beyond the buffer count trickle in as logits consumes)
        lwp = ctx.enter_context(tc.tile_pool(name="lw", bufs=42))
        wtiles = []
        for mt in range(VT):
            wt = lwp.tile([128, 10, 128], F16, tag="wt", name=f"wt{mt}")
            nc.sync.dma_start(
                wt[:].rearrange("p kc m -> p (kc m)"), d_WoT.ap()[mt, :, :])
            wtiles.append(wt)

        # ======================= Phase D: the scan =======================
        with tc.tile_pool(name="scps", bufs=1, space="PSUM") as scps, \
             tc.tile_pool(name="scsb", bufs=2) as scsb:
            ps_sc2 = [scps.tile([128, 2, 512], F32, tag=f"sc{i}", name=f"ps_sc{i}")
                      for i in range(2)]
            nc.vector.memset(ps_sc2[0][:], 0.0)
            nc.vector.memset(ps_sc2[1][:], 0.0)
            ps_tr = scps.tile([128, 4, 128], F32)
            ps_aT = scps.tile([128, 4, BL], F32)
            ps_g2 = [scps.tile([128, 2, 16, BL], F32, tag=f"g{i}",
                               name=f"ps_g{i}") for i in range(2)]
            w_sb2 = [scsb.tile([128, 2, KP], F32, tag=f"w{i}", name=f"w_sb{i}")
                     for i in range(2)]
            nc.gpsimd.memset(w_sb2[0][:], 0.0)
            nc.gpsimd.memset(w_sb2[1][:], 0.0)

            for t in range(t_steps):
                ps_sc = ps_sc2[t % 2]
                ps_g = ps_g2[t % 2]

                # 1. gates hid-part (can overlap with attention)
                for jt in range(16):
                    for kc in range(4, 8):
                        nc.tensor.matmul(
                            ps_g[:, 0, jt, :],
                            WahT_sb[:, kc, jt * 128:(jt + 1) * 128],
                            xT16[:, kc, 0:BL],
                            start=(kc == 4), stop=(kc == 7))

                # 2. scores (col-tiled, f32r): batch b=g*4+c -> psum part 32c
                for g in range(2):
                    for c in range(4):
                        b = g * 4 + c
                        for jj in range(4):
                            nc.tensor.matmul(
                                ps_sc[32 * c:32 * c + 32, g, 0:KP],
                                xT16[:, 4 + jj, b:b + 32],
                                keysT_sb[:, jj, b, :],
                                start=(jj == 0), stop=(jj == 3),
                                tile_position=(0, 32 * c))

                # 3-5. softmax (no max-subtract; normalize via row scalars)
                w_sb = w_sb2[t % 2]
                sume = scsb.tile([128, 2], F32, tag="sume")
                rinv = scsb.tile([128, 2], F32, tag="rinv")
                for g in range(2):
                    nc.scalar.activation(
                        w_sb[:, g, 0:HW], ps_sc[:, g, 0:HW], AF.Exp,
                        scale=INV_SCALE, accum_out=sume[:, g:g + 1])
                nc.vector.reciprocal(rinv[:], sume[:])
                for g in range(2):
                    nc.vector.tensor_scalar_mul(
                        w_sb[:, g, 0:HW], w_sb[:, g, 0:HW], rinv[:, g:g + 1])
                # attn output rows b=g*4+c live at partition 32c
                for g in range(2):
                    nc.gpsimd.dma_start(
                        d_attn.ap()[g * 4:(g + 1) * 4, t, :],
                        w_sb[0:128:32, g, 0:HW])

                # 6. transpose w -> wT (cols = batches at 32c)
                for hh in range(4):
                    g, h = hh // 2, hh % 2
                    nc.tensor.transpose(
                        ps_tr[:, hh, :],
                        w_sb[:, g, h * 128:(h + 1) * 128],
                        ident[:])
                eT16 = scsb.tile([128, 4, 4], F16, tag="eT")
                nc.vector.tensor_copy(eT16[:], ps_tr[:, :, 0:128:32])

                # 7. attention output aT[c,b] (fp16 feats as weights)
                for b in range(BL):
                    g, c = b // 4, b % 4
                    for cm in range(4):
                        for h in range(2):
                            nc.tensor.matmul(
                                ps_aT[:, cm, b:b + 1],
                                feats16_sb[:, b * 2 + h,
                                           cm * 128:(cm + 1) * 128],
                                eT16[:, g * 2 + h, c:c + 1],
                                start=(h == 0), stop=(h == 1))
                nc.vector.tensor_copy(xT16[:, 0:4, 0:BL], ps_aT[:])
                nc.vector.tensor_copy(outsT16[:, 4:8, t, :], ps_aT[:])

                # 8. gates a-part
                for jt in range(16):
                    for kc in range(0, 4):
                        nc.tensor.matmul(
                            ps_g[:, 1, jt, :],
                            WahT_sb[:, kc, jt * 128:(jt + 1) * 128],
                            xT16[:, kc, 0:BL],
                            start=(kc == 0), stop=(kc == 3))

                # 9. add hid-part + emb-part(with bias) + a-part
                gf = scsb.tile([128, 16, BL], F32, tag="gf")
                nc.vector.tensor_tensor(
                    out=gf[:], in0=ps_g[:, 0, :, :],
                    in1=ge_sb[:, :, t * BL:(t + 1) * BL], op=OP.add)
                nc.vector.tensor_tensor(
                    out=gf[:], in0=ps_g[:, 1, :, :], in1=gf[:], op=OP.add)

                # 10. LSTM pointwise on [128, (16jj, 8b)]
                gff = gf[:].rearrange("p jj b -> p (jj b)")
                pw = scsb.tile([128, 128], F32, tag="pw")
                pwf = pw[:]
                nc.scalar.activation(pwf[:, 0:96], gff[:, 0:96], AF.Sigmoid)
                nc.scalar.activation(pwf[:, 96:128], gff[:, 96:128], AF.Tanh)
                m1 = scsb.tile([128, 32], F32, tag="m1")
                m2 = scsb.tile([128, 32], F32, tag="m2")
                cellf = cellT_sb[:].rearrange("p jj b -> p (jj b)")
                hidf = hidT_sb[:].rearrange("p jj b -> p (jj b)")
                nc.vector.tensor_tensor(out=m1[:], in0=pwf[:, 32:64],
                                        in1=cellf, op=OP.mult)
                nc.vector.tensor_tensor(out=m2[:], in0=pwf[:, 0:32],
                                        in1=pwf[:, 96:128], op=OP.mult)
                nc.vector.tensor_tensor(out=cellf, in0=m1[:], in1=m2[:],
                                        op=OP.add)
                tc_ = scsb.tile([128, 32], F32, tag="tc")
                nc.scalar.activation(tc_[:], cellf, AF.Tanh)
                nc.vector.tensor_tensor(out=hidf, in0=pwf[:, 64:96],
                                        in1=tc_[:], op=OP.mult)
                nc.vector.tensor_copy(xT16[:, 4:8, 0:BL], hidT_sb[:])
                nc.vector.tensor_copy(outsT16[:, 0:4, t, :], hidT_sb[:])

        # ======================= Phase E: logits =======================
        with tc.tile_pool(name="lo", bufs=2) as lop, \
             tc.tile_pool(name="lps", bufs=4, space="PSUM") as lps:
            GRP = 8
            for mt0 in range(0, VT, GRP):
                n_mt = min(GRP, VT - mt0)
                stage = lop.tile([128, GRP, TB], F32, tag="lo",
                                 name=f"st{mt0}")
                for mi in range(n_mt):
                    mt = mt0 + mi
                    ps_l = lps.tile([128, 256], F32, tag="l",
                                    name=f"psl{mt}")
                    for kc in range(10):
                        nc.tensor.matmul(
                            ps_l[:, 0:TB], wtiles[mt][:, kc, :],
                            outsT16[:, kc, :, :],
                            start=(kc == 0), stop=(kc == 9))
                    if mt % 2 == 0:
                        nc.vector.tensor_scalar_add(
                            stage[:, mi, :], ps_l[:, 0:TB],
                            boT_sb[:, mt:mt + 1])
                    else:
                        nc.scalar.activation(
                            stage[:, mi, :], ps_l[:, 0:TB], AF.Identity,
                            bias=boT_sb[:, mt:mt + 1])
                nc.sync.dma_start(
                    d_logitsT.ap()[mt0:mt0 + n_mt, :, :]
                    .rearrange("m p n -> p m n"),
                    stage[:, 0:n_mt, :])

    nc.compile()
    return nc


def _prep_inputs(image_features, captions_ix, W_h0, b_h0, W_c0, b_c0, emb,
                 W_key, b_key, W_ih, b_ih, W_hh, b_hh, W_out, b_out,
                 t_steps=T):
    """Host-side sharding/layout. Returns list of per-core in_maps."""
    f32 = np.float32
    f16 = np.float16
    img = np.ascontiguousarray(np.asarray(image_features, dtype=f32))
    cap = np.asarray(captions_ix).astype(np.int32)[:, :t_steps]

    WkT = np.ascontiguousarray(np.asarray(W_key, f32).T.astype(f16))
    bkT = np.ascontiguousarray(np.asarray(b_key, f32).reshape(4, 128).T)
    Wh0T = np.ascontiguousarray(np.asarray(W_h0, f32).T / float(HW))
    bh0T = np.ascontiguousarray(np.asarray(b_h0, f32).reshape(4, 128).T)
    Wc0T = np.ascontiguousarray(np.asarray(W_c0, f32).T / float(HW))
    bc0T = np.ascontiguousarray(np.asarray(b_c0, f32).reshape(4, 128).T)
    W_ih = np.asarray(W_ih, f32)
    W_hh = np.asarray(W_hh, f32)
    gperm = np.r_[0:2 * U, 3 * U:4 * U, 2 * U:3 * U]   # (i,f,g,o)->(i,f,o,g)
    WieT = np.ascontiguousarray(W_ih[gperm, :E].T.astype(f16))
    WahT = np.ascontiguousarray(
        np.concatenate([W_ih[gperm, E:].T,
                        np.asarray(W_hh, f32)[gperm].T], axis=0).astype(f16))
    bgT = np.ascontiguousarray(
        (np.asarray(b_ih, f32) + np.asarray(b_hh, f32))[gperm]
        .reshape(16, 128).T)
    WoTf = np.zeros((OD, VP), f16)
    WoTf[:, :V] = np.asarray(W_out, f32).T.astype(f16)
    # per-tile layout [VT, 128, (kc, m)]: partition p's whole SBUF row is one
    # contiguous 2.5KB DRAM run per tile
    WoT = np.ascontiguousarray(
        WoTf.reshape(10, 128, VT, 128).transpose(2, 1, 0, 3)
        .reshape(VT, 128, OD))
    boT = np.zeros((VP,), f32)
    boT[:V] = np.asarray(b_out, f32)
    boT = np.ascontiguousarray(boT.reshape(VT, 128).T)
    embf = np.ascontiguousarray(np.asarray(emb, f32))

    in_maps = []
    for ci in range(NCORES):
        sl = slice(ci * BL, (ci + 1) * BL)
        img_l = img[sl]                                   # [BL, C, HW]
        featsT = np.ascontiguousarray(
            img_l.transpose(1, 0, 2).reshape(C, BL * HW).astype(f16))
        fp = np.zeros((BL, KP, C), f32)
        fp[:, :HW, :] = img_l.transpose(0, 2, 1)
        feats16 = np.ascontiguousarray(fp.reshape(BL * KP, C).astype(f16))
        emb_rows = np.ascontiguousarray(
            embf[cap[sl].T.reshape(t_steps * BL)])        # row r = t*BL+b
        in_maps.append({
            "featsT": featsT, "feats16": feats16, "emb": emb_rows,
            "WkT": WkT, "bkT": bkT,
            "Wh0T": Wh0T, "bh0T": bh0T, "Wc0T": Wc0T, "bc0T": bc0T,
            "WieT": WieT, "WahT": WahT, "bgT": bgT,
            "WoT": WoT, "boT": boT,
        })
    return in_maps


def _assemble(results, t_steps=T):
    logits = np.empty((B, t_steps, V), np.float32)
    attn = np.empty((B, t_steps, HW), np.float32)
    for ci, r in enumerate(results):
        lt = r["logitsT"].reshape(VP, t_steps * BL)[:V]   # [V, (t,b)]
        logits[ci * BL:(ci + 1) * BL] = (
            lt.reshape(V, t_steps, BL).transpose(2, 1, 0))
        attn[ci * BL:(ci + 1) * BL] = r["attn"]
    return logits, attn


def kernel(**inputs):
    if "nc" not in _CACHE:
        _CACHE["nc"] = _build(T)
    nc = _CACHE["nc"]
    in_maps = _prep_inputs(**inputs)
    res = run_bass_kernel_spmd(nc, in_maps, core_ids=list(range(NCORES)))
    return _assemble(res.results)


# revision 33
# speedup vs baseline: 1.4008x; 1.0130x over previous
"""CaptionNet Trainium2 kernel (Bass/Tile, 8-core SPMD, batch-sharded).

Strategy:
- Batch-parallel over 8 NeuronCores (8 batches/core). No collectives.
- LSTM state kept transposed ([feature-part, batch-free]) so pointwise ops
  use all 128 lanes and the recurrent matmuls run weight-stationary.
- Attention scores via 4-way column-tiled PE matmuls (M=1 per batch).
- Matmuls run float32r (fp32 data, ~1e-4 error, 4x faster than fp32) except
  the weight-heavy gates / attention-value / logits matmuls which use fp16
  weights+activations (~5e-4) to halve PE ingest bytes.
- Softmax: exp(scale*x) with fused accumulate, no max-subtraction (scores
  are small by construction), normalization folded in as per-row scalars.
"""
import numpy as np
import ml_dtypes

import concourse.bass as bass
import concourse.tile as tile
from concourse import bacc, mybir
from concourse.bass_utils import run_bass_kernel_spmd
from concourse.masks import make_identity
from contextlib import ExitStack

F32 = mybir.dt.float32
F32R = mybir.dt.float32r
F16 = mybir.dt.float16
I32 = mybir.dt.int32
AF = mybir.ActivationFunctionType
OP = mybir.AluOpType

B, T, C, HW, V, E, U = 64, 20, 512, 196, 10000, 256, 512
NCORES = 8
BL = B // NCORES          # 8 batches per core
KP = 256                  # per-batch padded attention length (HW=196 -> 256)
G4 = 4 * U                # 2048 gate width
OD = U + C + E            # 1280 outs feature dim
VT = (V + 127) // 128     # 79 vocab M-tiles
VP = VT * 128             # 10112 padded vocab
INV_SCALE = 1.0 / float(U) ** 0.5

_CACHE = {}


def _build(t_steps=T):
    """Build + compile the per-core Bass program. Returns nc."""
    nc = bacc.Bacc("TRN2", target_bir_lowering=False, debug=False,
                   num_devices=NCORES)
    TB = t_steps * BL

    # ---- DRAM I/O ----
    d_featsT = nc.dram_tensor("featsT", [C, BL * HW], F16, kind="ExternalInput")
    d_feats16 = nc.dram_tensor("feats16", [BL * KP, C], F16, kind="ExternalInput")
    d_emb = nc.dram_tensor("emb", [TB, E], F32, kind="ExternalInput")
    d_WkT = nc.dram_tensor("WkT", [C, U], F16, kind="ExternalInput")
    d_bkT = nc.dram_tensor("bkT", [128, 4], F32, kind="ExternalInput")
    d_Wh0T = nc.dram_tensor("Wh0T", [C, U], F32R, kind="ExternalInput")
    d_bh0T = nc.dram_tensor("bh0T", [128, 4], F32, kind="ExternalInput")
    d_Wc0T = nc.dram_tensor("Wc0T", [C, U], F32R, kind="ExternalInput")
    d_bc0T = nc.dram_tensor("bc0T", [128, 4], F32, kind="ExternalInput")
    d_WieT = nc.dram_tensor("WieT", [E, G4], F16, kind="ExternalInput")
    d_WahT = nc.dram_tensor("WahT", [C + U, G4], F16, kind="ExternalInput")
    d_bgT = nc.dram_tensor("bgT", [128, 16], F32, kind="ExternalInput")
    d_WoT = nc.dram_tensor("WoT", [VT, 128, OD], F16, kind="ExternalInput")
    d_boT = nc.dram_tensor("boT", [128, VT], F32, kind="ExternalInput")

    d_logitsT = nc.dram_tensor("logitsT", [VT, 128, TB], F32, kind="ExternalOutput")
    d_attn = nc.dram_tensor("attn", [BL, t_steps, HW], F32, kind="ExternalOutput")

    with tile.TileContext(nc) as tc, ExitStack() as ctx:
        ctx.enter_context(nc.allow_low_precision(
            reason="float32r stores feed f32r matmuls by design"))
        per = ctx.enter_context(tc.tile_pool(name="per", bufs=1))

        # ---------- persistent SBUF ----------
        ident = per.tile([128, 128], F32)
        make_identity(nc, ident[:])

        early = tc.tile_pool(name="early", bufs=1)
        earlyp = early.__enter__()
        featsT_sb = earlyp.tile([128, 4, BL * HW], F16)
        for _cc in range(4):
            nc.sync.dma_start(
                featsT_sb[:, _cc, :],
                d_featsT.ap().rearrange("(cc p) n -> p cc n", p=128)[:, _cc, :])
        feats16_sb = per.tile([128, 2 * BL, C], F16)
        nc.sync.dma_start(feats16_sb[:],
                          d_feats16.ap().rearrange("(bh p) c -> p bh c", p=128))
        WkT_sb = earlyp.tile([128, 4, U], F16)
        nc.sync.dma_start(WkT_sb[:],
                          d_WkT.ap().rearrange("(cc p) u -> p cc u", p=128))
        Wh0T_sb = earlyp.tile([128, 4, U], F32R)
        nc.sync.dma_start(Wh0T_sb[:],
                          d_Wh0T.ap().rearrange("(cc p) u -> p cc u", p=128))
        Wc0T_sb = earlyp.tile([128, 4, U], F32R)
        nc.sync.dma_start(Wc0T_sb[:],
                          d_Wc0T.ap().rearrange("(cc p) u -> p cc u", p=128))
        WieT_sb = earlyp.tile([128, 2, G4], F16)
        nc.sync.dma_start(WieT_sb[:],
                          d_WieT.ap().rearrange("(ee p) j -> p ee j", p=128))
        WahT_sb = per.tile([128, 8, G4], F16)
        nc.sync.dma_start(WahT_sb[:],
                          d_WahT.ap().rearrange("(kc p) j -> p kc j", p=128))
        bkT_sb = per.tile([128, 4], F32)
        nc.sync.dma_start(bkT_sb[:], d_bkT.ap())
        bh0T_sb = per.tile([128, 4], F32)
        nc.sync.dma_start(bh0T_sb[:], d_bh0T.ap())
        bc0T_sb = per.tile([128, 4], F32)
        nc.sync.dma_start(bc0T_sb[:], d_bc0T.ap())
        bgT_sb = per.tile([128, 16], F32)
        nc.sync.dma_start(bgT_sb[:], d_bgT.ap())
        boT_sb = per.tile([128, VT], F32)
        nc.sync.dma_start(boT_sb[:], d_boT.ap())

        keysT_sb = per.tile([128, 4, BL, KP], F16)   # padded [u, b, k]
        ge_sb = per.tile([128, 16, TB], F32)         # emb-part of gates + bias
        embT_sb = per.tile([128, 2, TB], F32)
        outsT16 = per.tile([128, 10, t_steps, BL], F16)

        hidT_sb = per.tile([128, 4, BL], F32)
        cellT_sb = per.tile([128, 4, BL], F32)
        # kc 0-3 = aT, 4-7 = hidT; 40-wide so score matmuls can take 32-col
        # lhsT slices (cols 8..39 zeroed; junk rows land on unused partitions)
        xT16 = per.tile([128, 8, BL + 32], F16)

        # =========== Phase A: embedding gather + renorm + transpose =========
        with tc.tile_pool(name="embp", bufs=1) as embp, \
             tc.tile_pool(name="embps", bufs=1, space="PSUM") as embps:
            segs = [(0, min(128, TB))]
            if TB > 128:
                segs.append((128, TB))
            ps_e = embps.tile([128, 4, 128], F32)
            for si, (r0, r1) in enumerate(segs):
                npart, off = r1 - r0, r0
                g = embp.tile([npart, E], F32, tag=f"eg{si}")
                nc.sync.dma_start(g[:], d_emb.ap()[r0:r1, :])
                sq = embp.tile([npart, E], F32, tag=f"sq{si}")
                n2 = embp.tile([npart, 1], F32, tag=f"n2{si}")
                nc.scalar.activation(sq[:], g[:], AF.Square,
                                     accum_out=n2[:, :1])
                nrm = embp.tile([npart, 1], F32, tag=f"nr{si}")
                nc.scalar.sqrt(nrm[:], n2[:])
                nc.vector.tensor_scalar_max(nrm[:], nrm[:], 1e-12)
                inv = embp.tile([npart, 1], F32, tag=f"iv{si}")
                nc.vector.reciprocal(inv[:], nrm[:])
                nc.vector.tensor_scalar_mul(inv[:], inv[:], 5.0)
                nc.vector.tensor_scalar_min(inv[:], inv[:], 1.0)
                nc.vector.tensor_scalar_mul(g[:], g[:], inv[:, :1])
                # transpose [npart, 256] -> embT [256, npart]
                for ee in range(2):
                    nc.tensor.transpose(
                        ps_e[:, 2 * si + ee, 0:npart],
                        g[:, ee * 128:(ee + 1) * 128],
                        ident[0:npart, 0:npart])
                    nc.vector.tensor_copy(
                        embT_sb[:, ee, off:off + npart],
                        ps_e[:, 2 * si + ee, 0:npart])
            # fp16 copy into outsT16 emb rows (kc 8,9)
            nc.vector.tensor_copy(
                outsT16[:, 8:10, :, :],
                embT_sb[:].rearrange("p ee (t b) -> p ee t b", b=BL))
            emb16 = embp.tile([128, 2, TB], F16)
            nc.vector.tensor_copy(emb16[:], embT_sb[:])

            # ======= Phase B: gates_e = W_ihe @ embT (+bias), fp16 =======
            with tc.tile_pool(name="geps", bufs=1, space="PSUM") as geps:
                for half in range(2):
                    ps_ge = geps.tile([128, 8, 256], F32, tag="ge")
                    for jt8 in range(8):
                        jt = half * 8 + jt8
                        for ee in range(2):
                            nc.tensor.matmul(
                                ps_ge[:, jt8, 0:TB],
                                WieT_sb[:, ee, jt * 128:(jt + 1) * 128],
                                emb16[:, ee, :],
                                start=(ee == 0), stop=(ee == 1))
                    for jt8 in range(8):
                        jt = half * 8 + jt8
                        eng = nc.vector if jt8 % 2 == 0 else nc.scalar
                        if eng is nc.vector:
                            nc.vector.tensor_scalar_add(
                                ge_sb[:, jt, :], ps_ge[:, jt8, 0:TB],
                                bgT_sb[:, jt:jt + 1])
                        else:
                            nc.scalar.activation(
                                ge_sb[:, jt, :], ps_ge[:, jt8, 0:TB],
                                AF.Identity, bias=bgT_sb[:, jt:jt + 1])

        # =========== Phase C: feats_mean + keys + initial state ===========
        with tc.tile_pool(name="inip", bufs=1) as inip, \
             tc.tile_pool(name="inips", bufs=1, space="PSUM") as inips, \
             tc.tile_pool(name="keyps", bufs=2, space="PSUM") as keyps:
            fsum = inip.tile([128, 4, BL], F32R)
            for cc in range(4):
                nc.vector.tensor_reduce(
                    fsum[:, cc, :],
                    featsT_sb[:, cc, :].rearrange("p (b k) -> p b k", b=BL),
                    axis=mybir.AxisListType.X, op=OP.add)
            # keys: out keysT [u-tile, (b,k)] ; evac into padded [u, b, KP]
            HHW = 4 * HW  # half the (b,k) range = 784
            for jt in range(4):
                for half in range(2):
                    ps_k = keyps.tile([128, HHW], F32, tag="k")
                    for cc in range(4):
                        for (n0, n1) in [(0, 512), (512, HHW)]:
                            nc.tensor.matmul(
                                ps_k[:, n0:n1],
                                WkT_sb[:, cc, jt * 128:(jt + 1) * 128],
                                featsT_sb[:, cc,
                                          half * HHW + n0:half * HHW + n1],
                                start=(cc == 0), stop=(cc == 3))
                    out_ap = keysT_sb[:, jt, half * 4:(half + 1) * 4, 0:HW]
                    in_ap = ps_k[:].rearrange("p (b k) -> p b k", b=4)
                    if (2 * jt + half) % 2 == 0:
                        nc.vector.tensor_scalar_add(out_ap, in_ap,
                                                    bkT_sb[:, jt:jt + 1])
                    else:
                        nc.scalar.activation(out_ap, in_ap, AF.Identity,
                                             bias=bkT_sb[:, jt:jt + 1])
                    # zero the k-padding (196..256) with a rounding store
                    nc.vector.tensor_scalar_mul(
                        keysT_sb[:, jt, half * 4:(half + 1) * 4, HW:KP],
                        ps_k[:, 0:4 * (KP - HW)]
                        .rearrange("p (b k) -> p b k", b=4),
                        0.0)
            # hid0 / cell0 (M-orientation, f32r)
            ps_i = inips.tile([128, 4, 128], F32, tag="i0")
            ps_c = inips.tile([128, 4, 128], F32, tag="c0")
            for jt in range(4):
                for cc in range(4):
                    nc.tensor.matmul(
                        ps_i[:, jt, 0:BL],
                        Wh0T_sb[:, cc, jt * 128:(jt + 1) * 128],
                        fsum[:, cc, :],
                        start=(cc == 0), stop=(cc == 3))
                    nc.tensor.matmul(
                        ps_c[:, jt, 0:BL],
                        Wc0T_sb[:, cc, jt * 128:(jt + 1) * 128],
                        fsum[:, cc, :],
                        start=(cc == 0), stop=(cc == 3))
            for jt in range(4):
                nc.vector.tensor_scalar_add(hidT_sb[:, jt, :],
                                            ps_i[:, jt, 0:BL],
                                            bh0T_sb[:, jt:jt + 1])
                nc.vector.tensor_scalar_add(cellT_sb[:, jt, :],
                                            ps_c[:, jt, 0:BL],
                                            bc0T_sb[:, jt:jt + 1])
            nc.vector.tensor_copy(xT16[:, 4:8, 0:BL], hidT_sb[:])
            nc.vector.tensor_scalar_mul(
                xT16[:, :, BL:], bh0T_sb[:, 0:1].unsqueeze(-1)
                .to_broadcast([128, 8, 32]), 0.0)

        early.__exit__(None, None, None)

        # prefetch all logits weight tiles (sync queue stays unblocked;
        # loads beyond the buffer count trickle in as logits consumes)
        lwp = ctx.enter_context(tc.tile_pool(name="lw", bufs=42))
        wtiles = []
        for mt in range(VT):
            wt = lwp.tile([128, 10, 128], F16, tag="wt", name=f"wt{mt}")
            nc.sync.dma_start(
                wt[:].rearrange("p kc m -> p (kc m)"), d_WoT.ap()[mt, :, :])
            wtiles.append(wt)

        # ======================= Phase D: the scan =======================
        with tc.tile_pool(name="scps", bufs=1, space="PSUM") as scps, \
             tc.tile_pool(name="scsb", bufs=2) as scsb:
            ps_sc2 = [scps.tile([128, 2, 512], F32, tag=f"sc{i}", name=f"ps_sc{i}")
                      for i in range(2)]
            nc.vector.memset(ps_sc2[0][:], 0.0)
            nc.vector.memset(ps_sc2[1][:], 0.0)
            ps_tr = scps.tile([128, 4, 128], F32)
            ps_aT = scps.tile([128, 4, BL], F32)
            ps_g2 = [scps.tile([128, 2, 16, BL], F32, tag=f"g{i}",
                               name=f"ps_g{i}") for i in range(2)]
            w_sb2 = [scsb.tile([128, 2, KP], F32, tag=f"w{i}", name=f"w_sb{i}")
                     for i in range(2)]
            nc.gpsimd.memset(w_sb2[0][:], 0.0)
            nc.gpsimd.memset(w_sb2[1][:], 0.0)

            for t in range(t_steps):
                ps_sc = ps_sc2[t % 2]
                ps_g = ps_g2[t % 2]

                # 1. scores (col-tiled, f32r): batch b=g*4+c -> psum part 32c
                for g in range(2):
                    for c in range(4):
                        b = g * 4 + c
                        for jj in range(4):
                            nc.tensor.matmul(
                                ps_sc[32 * c:32 * c + 32, g, 0:KP],
                                xT16[:, 4 + jj, b:b + 32],
                                keysT_sb[:, jj, b, :],
                                start=(jj == 0), stop=(jj == 3),
                                tile_position=(0, 32 * c))

                # 2. gates hid-part (can overlap with attention)
                for jt in range(16):
                    for kc in range(4, 8):
                        nc.tensor.matmul(
                            ps_g[:, 0, jt, :],
                            WahT_sb[:, kc, jt * 128:(jt + 1) * 128],
                            xT16[:, kc, 0:BL],
                            start=(kc == 4), stop=(kc == 7))

                # 3-5. softmax (no max-subtract; normalize via row scalars)
                w_sb = w_sb2[t % 2]
                sume = scsb.tile([128, 2], F32, tag="sume")
                rinv = scsb.tile([128, 2], F32, tag="rinv")
                for g in range(2):
                    nc.scalar.activation(
                        w_sb[:, g, 0:HW], ps_sc[:, g, 0:HW], AF.Exp,
                        scale=INV_SCALE, accum_out=sume[:, g:g + 1])
                nc.vector.reciprocal(rinv[:], sume[:])
                for g in range(2):
                    nc.vector.tensor_scalar_mul(
                        w_sb[:, g, 0:HW], w_sb[:, g, 0:HW], rinv[:, g:g + 1])
                # attn output rows b=g*4+c live at partition 32c
                for g in range(2):
                    nc.gpsimd.dma_start(
                        d_attn.ap()[g * 4:(g + 1) * 4, t, :],
                        w_sb[0:128:32, g, 0:HW])

                # 6. transpose w -> wT (cols = batches at 32c)
                for hh in range(4):
                    g, h = hh // 2, hh % 2
                    nc.tensor.transpose(
                        ps_tr[:, hh, :],
                        w_sb[:, g, h * 128:(h + 1) * 128],
                        ident[:])
                eT16 = scsb.tile([128, 4, 4], F16, tag="eT")
                nc.vector.tensor_copy(eT16[:], ps_tr[:, :, 0:128:32])

                # 7. attention output aT[c,b] (fp16 feats as weights)
                for b in range(BL):
                    g, c = b // 4, b % 4
                    for cm in range(4):
                        for h in range(2):
                            nc.tensor.matmul(
                                ps_aT[:, cm, b:b + 1],
                                feats16_sb[:, b * 2 + h,
                                           cm * 128:(cm + 1) * 128],
                                eT16[:, g * 2 + h, c:c + 1],
                                start=(h == 0), stop=(h == 1))
                nc.vector.tensor_copy(xT16[:, 0:4, 0:BL], ps_aT[:])
                nc.vector.tensor_copy(outsT16[:, 4:8, t, :], ps_aT[:])

                # 8. gates a-part
                for jt in range(16):
                    for kc in range(0, 4):
                        nc.tensor.matmul(
                            ps_g[:, 1, jt, :],
                            WahT_sb[:, kc, jt * 128:(jt + 1) * 128],
                            xT16[:, kc, 0:BL],
                            start=(kc == 0), stop=(kc == 3))

                # 9. add hid-part + emb-part(with bias) + a-part
                gf = scsb.tile([128, 16, BL], F32, tag="gf")
                nc.vector.tensor_tensor(
                    out=gf[:], in0=ps_g[:, 0, :, :],
                    in1=ge_sb[:, :, t * BL:(t + 1) * BL], op=OP.add)
                nc.vector.tensor_tensor(
                    out=gf[:], in0=ps_g[:, 1, :, :], in1=gf[:], op=OP.add)

                # 10. LSTM pointwise on [128, (16jj, 8b)]
                gff = gf[:].rearrange("p jj b -> p (jj b)")
                pw = scsb.tile([128, 128], F32, tag="pw")
                pwf = pw[:]
                nc.scalar.activation(pwf[:, 0:96], gff[:, 0:96], AF.Sigmoid)
                nc.scalar.activation(pwf[:, 96:128], gff[:, 96:128], AF.Tanh)
                m1 = scsb.tile([128, 32], F32, tag="m1")
                m2 = scsb.tile([128, 32], F32, tag="m2")
                cellf = cellT_sb[:].rearrange("p jj b -> p (jj b)")
                hidf = hidT_sb[:].rearrange("p jj b -> p (jj b)")
                nc.vector.tensor_tensor(out=m1[:], in0=pwf[:, 32:64],
                                        in1=cellf, op=OP.mult)
                nc.vector.tensor_tensor(out=m2[:], in0=pwf[:, 0:32],
                                        in1=pwf[:, 96:128], op=OP.mult)
                nc.vector.tensor_tensor(out=cellf, in0=m1[:], in1=m2[:],
                                        op=OP.add)
                tc_ = scsb.tile([128, 32], F32, tag="tc")
                nc.scalar.activation(tc_[:], cellf, AF.Tanh)
                nc.vector.tensor_tensor(out=hidf, in0=pwf[:, 64:96],
                                        in1=tc_[:], op=OP.mult)
                nc.vector.tensor_copy(xT16[:, 4:8, 0:BL], hidT_sb[:])
                nc.vector.tensor_copy(outsT16[:, 0:4, t, :], hidT_sb[:])

        # ======================= Phase E: logits =======================
        with tc.tile_pool(name="lo", bufs=2) as lop, \
             tc.tile_pool(name="lps", bufs=4, space="PSUM") as lps:
            GRP = 8
            for mt0 in range(0, VT, GRP):
                n_mt = min(GRP, VT - mt0)
                stage = lop.tile([128, GRP, TB], F32, tag="lo",
                                 name=f"st{mt0}")
                for mi in range(n_mt):
                    mt = mt0 + mi
                    ps_l = lps.tile([128, 256], F32, tag="l",
                                    name=f"psl{mt}")
                    for kc in range(10):
                        nc.tensor.matmul(
                            ps_l[:, 0:TB], wtiles[mt][:, kc, :],
                            outsT16[:, kc, :, :],
                            start=(kc == 0), stop=(kc == 9))
                    if mt % 2 == 0:
                        nc.vector.tensor_scalar_add(
                            stage[:, mi, :], ps_l[:, 0:TB],
                            boT_sb[:, mt:mt + 1])
                    else:
                        nc.scalar.activation(
                            stage[:, mi, :], ps_l[:, 0:TB], AF.Identity,
                            bias=boT_sb[:, mt:mt + 1])
                nc.sync.dma_start(
                    d_logitsT.ap()[mt0:mt0 + n_mt, :, :]
                    .rearrange("m p n -> p m n"),
                    stage[:, 0:n_mt, :])

    nc.compile()
    return nc


def _prep_inputs(image_features, captions_ix, W_h0, b_h0, W_c0, b_c0, emb,
                 W_key, b_key, W_ih, b_ih, W_hh, b_hh, W_out, b_out,
                 t_steps=T):
    """Host-side sharding/layout. Returns list of per-core in_maps."""
    f32 = np.float32
    f16 = np.float16
    img = np.ascontiguousarray(np.asarray(image_features, dtype=f32))
    cap = np.asarray(captions_ix).astype(np.int32)[:, :t_steps]

    WkT = np.ascontiguousarray(np.asarray(W_key, f32).T.astype(f16))
    bkT = np.ascontiguousarray(np.asarray(b_key, f32).reshape(4, 128).T)
    Wh0T = np.ascontiguousarray(np.asarray(W_h0, f32).T / float(HW))
    bh0T = np.ascontiguousarray(np.asarray(b_h0, f32).reshape(4, 128).T)
    Wc0T = np.ascontiguousarray(np.asarray(W_c0, f32).T / float(HW))
    bc0T = np.ascontiguousarray(np.asarray(b_c0, f32).reshape(4, 128).T)
    W_ih = np.asarray(W_ih, f32)
    W_hh = np.asarray(W_hh, f32)
    gperm = np.r_[0:2 * U, 3 * U:4 * U, 2 * U:3 * U]   # (i,f,g,o)->(i,f,o,g)
    WieT = np.ascontiguousarray(W_ih[gperm, :E].T.astype(f16))
    WahT = np.ascontiguousarray(
        np.concatenate([W_ih[gperm, E:].T,
                        np.asarray(W_hh, f32)[gperm].T], axis=0).astype(f16))
    bgT = np.ascontiguousarray(
        (np.asarray(b_ih, f32) + np.asarray(b_hh, f32))[gperm]
        .reshape(16, 128).T)
    WoTf = np.zeros((OD, VP), f16)
    WoTf[:, :V] = np.asarray(W_out, f32).T.astype(f16)
    # per-tile layout [VT, 128, (kc, m)]: partition p's whole SBUF row is one
    # contiguous 2.5KB DRAM run per tile
    WoT = np.ascontiguousarray(
        WoTf.reshape(10, 128, VT, 128).transpose(2, 1, 0, 3)
        .reshape(VT, 128, OD))
    boT = np.zeros((VP,), f32)
    boT[:V] = np.asarray(b_out, f32)
    boT = np.ascontiguousarray(boT.reshape(VT, 128).T)
    embf = np.ascontiguousarray(np.asarray(emb, f32))

    in_maps = []
    for ci in range(NCORES):
        sl = slice(ci * BL, (ci + 1) * BL)
        img_l = img[sl]                                   # [BL, C, HW]
        featsT = np.ascontiguousarray(
            img_l.transpose(1, 0, 2).reshape(C, BL * HW).astype(f16))
        fp = np.zeros((BL, KP, C), f32)
        fp[:, :HW, :] = img_l.transpose(0, 2, 1)
        feats16 = np.ascontiguousarray(fp.reshape(BL * KP, C).astype(f16))
        emb_rows = np.ascontiguousarray(
            embf[cap[sl].T.reshape(t_steps * BL)])        # row r = t*BL+b
        in_maps.append({
            "featsT": featsT, "feats16": feats16, "emb": emb_rows,
            "WkT": WkT, "bkT": bkT,
            "Wh0T": Wh0T, "bh0T": bh0T, "Wc0T": Wc0T, "bc0T": bc0T,
            "WieT": WieT, "WahT": WahT, "bgT": bgT,
            "WoT": WoT, "boT": boT,
        })
    return in_maps


def _assemble(results, t_steps=T):
    logits = np.empty((B, t_steps, V), np.float32)
    attn = np.empty((B, t_steps, HW), np.float32)
    for ci, r in enumerate(results):
        lt = r["logitsT"].reshape(VP, t_steps * BL)[:V]   # [V, (t,b)]
        logits[ci * BL:(ci + 1) * BL] = (
            lt.reshape(V, t_steps, BL).transpose(2, 1, 0))
        attn[ci * BL:(ci + 1) * BL] = r["attn"]
    return logits, attn


def kernel(**inputs):
    if "nc" not in _CACHE:
        _CACHE["nc"] = _build(T)
    nc = _CACHE["nc"]
    in_maps = _prep_inputs(**inputs)
    res = run_bass_kernel_spmd(nc, in_maps, core_ids=list(range(NCORES)))
    return _assemble(res.results)


# revision 39
# speedup vs baseline: 1.4467x; 1.0327x over previous
"""CaptionNet Trainium2 kernel (Bass/Tile, 8-core SPMD, batch-sharded).

Strategy:
- Batch-parallel over 8 NeuronCores (8 local batches/core). No collectives;
  the host shards inputs and concatenates outputs.
- LSTM state kept transposed ([feature-part, batch-free]) so pointwise ops
  use all 128 DVE lanes and the recurrent matmuls run weight-stationary.
- Attention scores via 4-way column-tiled PE matmuls (one batch per 32-wide
  column group; fp16 operands — 4-byte dtypes cannot column-tile).
- hid0/cell0 projections use float32r (fp32 bits, 1 cycle/col at N>=256);
  weight-heavy matmuls (gates / attention values / logits) use fp16
  weights+activations (~5e-4 rel err) to halve PE ingest bytes.
- Softmax: exp(scale*x) with fused accumulate sum, no max-subtraction
  (scores are small by construction); 1/sum applied as per-row scalars.
- All 79 logits weight tiles are prefetched on the sync DMA queue during the
  scan (contiguous per-tile DRAM layout, prepared host-side).
"""
import numpy as np

import concourse.bass as bass
import concourse.tile as tile
from concourse import bacc, mybir
from concourse.bass_utils import run_bass_kernel_spmd
from concourse.masks import make_identity
from contextlib import ExitStack

F32 = mybir.dt.float32
F32R = mybir.dt.float32r
F16 = mybir.dt.float16
I32 = mybir.dt.int32
AF = mybir.ActivationFunctionType
OP = mybir.AluOpType

B, T, C, HW, V, E, U = 64, 20, 512, 196, 10000, 256, 512
NCORES = 8
BL = B // NCORES          # 8 batches per core
KP = 256                  # per-batch padded attention length (HW=196 -> 256)
G4 = 4 * U                # 2048 gate width
OD = U + C + E            # 1280 outs feature dim
VT = (V + 127) // 128     # 79 vocab M-tiles
VP = VT * 128             # 10112 padded vocab
INV_SCALE = 1.0 / float(U) ** 0.5

_CACHE = {}


def _build(t_steps=T):
    """Build + compile the per-core Bass program. Returns nc."""
    nc = bacc.Bacc("TRN2", target_bir_lowering=False, debug=False,
                   num_devices=NCORES)
    TB = t_steps * BL

    # ---- DRAM I/O ----
    d_featsT = nc.dram_tensor("featsT", [C, BL * HW], F16, kind="ExternalInput")
    d_feats16 = nc.dram_tensor("feats16", [BL * KP, C], F16, kind="ExternalInput")
    d_emb = nc.dram_tensor("emb", [TB, E], F32, kind="ExternalInput")
    d_WkT = nc.dram_tensor("WkT", [C, U], F16, kind="ExternalInput")
    d_bkT = nc.dram_tensor("bkT", [128, 4], F32, kind="ExternalInput")
    d_Wh0T = nc.dram_tensor("Wh0T", [C, U], F16, kind="ExternalInput")
    d_bh0T = nc.dram_tensor("bh0T", [128, 4], F32, kind="ExternalInput")
    d_Wc0T = nc.dram_tensor("Wc0T", [C, U], F16, kind="ExternalInput")
    d_bc0T = nc.dram_tensor("bc0T", [128, 4], F32, kind="ExternalInput")
    d_WieT = nc.dram_tensor("WieT", [E, G4], F16, kind="ExternalInput")
    d_WahT = nc.dram_tensor("WahT", [C + U, G4], F16, kind="ExternalInput")
    d_bgT = nc.dram_tensor("bgT", [128, 16], F32, kind="ExternalInput")
    d_WoT = nc.dram_tensor("WoT", [VT, 128, OD], F16, kind="ExternalInput")
    d_boT = nc.dram_tensor("boT", [128, VT], F32, kind="ExternalInput")

    d_logitsT = nc.dram_tensor("logitsT", [VT, 128, TB], F32, kind="ExternalOutput")
    d_attn = nc.dram_tensor("attn", [BL, t_steps, HW], F32, kind="ExternalOutput")

    with tile.TileContext(nc) as tc, ExitStack() as ctx:
        ctx.enter_context(nc.allow_low_precision(
            reason="float32r stores feed f32r matmuls by design"))
        per = ctx.enter_context(tc.tile_pool(name="per", bufs=1))

        # ---------- persistent SBUF ----------
        ident = per.tile([128, 128], F32)
        make_identity(nc, ident[:])

        early = tc.tile_pool(name="early", bufs=1)
        earlyp = early.__enter__()
        featsT_sb = earlyp.tile([128, 4, BL * HW], F16)
        for _cc in range(4):
            nc.sync.dma_start(
                featsT_sb[:, _cc, :],
                d_featsT.ap().rearrange("(cc p) n -> p cc n", p=128)[:, _cc, :])
        feats16_sb = per.tile([128, 2 * BL, C], F16)
        nc.sync.dma_start(feats16_sb[:],
                          d_feats16.ap().rearrange("(bh p) c -> p bh c", p=128))
        WkT_sb = earlyp.tile([128, 4, U], F16)
        nc.sync.dma_start(WkT_sb[:],
                          d_WkT.ap().rearrange("(cc p) u -> p cc u", p=128))
        Wh0T_sb = earlyp.tile([128, 4, U], F16)
        nc.sync.dma_start(Wh0T_sb[:],
                          d_Wh0T.ap().rearrange("(cc p) u -> p cc u", p=128))
        Wc0T_sb = earlyp.tile([128, 4, U], F16)
        nc.sync.dma_start(Wc0T_sb[:],
                          d_Wc0T.ap().rearrange("(cc p) u -> p cc u", p=128))
        WieT_sb = earlyp.tile([128, 2, G4], F16)
        nc.sync.dma_start(WieT_sb[:],
                          d_WieT.ap().rearrange("(ee p) j -> p ee j", p=128))
        WahT_sb = per.tile([128, 8, G4], F16)
        nc.sync.dma_start(WahT_sb[:],
                          d_WahT.ap().rearrange("(kc p) j -> p kc j", p=128))
        bkT_sb = per.tile([128, 4], F32)
        nc.sync.dma_start(bkT_sb[:], d_bkT.ap())
        bh0T_sb = per.tile([128, 4], F32)
        nc.sync.dma_start(bh0T_sb[:], d_bh0T.ap())
        bc0T_sb = per.tile([128, 4], F32)
        nc.sync.dma_start(bc0T_sb[:], d_bc0T.ap())
        bgT_sb = per.tile([128, 16], F32)
        nc.sync.dma_start(bgT_sb[:], d_bgT.ap())
        boT_sb = per.tile([128, VT], F32)
        nc.sync.dma_start(boT_sb[:], d_boT.ap())

        keysT_sb = per.tile([128, 4, BL, KP], F16)   # padded [u, b, k]
        ge_sb = per.tile([128, 16, TB], F32)         # emb-part of gates + bias
        embT_sb = per.tile([128, 2, TB], F32)
        outsT16 = per.tile([128, 10, t_steps, BL], F16)

        hidT_sb = per.tile([128, 4, BL], F32)
        cellT_sb = per.tile([128, 4, BL], F32)
        # kc 0-3 = aT, 4-7 = hidT; 40-wide so score matmuls can take 32-col
        # lhsT slices (cols 8..39 zeroed; junk rows land on unused partitions)
        xT16 = per.tile([128, 8, BL + 32], F16)

        # =========== Phase A: embedding gather + renorm + transpose =========
        with tc.tile_pool(name="embp", bufs=1) as embp, \
             tc.tile_pool(name="embps", bufs=1, space="PSUM") as embps:
            segs = [(0, min(128, TB))]
            if TB > 128:
                segs.append((128, TB))
            ps_e = embps.tile([128, 4, 128], F32)
            for si, (r0, r1) in enumerate(segs):
                npart, off = r1 - r0, r0
                g = embp.tile([npart, E], F32, tag=f"eg{si}")
                nc.sync.dma_start(g[:], d_emb.ap()[r0:r1, :])
                sq = embp.tile([npart, E], F32, tag=f"sq{si}")
                n2 = embp.tile([npart, 1], F32, tag=f"n2{si}")
                nc.scalar.activation(sq[:], g[:], AF.Square,
                                     accum_out=n2[:, :1])
                nrm = embp.tile([npart, 1], F32, tag=f"nr{si}")
                nc.scalar.sqrt(nrm[:], n2[:])
                nc.vector.tensor_scalar_max(nrm[:], nrm[:], 1e-12)
                inv = embp.tile([npart, 1], F32, tag=f"iv{si}")
                nc.vector.reciprocal(inv[:], nrm[:])
                nc.vector.tensor_scalar_mul(inv[:], inv[:], 5.0)
                nc.vector.tensor_scalar_min(inv[:], inv[:], 1.0)
                nc.vector.tensor_scalar_mul(g[:], g[:], inv[:, :1])
                # transpose [npart, 256] -> embT [256, npart]
                for ee in range(2):
                    nc.tensor.transpose(
                        ps_e[:, 2 * si + ee, 0:npart],
                        g[:, ee * 128:(ee + 1) * 128],
                        ident[0:npart, 0:npart])
                    nc.vector.tensor_copy(
                        embT_sb[:, ee, off:off + npart],
                        ps_e[:, 2 * si + ee, 0:npart])
            # fp16 copy into outsT16 emb rows (kc 8,9)
            nc.vector.tensor_copy(
                outsT16[:, 8:10, :, :],
                embT_sb[:].rearrange("p ee (t b) -> p ee t b", b=BL))
            emb16 = embp.tile([128, 2, TB], F16)
            nc.vector.tensor_copy(emb16[:], embT_sb[:])

            # ======= Phase B: gates_e = W_ihe @ embT (+bias), fp16 =======
            with tc.tile_pool(name="geps", bufs=1, space="PSUM") as geps:
                for half in range(2):
                    ps_ge = geps.tile([128, 8, 256], F32, tag="ge")
                    for jt8 in range(8):
                        jt = half * 8 + jt8
                        for ee in range(2):
                            nc.tensor.matmul(
                                ps_ge[:, jt8, 0:TB],
                                WieT_sb[:, ee, jt * 128:(jt + 1) * 128],
                                emb16[:, ee, :],
                                start=(ee == 0), stop=(ee == 1))
                    for jt8 in range(8):
                        jt = half * 8 + jt8
                        eng = nc.vector if jt8 % 2 == 0 else nc.scalar
                        if eng is nc.vector:
                            nc.vector.tensor_scalar_add(
                                ge_sb[:, jt, :], ps_ge[:, jt8, 0:TB],
                                bgT_sb[:, jt:jt + 1])
                        else:
                            nc.scalar.activation(
                                ge_sb[:, jt, :], ps_ge[:, jt8, 0:TB],
                                AF.Identity, bias=bgT_sb[:, jt:jt + 1])

        # =========== Phase C: feats_mean + keys + initial state ===========
        with tc.tile_pool(name="inip", bufs=1) as inip, \
             tc.tile_pool(name="inips", bufs=1, space="PSUM") as inips, \
             tc.tile_pool(name="keyps", bufs=2, space="PSUM") as keyps:
            fsum = inip.tile([128, 4, BL], F16)
            for cc in range(4):
                nc.vector.tensor_reduce(
                    fsum[:, cc, :],
                    featsT_sb[:, cc, :].rearrange("p (b k) -> p b k", b=BL),
                    axis=mybir.AxisListType.X, op=OP.add)
            # keys: out keysT [u-tile, (b,k)] ; evac into padded [u, b, KP]
            HHW = 4 * HW  # half the (b,k) range = 784
            for jt in range(4):
                for half in range(2):
                    ps_k = keyps.tile([128, HHW], F32, tag="k")
                    for cc in range(4):
                        for (n0, n1) in [(0, 512), (512, HHW)]:
                            nc.tensor.matmul(
                                ps_k[:, n0:n1],
                                WkT_sb[:, cc, jt * 128:(jt + 1) * 128],
                                featsT_sb[:, cc,
                                          half * HHW + n0:half * HHW + n1],
                                start=(cc == 0), stop=(cc == 3))
                    out_ap = keysT_sb[:, jt, half * 4:(half + 1) * 4, 0:HW]
                    in_ap = ps_k[:].rearrange("p (b k) -> p b k", b=4)
                    if (2 * jt + half) % 2 == 0:
                        nc.vector.tensor_scalar_add(out_ap, in_ap,
                                                    bkT_sb[:, jt:jt + 1])
                    else:
                        nc.scalar.activation(out_ap, in_ap, AF.Identity,
                                             bias=bkT_sb[:, jt:jt + 1])
                    # zero the k-padding (196..256) with a rounding store
                    nc.vector.tensor_scalar_mul(
                        keysT_sb[:, jt, half * 4:(half + 1) * 4, HW:KP],
                        ps_k[:, 0:4 * (KP - HW)]
                        .rearrange("p (b k) -> p b k", b=4),
                        0.0)
            # hid0 / cell0 (M-orientation, f32r)
            ps_i = inips.tile([128, 4, 128], F32, tag="i0")
            ps_c = inips.tile([128, 4, 128], F32, tag="c0")
            for jt in range(4):
                for cc in range(4):
                    nc.tensor.matmul(
                        ps_i[:, jt, 0:BL],
                        Wh0T_sb[:, cc, jt * 128:(jt + 1) * 128],
                        fsum[:, cc, :],
                        start=(cc == 0), stop=(cc == 3))
                    nc.tensor.matmul(
                        ps_c[:, jt, 0:BL],
                        Wc0T_sb[:, cc, jt * 128:(jt + 1) * 128],
                        fsum[:, cc, :],
                        start=(cc == 0), stop=(cc == 3))
            for jt in range(4):
                nc.vector.tensor_scalar_add(hidT_sb[:, jt, :],
                                            ps_i[:, jt, 0:BL],
                                            bh0T_sb[:, jt:jt + 1])
                nc.vector.tensor_scalar_add(cellT_sb[:, jt, :],
                                            ps_c[:, jt, 0:BL],
                                            bc0T_sb[:, jt:jt + 1])
            nc.vector.tensor_copy(xT16[:, 4:8, 0:BL], hidT_sb[:])
            nc.vector.tensor_scalar_mul(
                xT16[:, :, BL:], bh0T_sb[:, 0:1].unsqueeze(-1)
                .to_broadcast([128, 8, 32]), 0.0)

        early.__exit__(None, None, None)

        # prefetch all logits weight tiles (sync queue stays unblocked;
        # loads beyond the buffer count trickle in as logits consumes)
        lwp = ctx.enter_context(tc.tile_pool(name="lw", bufs=46))
        wtiles = []
        for mt in range(VT):
            wt = lwp.tile([128, 10, 128], F16, tag="wt", name=f"wt{mt}")
            nc.sync.dma_start(
                wt[:].rearrange("p kc m -> p (kc m)"), d_WoT.ap()[mt, :, :])
            wtiles.append(wt)

        # ======================= Phase D: the scan =======================
        with tc.tile_pool(name="scps", bufs=1, space="PSUM") as scps, \
             tc.tile_pool(name="scsb", bufs=2) as scsb:
            ps_sc2 = [scps.tile([128, 2, 512], F32, tag=f"sc{i}", name=f"ps_sc{i}")
                      for i in range(2)]
            nc.vector.memset(ps_sc2[0][:], 0.0)
            nc.vector.memset(ps_sc2[1][:], 0.0)
            ps_tr = scps.tile([128, 4, 128], F32)
            ps_aT = scps.tile([128, 4, BL], F32)
            ps_g2 = [scps.tile([128, 2, 16, BL], F32, tag=f"g{i}",
                               name=f"ps_g{i}") for i in range(2)]
            w_sb2 = [scsb.tile([128, 2, KP], F32, tag=f"w{i}", name=f"w_sb{i}")
                     for i in range(2)]
            nc.gpsimd.memset(w_sb2[0][:], 0.0)
            nc.gpsimd.memset(w_sb2[1][:], 0.0)

            for t in range(t_steps):
                ps_sc = ps_sc2[t % 2]
                ps_g = ps_g2[t % 2]

                # 1. scores (col-tiled, f32r): batch b=g*4+c -> psum part 32c
                for g in range(2):
                    for c in range(4):
                        b = g * 4 + c
                        for jj in range(4):
                            nc.tensor.matmul(
                                ps_sc[32 * c:32 * c + 32, g, 0:KP],
                                xT16[:, 4 + jj, b:b + 32],
                                keysT_sb[:, jj, b, :],
                                start=(jj == 0), stop=(jj == 3),
                                tile_position=(0, 32 * c))

                # 2. gates hid-part (can overlap with attention)
                for jt in range(16):
                    for kc in range(4, 8):
                        nc.tensor.matmul(
                            ps_g[:, 0, jt, :],
                            WahT_sb[:, kc, jt * 128:(jt + 1) * 128],
                            xT16[:, kc, 0:BL],
                            start=(kc == 4), stop=(kc == 7))

                # 3-6. per-group softmax -> transpose -> eT (g0 chain
                # overlaps g1 scores/exp on other engines)
                w_sb = w_sb2[t % 2]
                sume = scsb.tile([128, 2], F32, tag="sume")
                rinv = scsb.tile([128, 2], F32, tag="rinv")
                eT16 = scsb.tile([128, 4, 4], F16, tag="eT")
                for g in range(2):
                    nc.scalar.activation(
                        w_sb[:, g, 0:HW], ps_sc[:, g, 0:HW], AF.Exp,
                        scale=INV_SCALE, accum_out=sume[:, g:g + 1])
                    nc.vector.reciprocal(rinv[:, g:g + 1], sume[:, g:g + 1])
                    nc.vector.tensor_scalar_mul(
                        w_sb[:, g, 0:HW], w_sb[:, g, 0:HW], rinv[:, g:g + 1])
                    for h in range(2):
                        hh = g * 2 + h
                        nc.tensor.transpose(
                            ps_tr[:, hh, :],
                            w_sb[:, g, h * 128:(h + 1) * 128],
                            ident[:])
                    nc.vector.tensor_copy(
                        eT16[:, 2 * g:2 * g + 2, :],
                        ps_tr[:, 2 * g:2 * g + 2, 0:128:32])
                    nc.gpsimd.dma_start(
                        d_attn.ap()[g * 4:(g + 1) * 4, t, :],
                        w_sb[0:128:32, g, 0:HW])

                # 7. attention output aT[c,b] (fp16 feats as weights)
                for b in range(BL):
                    g, c = b // 4, b % 4
                    for cm in range(4):
                        for h in range(2):
                            nc.tensor.matmul(
                                ps_aT[:, cm, b:b + 1],
                                feats16_sb[:, b * 2 + h,
                                           cm * 128:(cm + 1) * 128],
                                eT16[:, g * 2 + h, c:c + 1],
                                start=(h == 0), stop=(h == 1))
                nc.vector.tensor_copy(xT16[:, 0:4, 0:BL], ps_aT[:])
                nc.vector.tensor_copy(outsT16[:, 4:8, t, :], ps_aT[:])

                # 8. gates a-part
                for jt in range(16):
                    for kc in range(0, 4):
                        nc.tensor.matmul(
                            ps_g[:, 1, jt, :],
                            WahT_sb[:, kc, jt * 128:(jt + 1) * 128],
                            xT16[:, kc, 0:BL],
                            start=(kc == 0), stop=(kc == 3))

                # 9. add hid-part + emb-part(with bias) + a-part
                gf = scsb.tile([128, 16, BL], F32, tag="gf")
                nc.vector.tensor_tensor(
                    out=gf[:], in0=ps_g[:, 0, :, :],
                    in1=ge_sb[:, :, t * BL:(t + 1) * BL], op=OP.add)
                nc.vector.tensor_tensor(
                    out=gf[:], in0=ps_g[:, 1, :, :], in1=gf[:], op=OP.add)

                # 10. LSTM pointwise on [128, (16jj, 8b)]
                gff = gf[:].rearrange("p jj b -> p (jj b)")
                pw = scsb.tile([128, 128], F32, tag="pw")
                pwf = pw[:]
                nc.scalar.activation(pwf[:, 0:96], gff[:, 0:96], AF.Sigmoid)
                nc.scalar.activation(pwf[:, 96:128], gff[:, 96:128], AF.Tanh)
                m1 = scsb.tile([128, 32], F32, tag="m1")
                m2 = scsb.tile([128, 32], F32, tag="m2")
                cellf = cellT_sb[:].rearrange("p jj b -> p (jj b)")
                hidf = hidT_sb[:].rearrange("p jj b -> p (jj b)")
                nc.vector.tensor_tensor(out=m1[:], in0=pwf[:, 32:64],
                                        in1=cellf, op=OP.mult)
                nc.vector.tensor_tensor(out=m2[:], in0=pwf[:, 0:32],
                                        in1=pwf[:, 96:128], op=OP.mult)
                nc.vector.tensor_tensor(out=cellf, in0=m1[:], in1=m2[:],
                                        op=OP.add)
                tc_ = scsb.tile([128, 32], F32, tag="tc")
                nc.scalar.activation(tc_[:], cellf, AF.Tanh)
                nc.vector.tensor_tensor(out=hidf, in0=pwf[:, 64:96],
                                        in1=tc_[:], op=OP.mult)
                nc.vector.tensor_copy(xT16[:, 4:8, 0:BL], hidT_sb[:])
                nc.vector.tensor_copy(outsT16[:, 0:4, t, :], hidT_sb[:])

        # ======================= Phase E: logits =======================
        with tc.tile_pool(name="lo", bufs=2) as lop, \
             tc.tile_pool(name="lps", bufs=4, space="PSUM") as lps:
            GRP = 8
            for mt0 in range(0, VT, GRP):
                n_mt = min(GRP, VT - mt0)
                stage = lop.tile([128, GRP, TB], F32, tag="lo",
                                 name=f"st{mt0}")
                for mi in range(n_mt):
                    mt = mt0 + mi
                    ps_l = lps.tile([128, 256], F32, tag="l",
                                    name=f"psl{mt}")
                    for kc in range(10):
                        nc.tensor.matmul(
                            ps_l[:, 0:TB], wtiles[mt][:, kc, :],
                            outsT16[:, kc, :, :],
                            start=(kc == 0), stop=(kc == 9))
                    if mt % 2 == 0:
                        nc.vector.tensor_scalar_add(
                            stage[:, mi, :], ps_l[:, 0:TB],
                            boT_sb[:, mt:mt + 1])
                    else:
                        nc.scalar.activation(
                            stage[:, mi, :], ps_l[:, 0:TB], AF.Identity,
                            bias=boT_sb[:, mt:mt + 1])
                nc.sync.dma_start(
                    d_logitsT.ap()[mt0:mt0 + n_mt, :, :]
                    .rearrange("m p n -> p m n"),
                    stage[:, 0:n_mt, :])

    nc.compile()
    return nc


def _prep_inputs(image_features, captions_ix, W_h0, b_h0, W_c0, b_c0, emb,
                 W_key, b_key, W_ih, b_ih, W_hh, b_hh, W_out, b_out,
                 t_steps=T):
    """Host-side sharding/layout. Returns list of per-core in_maps."""
    f32 = np.float32
    f16 = np.float16
    img = np.ascontiguousarray(np.asarray(image_features, dtype=f32))
    cap = np.asarray(captions_ix).astype(np.int32)[:, :t_steps]

    WkT = np.ascontiguousarray(np.asarray(W_key, f32).T.astype(f16))
    bkT = np.ascontiguousarray(np.asarray(b_key, f32).reshape(4, 128).T)
    Wh0T = np.ascontiguousarray((np.asarray(W_h0, f32).T / float(HW)).astype(f16))
    bh0T = np.ascontiguousarray(np.asarray(b_h0, f32).reshape(4, 128).T)
    Wc0T = np.ascontiguousarray((np.asarray(W_c0, f32).T / float(HW)).astype(f16))
    bc0T = np.ascontiguousarray(np.asarray(b_c0, f32).reshape(4, 128).T)
    W_ih = np.asarray(W_ih, f32)
    W_hh = np.asarray(W_hh, f32)
    gperm = np.r_[0:2 * U, 3 * U:4 * U, 2 * U:3 * U]   # (i,f,g,o)->(i,f,o,g)
    WieT = np.ascontiguousarray(W_ih[gperm, :E].T.astype(f16))
    WahT = np.ascontiguousarray(
        np.concatenate([W_ih[gperm, E:].T,
                        np.asarray(W_hh, f32)[gperm].T], axis=0).astype(f16))
    bgT = np.ascontiguousarray(
        (np.asarray(b_ih, f32) + np.asarray(b_hh, f32))[gperm]
        .reshape(16, 128).T)
    WoTf = np.zeros((OD, VP), f16)
    WoTf[:, :V] = np.asarray(W_out, f32).T.astype(f16)
    # per-tile layout [VT, 128, (kc, m)]: partition p's whole SBUF row is one
    # contiguous 2.5KB DRAM run per tile
    WoT = np.ascontiguousarray(
        WoTf.reshape(10, 128, VT, 128).transpose(2, 1, 0, 3)
        .reshape(VT, 128, OD))
    boT = np.zeros((VP,), f32)
    boT[:V] = np.asarray(b_out, f32)
    boT = np.ascontiguousarray(boT.reshape(VT, 128).T)
    embf = np.ascontiguousarray(np.asarray(emb, f32))

    in_maps = []
    for ci in range(NCORES):
        sl = slice(ci * BL, (ci + 1) * BL)
        img_l = img[sl]                                   # [BL, C, HW]
        featsT = np.ascontiguousarray(
            img_l.transpose(1, 0, 2).reshape(C, BL * HW).astype(f16))
        fp = np.zeros((BL, KP, C), f32)
        fp[:, :HW, :] = img_l.transpose(0, 2, 1)
        feats16 = np.ascontiguousarray(fp.reshape(BL * KP, C).astype(f16))
        emb_rows = np.ascontiguousarray(
            embf[cap[sl].T.reshape(t_steps * BL)])        # row r = t*BL+b
        in_maps.append({
            "featsT": featsT, "feats16": feats16, "emb": emb_rows,
            "WkT": WkT, "bkT": bkT,
            "Wh0T": Wh0T, "bh0T": bh0T, "Wc0T": Wc0T, "bc0T": bc0T,
            "WieT": WieT, "WahT": WahT, "bgT": bgT,
            "WoT": WoT, "boT": boT,
        })
    return in_maps


def _assemble(results, t_steps=T):
    logits = np.empty((B, t_steps, V), np.float32)
    attn = np.empty((B, t_steps, HW), np.float32)
    for ci, r in enumerate(results):
        lt = r["logitsT"].reshape(VP, t_steps * BL)[:V]   # [V, (t,b)]
        logits[ci * BL:(ci + 1) * BL] = (
            lt.reshape(V, t_steps, BL).transpose(2, 1, 0))
        attn[ci * BL:(ci + 1) * BL] = r["attn"]
    return logits, attn


def kernel(**inputs):
    if "nc" not in _CACHE:
        _CACHE["nc"] = _build(T)
    nc = _CACHE["nc"]
    in_maps = _prep_inputs(**inputs)
    res = run_bass_kernel_spmd(nc, in_maps, core_ids=list(range(NCORES)))
    return _assemble(res.results)


# revision 41
# speedup vs baseline: 1.4471x; 1.0002x over previous
"""CaptionNet Trainium2 kernel (Bass/Tile, 8-core SPMD, batch-sharded).

Strategy:
- Batch-parallel over 8 NeuronCores (8 local batches/core). No collectives;
  the host shards inputs and concatenates outputs.
- LSTM state kept transposed ([feature-part, batch-free]) so pointwise ops
  use all 128 DVE lanes and the recurrent matmuls run weight-stationary.
- Attention scores via 4-way column-tiled PE matmuls (one batch per 32-wide
  column group; fp16 operands — 4-byte dtypes cannot column-tile).
- hid0/cell0 projections use float32r (fp32 bits, 1 cycle/col at N>=256);
  weight-heavy matmuls (gates / attention values / logits) use fp16
  weights+activations (~5e-4 rel err) to halve PE ingest bytes.
- Softmax: exp(scale*x) with fused accumulate sum, no max-subtraction
  (scores are small by construction); 1/sum applied as per-row scalars.
- All 79 logits weight tiles are prefetched on the sync DMA queue during the
  scan (contiguous per-tile DRAM layout, prepared host-side).
"""
import numpy as np

import concourse.bass as bass
import concourse.tile as tile
from concourse import bacc, mybir
from concourse.bass_utils import run_bass_kernel_spmd
from concourse.masks import make_identity
from contextlib import ExitStack

F32 = mybir.dt.float32
F32R = mybir.dt.float32r
F16 = mybir.dt.float16
I32 = mybir.dt.int32
AF = mybir.ActivationFunctionType
OP = mybir.AluOpType

B, T, C, HW, V, E, U = 64, 20, 512, 196, 10000, 256, 512
NCORES = 8
BL = B // NCORES          # 8 batches per core
KP = 256                  # per-batch padded attention length (HW=196 -> 256)
G4 = 4 * U                # 2048 gate width
OD = U + C + E            # 1280 outs feature dim
VT = (V + 127) // 128     # 79 vocab M-tiles
VP = VT * 128             # 10112 padded vocab
INV_SCALE = 1.0 / float(U) ** 0.5

_CACHE = {}


def _build(t_steps=T):
    """Build + compile the per-core Bass program. Returns nc."""
    nc = bacc.Bacc("TRN2", target_bir_lowering=False, debug=False,
                   num_devices=NCORES)
    TB = t_steps * BL

    # ---- DRAM I/O ----
    d_featsT = nc.dram_tensor("featsT", [C, BL * HW], F16, kind="ExternalInput")
    d_feats16 = nc.dram_tensor("feats16", [BL * KP, C], F16, kind="ExternalInput")
    d_emb = nc.dram_tensor("emb", [TB, E], F32, kind="ExternalInput")
    d_WkT = nc.dram_tensor("WkT", [C, U], F16, kind="ExternalInput")
    d_bkT = nc.dram_tensor("bkT", [128, 4], F32, kind="ExternalInput")
    d_Wh0T = nc.dram_tensor("Wh0T", [C, U], F16, kind="ExternalInput")
    d_bh0T = nc.dram_tensor("bh0T", [128, 4], F32, kind="ExternalInput")
    d_Wc0T = nc.dram_tensor("Wc0T", [C, U], F16, kind="ExternalInput")
    d_bc0T = nc.dram_tensor("bc0T", [128, 4], F32, kind="ExternalInput")
    d_WieT = nc.dram_tensor("WieT", [E, G4], F16, kind="ExternalInput")
    d_WahT = nc.dram_tensor("WahT", [C + U, G4], F16, kind="ExternalInput")
    d_bgT = nc.dram_tensor("bgT", [128, 16], F32, kind="ExternalInput")
    d_WoT = nc.dram_tensor("WoT", [VT, 128, OD], F16, kind="ExternalInput")
    d_boT = nc.dram_tensor("boT", [128, VT], F32, kind="ExternalInput")

    d_logitsT = nc.dram_tensor("logitsT", [VT, 128, TB], F32, kind="ExternalOutput")
    d_attn = nc.dram_tensor("attn", [BL, t_steps, HW], F32, kind="ExternalOutput")

    with tile.TileContext(nc) as tc, ExitStack() as ctx:
        ctx.enter_context(nc.allow_low_precision(
            reason="float32r stores feed f32r matmuls by design"))
        per = ctx.enter_context(tc.tile_pool(name="per", bufs=1))

        # ---------- persistent SBUF ----------
        ident = per.tile([128, 128], F32)
        make_identity(nc, ident[:])

        early = tc.tile_pool(name="early", bufs=1)
        earlyp = early.__enter__()
        featsT_sb = earlyp.tile([128, 4, BL * HW], F16)
        for _cc in range(4):
            nc.sync.dma_start(
                featsT_sb[:, _cc, :],
                d_featsT.ap().rearrange("(cc p) n -> p cc n", p=128)[:, _cc, :])
        WkT_sb = earlyp.tile([128, 4, U], F16)
        nc.sync.dma_start(WkT_sb[:],
                          d_WkT.ap().rearrange("(cc p) u -> p cc u", p=128))
        Wh0T_sb = earlyp.tile([128, 4, U], F16)
        nc.sync.dma_start(Wh0T_sb[:],
                          d_Wh0T.ap().rearrange("(cc p) u -> p cc u", p=128))
        Wc0T_sb = earlyp.tile([128, 4, U], F16)
        nc.sync.dma_start(Wc0T_sb[:],
                          d_Wc0T.ap().rearrange("(cc p) u -> p cc u", p=128))
        WieT_sb = earlyp.tile([128, 2, G4], F16)
        nc.sync.dma_start(WieT_sb[:],
                          d_WieT.ap().rearrange("(ee p) j -> p ee j", p=128))
        bkT_sb = per.tile([128, 4], F32)
        nc.sync.dma_start(bkT_sb[:], d_bkT.ap())
        bh0T_sb = per.tile([128, 4], F32)
        nc.sync.dma_start(bh0T_sb[:], d_bh0T.ap())
        bc0T_sb = per.tile([128, 4], F32)
        nc.sync.dma_start(bc0T_sb[:], d_bc0T.ap())
        bgT_sb = per.tile([128, 16], F32)
        nc.sync.dma_start(bgT_sb[:], d_bgT.ap())
        boT_sb = per.tile([128, VT], F32)
        nc.sync.dma_start(boT_sb[:], d_boT.ap())
        feats16_sb = per.tile([128, 2 * BL, C], F16)
        nc.sync.dma_start(feats16_sb[:],
                          d_feats16.ap().rearrange("(bh p) c -> p bh c", p=128))
        WahT_sb = per.tile([128, 8, G4], F16)
        nc.sync.dma_start(WahT_sb[:],
                          d_WahT.ap().rearrange("(kc p) j -> p kc j", p=128))

        keysT_sb = per.tile([128, 4, BL, KP], F16)   # padded [u, b, k]
        ge_sb = per.tile([128, 16, TB], F32)         # emb-part of gates + bias
        embT_sb = per.tile([128, 2, TB], F32)
        outsT16 = per.tile([128, 10, t_steps, BL], F16)

        hidT_sb = per.tile([128, 4, BL], F32)
        cellT_sb = per.tile([128, 4, BL], F32)
        # kc 0-3 = aT, 4-7 = hidT; 40-wide so score matmuls can take 32-col
        # lhsT slices (cols 8..39 zeroed; junk rows land on unused partitions)
        xT16 = per.tile([128, 8, BL + 32], F16)

        # =========== Phase A: embedding gather + renorm + transpose =========
        with tc.tile_pool(name="embp", bufs=1) as embp, \
             tc.tile_pool(name="embps", bufs=1, space="PSUM") as embps:
            segs = [(0, min(128, TB))]
            if TB > 128:
                segs.append((128, TB))
            ps_e = embps.tile([128, 4, 128], F32)
            for si, (r0, r1) in enumerate(segs):
                npart, off = r1 - r0, r0
                g = embp.tile([npart, E], F32, tag=f"eg{si}")
                nc.sync.dma_start(g[:], d_emb.ap()[r0:r1, :])
                sq = embp.tile([npart, E], F32, tag=f"sq{si}")
                n2 = embp.tile([npart, 1], F32, tag=f"n2{si}")
                nc.scalar.activation(sq[:], g[:], AF.Square,
                                     accum_out=n2[:, :1])
                nrm = embp.tile([npart, 1], F32, tag=f"nr{si}")
                nc.scalar.sqrt(nrm[:], n2[:])
                nc.vector.tensor_scalar_max(nrm[:], nrm[:], 1e-12)
                inv = embp.tile([npart, 1], F32, tag=f"iv{si}")
                nc.vector.reciprocal(inv[:], nrm[:])
                nc.vector.tensor_scalar_mul(inv[:], inv[:], 5.0)
                nc.vector.tensor_scalar_min(inv[:], inv[:], 1.0)
                nc.vector.tensor_scalar_mul(g[:], g[:], inv[:, :1])
                # transpose [npart, 256] -> embT [256, npart]
                for ee in range(2):
                    nc.tensor.transpose(
                        ps_e[:, 2 * si + ee, 0:npart],
                        g[:, ee * 128:(ee + 1) * 128],
                        ident[0:npart, 0:npart])
                    nc.vector.tensor_copy(
                        embT_sb[:, ee, off:off + npart],
                        ps_e[:, 2 * si + ee, 0:npart])
            # fp16 copy into outsT16 emb rows (kc 8,9)
            nc.vector.tensor_copy(
                outsT16[:, 8:10, :, :],
                embT_sb[:].rearrange("p ee (t b) -> p ee t b", b=BL))
            emb16 = embp.tile([128, 2, TB], F16)
            nc.vector.tensor_copy(emb16[:], embT_sb[:])

            # ======= Phase B: gates_e = W_ihe @ embT (+bias), fp16 =======
            with tc.tile_pool(name="geps", bufs=1, space="PSUM") as geps:
                for half in range(2):
                    ps_ge = geps.tile([128, 8, 256], F32, tag="ge")
                    for jt8 in range(8):
                        jt = half * 8 + jt8
                        for ee in range(2):
                            nc.tensor.matmul(
                                ps_ge[:, jt8, 0:TB],
                                WieT_sb[:, ee, jt * 128:(jt + 1) * 128],
                                emb16[:, ee, :],
                                start=(ee == 0), stop=(ee == 1))
                    for jt8 in range(8):
                        jt = half * 8 + jt8
                        eng = nc.vector if jt8 % 2 == 0 else nc.scalar
                        if eng is nc.vector:
                            nc.vector.tensor_scalar_add(
                                ge_sb[:, jt, :], ps_ge[:, jt8, 0:TB],
                                bgT_sb[:, jt:jt + 1])
                        else:
                            nc.scalar.activation(
                                ge_sb[:, jt, :], ps_ge[:, jt8, 0:TB],
                                AF.Identity, bias=bgT_sb[:, jt:jt + 1])

        # =========== Phase C: feats_mean + keys + initial state ===========
        with tc.tile_pool(name="inip", bufs=1) as inip, \
             tc.tile_pool(name="inips", bufs=1, space="PSUM") as inips, \
             tc.tile_pool(name="keyps", bufs=2, space="PSUM") as keyps:
            fsum = inip.tile([128, 4, BL], F16)
            for cc in range(4):
                nc.vector.tensor_reduce(
                    fsum[:, cc, :],
                    featsT_sb[:, cc, :].rearrange("p (b k) -> p b k", b=BL),
                    axis=mybir.AxisListType.X, op=OP.add)
            # keys: out keysT [u-tile, (b,k)] ; evac into padded [u, b, KP]
            HHW = 4 * HW  # half the (b,k) range = 784
            for jt in range(4):
                for half in range(2):
                    ps_k = keyps.tile([128, HHW], F32, tag="k")
                    for cc in range(4):
                        for (n0, n1) in [(0, 512), (512, HHW)]:
                            nc.tensor.matmul(
                                ps_k[:, n0:n1],
                                WkT_sb[:, cc, jt * 128:(jt + 1) * 128],
                                featsT_sb[:, cc,
                                          half * HHW + n0:half * HHW + n1],
                                start=(cc == 0), stop=(cc == 3))
                    out_ap = keysT_sb[:, jt, half * 4:(half + 1) * 4, 0:HW]
                    in_ap = ps_k[:].rearrange("p (b k) -> p b k", b=4)
                    if (2 * jt + half) % 2 == 0:
                        nc.vector.tensor_scalar_add(out_ap, in_ap,
                                                    bkT_sb[:, jt:jt + 1])
                    else:
                        nc.scalar.activation(out_ap, in_ap, AF.Identity,
                                             bias=bkT_sb[:, jt:jt + 1])
                    # zero the k-padding (196..256) with a rounding store
                    nc.vector.tensor_scalar_mul(
                        keysT_sb[:, jt, half * 4:(half + 1) * 4, HW:KP],
                        ps_k[:, 0:4 * (KP - HW)]
                        .rearrange("p (b k) -> p b k", b=4),
                        0.0)
            # hid0 / cell0 (M-orientation, f32r)
            ps_i = inips.tile([128, 4, 128], F32, tag="i0")
            ps_c = inips.tile([128, 4, 128], F32, tag="c0")
            for jt in range(4):
                for cc in range(4):
                    nc.tensor.matmul(
                        ps_i[:, jt, 0:BL],
                        Wh0T_sb[:, cc, jt * 128:(jt + 1) * 128],
                        fsum[:, cc, :],
                        start=(cc == 0), stop=(cc == 3))
                    nc.tensor.matmul(
                        ps_c[:, jt, 0:BL],
                        Wc0T_sb[:, cc, jt * 128:(jt + 1) * 128],
                        fsum[:, cc, :],
                        start=(cc == 0), stop=(cc == 3))
            for jt in range(4):
                nc.vector.tensor_scalar_add(hidT_sb[:, jt, :],
                                            ps_i[:, jt, 0:BL],
                                            bh0T_sb[:, jt:jt + 1])
                nc.vector.tensor_scalar_add(cellT_sb[:, jt, :],
                                            ps_c[:, jt, 0:BL],
                                            bc0T_sb[:, jt:jt + 1])
            nc.vector.tensor_copy(xT16[:, 4:8, 0:BL], hidT_sb[:])
            nc.vector.tensor_scalar_mul(
                xT16[:, :, BL:], bh0T_sb[:, 0:1].unsqueeze(-1)
                .to_broadcast([128, 8, 32]), 0.0)

        early.__exit__(None, None, None)

        # prefetch all logits weight tiles (sync queue stays unblocked;
        # loads beyond the buffer count trickle in as logits consumes)
        lwp = ctx.enter_context(tc.tile_pool(name="lw", bufs=46))
        wtiles = []
        for mt in range(VT):
            wt = lwp.tile([128, 10, 128], F16, tag="wt", name=f"wt{mt}")
            nc.sync.dma_start(
                wt[:].rearrange("p kc m -> p (kc m)"), d_WoT.ap()[mt, :, :])
            wtiles.append(wt)

        # ======================= Phase D: the scan =======================
        with tc.tile_pool(name="scps", bufs=1, space="PSUM") as scps, \
             tc.tile_pool(name="scsb", bufs=2) as scsb:
            ps_sc2 = [scps.tile([128, 2, 512], F32, tag=f"sc{i}", name=f"ps_sc{i}")
                      for i in range(2)]
            nc.vector.memset(ps_sc2[0][:], 0.0)
            nc.vector.memset(ps_sc2[1][:], 0.0)
            ps_tr = scps.tile([128, 4, 128], F32)
            ps_aT = scps.tile([128, 4, BL], F32)
            ps_g2 = [scps.tile([128, 2, 16, BL], F32, tag=f"g{i}",
                               name=f"ps_g{i}") for i in range(2)]
            w_sb2 = [scsb.tile([128, 2, KP], F32, tag=f"w{i}", name=f"w_sb{i}")
                     for i in range(2)]
            nc.gpsimd.memset(w_sb2[0][:], 0.0)
            nc.gpsimd.memset(w_sb2[1][:], 0.0)

            for t in range(t_steps):
                ps_sc = ps_sc2[t % 2]
                ps_g = ps_g2[t % 2]

                # 1. scores (col-tiled, f32r): batch b=g*4+c -> psum part 32c
                for g in range(2):
                    for c in range(4):
                        b = g * 4 + c
                        for jj in range(4):
                            nc.tensor.matmul(
                                ps_sc[32 * c:32 * c + 32, g, 0:KP],
                                xT16[:, 4 + jj, b:b + 32],
                                keysT_sb[:, jj, b, :],
                                start=(jj == 0), stop=(jj == 3),
                                tile_position=(0, 32 * c))

                # 2. gates hid-part (can overlap with attention)
                for jt in range(16):
                    for kc in range(4, 8):
                        nc.tensor.matmul(
                            ps_g[:, 0, jt, :],
                            WahT_sb[:, kc, jt * 128:(jt + 1) * 128],
                            xT16[:, kc, 0:BL],
                            start=(kc == 4), stop=(kc == 7))

                # 3-6. per-group softmax -> transpose -> eT (g0 chain
                # overlaps g1 scores/exp on other engines)
                w_sb = w_sb2[t % 2]
                sume = scsb.tile([128, 2], F32, tag="sume")
                rinv = scsb.tile([128, 2], F32, tag="rinv")
                eT16 = scsb.tile([128, 4, 4], F16, tag="eT")
                for g in range(2):
                    nc.scalar.activation(
                        w_sb[:, g, 0:HW], ps_sc[:, g, 0:HW], AF.Exp,
                        scale=INV_SCALE, accum_out=sume[:, g:g + 1])
                    nc.vector.reciprocal(rinv[:, g:g + 1], sume[:, g:g + 1])
                    nc.vector.tensor_scalar_mul(
                        w_sb[:, g, 0:HW], w_sb[:, g, 0:HW], rinv[:, g:g + 1])
                    for h in range(2):
                        hh = g * 2 + h
                        nc.tensor.transpose(
                            ps_tr[:, hh, :],
                            w_sb[:, g, h * 128:(h + 1) * 128],
                            ident[:])
                    nc.vector.tensor_copy(
                        eT16[:, 2 * g:2 * g + 2, :],
                        ps_tr[:, 2 * g:2 * g + 2, 0:128:32])
                    nc.gpsimd.dma_start(
                        d_attn.ap()[g * 4:(g + 1) * 4, t, :],
                        w_sb[0:128:32, g, 0:HW])

                # 7. attention output aT[c,b] (fp16 feats as weights)
                for b in range(BL):
                    g, c = b // 4, b % 4
                    for cm in range(4):
                        for h in range(2):
                            nc.tensor.matmul(
                                ps_aT[:, cm, b:b + 1],
                                feats16_sb[:, b * 2 + h,
                                           cm * 128:(cm + 1) * 128],
                                eT16[:, g * 2 + h, c:c + 1],
                                start=(h == 0), stop=(h == 1))
                nc.vector.tensor_copy(xT16[:, 0:4, 0:BL], ps_aT[:])
                nc.vector.tensor_copy(outsT16[:, 4:8, t, :], ps_aT[:])

                # 8. gates a-part
                for jt in range(16):
                    for kc in range(0, 4):
                        nc.tensor.matmul(
                            ps_g[:, 1, jt, :],
                            WahT_sb[:, kc, jt * 128:(jt + 1) * 128],
                            xT16[:, kc, 0:BL],
                            start=(kc == 0), stop=(kc == 3))

                # 9. add hid-part + emb-part(with bias) + a-part
                gf = scsb.tile([128, 16, BL], F32, tag="gf")
                nc.vector.tensor_tensor(
                    out=gf[:], in0=ps_g[:, 0, :, :],
                    in1=ge_sb[:, :, t * BL:(t + 1) * BL], op=OP.add)
                nc.vector.tensor_tensor(
                    out=gf[:], in0=ps_g[:, 1, :, :], in1=gf[:], op=OP.add)

                # 10. LSTM pointwise on [128, (16jj, 8b)]
                gff = gf[:].rearrange("p jj b -> p (jj b)")
                pw = scsb.tile([128, 128], F32, tag="pw")
                pwf = pw[:]
                # sigmoid via tanh so the scan only ever needs the
                # exp+tanh ACT table (sigmoid lives in a different LUT set;
                # mixing would cost 2 x 1.28us table reloads per step)
                nc.scalar.activation(pwf[:, 0:96], gff[:, 0:96], AF.Tanh,
                                     scale=0.5)
                nc.vector.tensor_scalar(
                    out=pwf[:, 0:96], in0=pwf[:, 0:96],
                    scalar1=0.5, scalar2=0.5,
                    op0=OP.mult, op1=OP.add)
                nc.scalar.activation(pwf[:, 96:128], gff[:, 96:128], AF.Tanh)
                m1 = scsb.tile([128, 32], F32, tag="m1")
                m2 = scsb.tile([128, 32], F32, tag="m2")
                cellf = cellT_sb[:].rearrange("p jj b -> p (jj b)")
                hidf = hidT_sb[:].rearrange("p jj b -> p (jj b)")
                nc.vector.tensor_tensor(out=m1[:], in0=pwf[:, 32:64],
                                        in1=cellf, op=OP.mult)
                nc.vector.tensor_tensor(out=m2[:], in0=pwf[:, 0:32],
                                        in1=pwf[:, 96:128], op=OP.mult)
                nc.vector.tensor_tensor(out=cellf, in0=m1[:], in1=m2[:],
                                        op=OP.add)
                tc_ = scsb.tile([128, 32], F32, tag="tc")
                nc.scalar.activation(tc_[:], cellf, AF.Tanh)
                nc.vector.tensor_tensor(out=hidf, in0=pwf[:, 64:96],
                                        in1=tc_[:], op=OP.mult)
                nc.vector.tensor_copy(xT16[:, 4:8, 0:BL], hidT_sb[:])
                nc.vector.tensor_copy(outsT16[:, 0:4, t, :], hidT_sb[:])

        # ======================= Phase E: logits =======================
        with tc.tile_pool(name="lo", bufs=2) as lop, \
             tc.tile_pool(name="lps", bufs=4, space="PSUM") as lps:
            GRP = 8
            for mt0 in range(0, VT, GRP):
                n_mt = min(GRP, VT - mt0)
                stage = lop.tile([128, GRP, TB], F32, tag="lo",
                                 name=f"st{mt0}")
                for mi in range(n_mt):
                    mt = mt0 + mi
                    ps_l = lps.tile([128, 256], F32, tag="l",
                                    name=f"psl{mt}")
                    for kc in range(10):
                        nc.tensor.matmul(
                            ps_l[:, 0:TB], wtiles[mt][:, kc, :],
                            outsT16[:, kc, :, :],
                            start=(kc == 0), stop=(kc == 9))
                    if mt % 2 == 0:
                        nc.vector.tensor_scalar_add(
                            stage[:, mi, :], ps_l[:, 0:TB],
                            boT_sb[:, mt:mt + 1])
                    else:
                        nc.scalar.activation(
                            stage[:, mi, :], ps_l[:, 0:TB], AF.Identity,
                            bias=boT_sb[:, mt:mt + 1])
                nc.sync.dma_start(
                    d_logitsT.ap()[mt0:mt0 + n_mt, :, :]
                    .rearrange("m p n -> p m n"),
                    stage[:, 0:n_mt, :])

    nc.compile()
    return nc


def _prep_inputs(image_features, captions_ix, W_h0, b_h0, W_c0, b_c0, emb,
                 W_key, b_key, W_ih, b_ih, W_hh, b_hh, W_out, b_out,
                 t_steps=T):
    """Host-side sharding/layout. Returns list of per-core in_maps."""
    f32 = np.float32
    f16 = np.float16
    img = np.ascontiguousarray(np.asarray(image_features, dtype=f32))
    cap = np.asarray(captions_ix).astype(np.int32)[:, :t_steps]

    WkT = np.ascontiguousarray(np.asarray(W_key, f32).T.astype(f16))
    bkT = np.ascontiguousarray(np.asarray(b_key, f32).reshape(4, 128).T)
    Wh0T = np.ascontiguousarray((np.asarray(W_h0, f32).T / float(HW)).astype(f16))
    bh0T = np.ascontiguousarray(np.asarray(b_h0, f32).reshape(4, 128).T)
    Wc0T = np.ascontiguousarray((np.asarray(W_c0, f32).T / float(HW)).astype(f16))
    bc0T = np.ascontiguousarray(np.asarray(b_c0, f32).reshape(4, 128).T)
    W_ih = np.asarray(W_ih, f32)
    W_hh = np.asarray(W_hh, f32)
    gperm = np.r_[0:2 * U, 3 * U:4 * U, 2 * U:3 * U]   # (i,f,g,o)->(i,f,o,g)
    WieT = np.ascontiguousarray(W_ih[gperm, :E].T.astype(f16))
    WahT = np.ascontiguousarray(
        np.concatenate([W_ih[gperm, E:].T,
                        np.asarray(W_hh, f32)[gperm].T], axis=0).astype(f16))
    bgT = np.ascontiguousarray(
        (np.asarray(b_ih, f32) + np.asarray(b_hh, f32))[gperm]
        .reshape(16, 128).T)
    WoTf = np.zeros((OD, VP), f16)
    WoTf[:, :V] = np.asarray(W_out, f32).T.astype(f16)
    # per-tile layout [VT, 128, (kc, m)]: partition p's whole SBUF row is one
    # contiguous 2.5KB DRAM run per tile
    WoT = np.ascontiguousarray(
        WoTf.reshape(10, 128, VT, 128).transpose(2, 1, 0, 3)
        .reshape(VT, 128, OD))
    boT = np.zeros((VP,), f32)
    boT[:V] = np.asarray(b_out, f32)
    boT = np.ascontiguousarray(boT.reshape(VT, 128).T)
    embf = np.ascontiguousarray(np.asarray(emb, f32))

    in_maps = []
    for ci in range(NCORES):
        sl = slice(ci * BL, (ci + 1) * BL)
        img_l = img[sl]                                   # [BL, C, HW]
        featsT = np.ascontiguousarray(
            img_l.transpose(1, 0, 2).reshape(C, BL * HW).astype(f16))
        fp = np.zeros((BL, KP, C), f32)
        fp[:, :HW, :] = img_l.transpose(0, 2, 1)
        feats16 = np.ascontiguousarray(fp.reshape(BL * KP, C).astype(f16))
        emb_rows = np.ascontiguousarray(
            embf[cap[sl].T.reshape(t_steps * BL)])        # row r = t*BL+b
        in_maps.append({
            "featsT": featsT, "feats16": feats16, "emb": emb_rows,
            "WkT": WkT, "bkT": bkT,
            "Wh0T": Wh0T, "bh0T": bh0T, "Wc0T": Wc0T, "bc0T": bc0T,
            "WieT": WieT, "WahT": WahT, "bgT": bgT,
            "WoT": WoT, "boT": boT,
        })
    return in_maps


def _assemble(results, t_steps=T):
    logits = np.empty((B, t_steps, V), np.float32)
    attn = np.empty((B, t_steps, HW), np.float32)
    for ci, r in enumerate(results):
        lt = r["logitsT"].reshape(VP, t_steps * BL)[:V]   # [V, (t,b)]
        logits[ci * BL:(ci + 1) * BL] = (
            lt.reshape(V, t_steps, BL).transpose(2, 1, 0))
        attn[ci * BL:(ci + 1) * BL] = r["attn"]
    return logits, attn


def kernel(**inputs):
    if "nc" not in _CACHE:
        _CACHE["nc"] = _build(T)
    nc = _CACHE["nc"]
    in_maps = _prep_inputs(**inputs)
    res = run_bass_kernel_spmd(nc, in_maps, core_ids=list(range(NCORES)))
    return _assemble(res.results)


# revision 44
# speedup vs baseline: 1.5140x; 1.0462x over previous
"""CaptionNet Trainium2 kernel (Bass/Tile, 8-core SPMD, batch-sharded).

Strategy:
- Batch-parallel over 8 NeuronCores (8 local batches/core). No collectives;
  the host shards inputs and concatenates outputs.
- LSTM state kept transposed ([feature-part, batch-free]) so pointwise ops
  use all 128 DVE lanes and the recurrent matmuls run weight-stationary.
- Attention scores via 4-way column-tiled PE matmuls (one batch per 32-wide
  column group; fp16 operands — 4-byte dtypes cannot column-tile).
- hid0/cell0 projections use float32r (fp32 bits, 1 cycle/col at N>=256);
  weight-heavy matmuls (gates / attention values / logits) use fp16
  weights+activations (~5e-4 rel err) to halve PE ingest bytes.
- Softmax: exp(scale*x) with fused accumulate sum, no max-subtraction
  (scores are small by construction); 1/sum applied as per-row scalars.
- All 79 logits weight tiles are prefetched on the sync DMA queue during the
  scan (contiguous per-tile DRAM layout, prepared host-side).
"""
import numpy as np

import concourse.bass as bass
import concourse.tile as tile
from concourse import bacc, mybir
from concourse.bass_utils import run_bass_kernel_spmd
from concourse.masks import make_identity
from contextlib import ExitStack

F32 = mybir.dt.float32
F32R = mybir.dt.float32r
F16 = mybir.dt.float16
I32 = mybir.dt.int32
AF = mybir.ActivationFunctionType
OP = mybir.AluOpType

B, T, C, HW, V, E, U = 64, 20, 512, 196, 10000, 256, 512
NCORES = 8
BL = B // NCORES          # 8 batches per core
KP = 256                  # per-batch padded attention length (HW=196 -> 256)
G4 = 4 * U                # 2048 gate width
OD = U + C + E            # 1280 outs feature dim
VT = (V + 127) // 128     # 79 vocab M-tiles
VP = VT * 128             # 10112 padded vocab
INV_SCALE = 1.0 / float(U) ** 0.5

_CACHE = {}


def _build(t_steps=T):
    """Build + compile the per-core Bass program. Returns nc."""
    nc = bacc.Bacc("TRN2", target_bir_lowering=False, debug=False,
                   num_devices=NCORES)
    TB = t_steps * BL

    # ---- DRAM I/O ----
    d_featsT = nc.dram_tensor("featsT", [C, BL * HW], F16, kind="ExternalInput")
    d_feats16 = nc.dram_tensor("feats16", [BL * KP, C], F16, kind="ExternalInput")
    d_emb = nc.dram_tensor("emb", [TB, E], F32, kind="ExternalInput")
    d_WkT = nc.dram_tensor("WkT", [C, U], F16, kind="ExternalInput")
    d_bkT = nc.dram_tensor("bkT", [128, 4], F32, kind="ExternalInput")
    d_Wh0T = nc.dram_tensor("Wh0T", [C, U], F16, kind="ExternalInput")
    d_bh0T = nc.dram_tensor("bh0T", [128, 4], F32, kind="ExternalInput")
    d_Wc0T = nc.dram_tensor("Wc0T", [C, U], F16, kind="ExternalInput")
    d_bc0T = nc.dram_tensor("bc0T", [128, 4], F32, kind="ExternalInput")
    d_WieT = nc.dram_tensor("WieT", [E, G4], F16, kind="ExternalInput")
    d_WahT = nc.dram_tensor("WahT", [C + U, G4], F16, kind="ExternalInput")
    d_bgT = nc.dram_tensor("bgT", [128, 16], F32, kind="ExternalInput")
    d_WoT = nc.dram_tensor("WoT", [VT, 128, OD], F16, kind="ExternalInput")
    d_boT = nc.dram_tensor("boT", [128, VT], F32, kind="ExternalInput")

    d_logitsT = nc.dram_tensor("logitsT", [VT, 128, TB], F32, kind="ExternalOutput")
    d_attn = nc.dram_tensor("attn", [BL, t_steps, HW], F32, kind="ExternalOutput")

    with tile.TileContext(nc) as tc, ExitStack() as ctx:
        ctx.enter_context(nc.allow_low_precision(
            reason="float32r stores feed f32r matmuls by design"))
        per = ctx.enter_context(tc.tile_pool(name="per", bufs=1))

        # ---------- persistent SBUF ----------
        ident = per.tile([128, 128], F32)
        make_identity(nc, ident[:])

        early = tc.tile_pool(name="early", bufs=1)
        earlyp = early.__enter__()
        featsT_sb = earlyp.tile([128, 4, BL * HW], F16)
        for _cc in range(4):
            nc.sync.dma_start(
                featsT_sb[:, _cc, :],
                d_featsT.ap().rearrange("(cc p) n -> p cc n", p=128)[:, _cc, :])
        WkT_sb = earlyp.tile([128, 4, U], F16)
        nc.sync.dma_start(WkT_sb[:],
                          d_WkT.ap().rearrange("(cc p) u -> p cc u", p=128))
        Wh0T_sb = earlyp.tile([128, 4, U], F16)
        nc.sync.dma_start(Wh0T_sb[:],
                          d_Wh0T.ap().rearrange("(cc p) u -> p cc u", p=128))
        Wc0T_sb = earlyp.tile([128, 4, U], F16)
        nc.sync.dma_start(Wc0T_sb[:],
                          d_Wc0T.ap().rearrange("(cc p) u -> p cc u", p=128))
        WieT_sb = earlyp.tile([128, 2, G4], F16)
        nc.sync.dma_start(WieT_sb[:],
                          d_WieT.ap().rearrange("(ee p) j -> p ee j", p=128))
        bkT_sb = per.tile([128, 4], F32)
        nc.sync.dma_start(bkT_sb[:], d_bkT.ap())
        bh0T_sb = per.tile([128, 4], F32)
        nc.sync.dma_start(bh0T_sb[:], d_bh0T.ap())
        bc0T_sb = per.tile([128, 4], F32)
        nc.sync.dma_start(bc0T_sb[:], d_bc0T.ap())
        bgT_sb = per.tile([128, 16], F32)
        nc.sync.dma_start(bgT_sb[:], d_bgT.ap())
        boT_sb = per.tile([128, VT], F32)
        nc.sync.dma_start(boT_sb[:], d_boT.ap())
        feats16_sb = per.tile([128, 2 * BL, C], F16)
        nc.sync.dma_start(feats16_sb[:],
                          d_feats16.ap().rearrange("(bh p) c -> p bh c", p=128))
        WahT_sb = per.tile([128, 8, G4], F16)
        nc.sync.dma_start(WahT_sb[:],
                          d_WahT.ap().rearrange("(kc p) j -> p kc j", p=128))

        keysT_sb = per.tile([128, 4, BL, KP], F16)   # padded [u, b, k]
        ge_sb = per.tile([128, 16, TB], F32)         # emb-part of gates + bias
        embT_sb = per.tile([128, 2, TB], F32)
        outsT16 = per.tile([128, 10, t_steps, BL], F16)

        hidT_sb = per.tile([128, 4, BL], F32)
        cellT_sb = per.tile([128, 4, BL], F32)
        # kc 0-3 = aT, 4-7 = hidT; 40-wide so score matmuls can take 32-col
        # lhsT slices (cols 8..39 zeroed; junk rows land on unused partitions)
        xT16 = per.tile([128, 8, BL + 32], F16)

        # =========== Phase A: embedding gather + renorm + transpose =========
        with tc.tile_pool(name="embp", bufs=1) as embp, \
             tc.tile_pool(name="embps", bufs=1, space="PSUM") as embps:
            segs = [(0, min(128, TB))]
            if TB > 128:
                segs.append((128, TB))
            ps_e = embps.tile([128, 4, 128], F32)
            for si, (r0, r1) in enumerate(segs):
                npart, off = r1 - r0, r0
                g = embp.tile([npart, E], F32, tag=f"eg{si}")
                nc.sync.dma_start(g[:], d_emb.ap()[r0:r1, :])
                sq = embp.tile([npart, E], F32, tag=f"sq{si}")
                n2 = embp.tile([npart, 1], F32, tag=f"n2{si}")
                nc.scalar.activation(sq[:], g[:], AF.Square,
                                     accum_out=n2[:, :1])
                nrm = embp.tile([npart, 1], F32, tag=f"nr{si}")
                nc.scalar.sqrt(nrm[:], n2[:])
                nc.vector.tensor_scalar_max(nrm[:], nrm[:], 1e-12)
                inv = embp.tile([npart, 1], F32, tag=f"iv{si}")
                nc.vector.reciprocal(inv[:], nrm[:])
                nc.vector.tensor_scalar_mul(inv[:], inv[:], 5.0)
                nc.vector.tensor_scalar_min(inv[:], inv[:], 1.0)
                nc.vector.tensor_scalar_mul(g[:], g[:], inv[:, :1])
                # transpose [npart, 256] -> embT [256, npart]
                for ee in range(2):
                    nc.tensor.transpose(
                        ps_e[:, 2 * si + ee, 0:npart],
                        g[:, ee * 128:(ee + 1) * 128],
                        ident[0:npart, 0:npart])
                    nc.vector.tensor_copy(
                        embT_sb[:, ee, off:off + npart],
                        ps_e[:, 2 * si + ee, 0:npart])
            # fp16 copy into outsT16 emb rows (kc 8,9)
            nc.vector.tensor_copy(
                outsT16[:, 8:10, :, :],
                embT_sb[:].rearrange("p ee (t b) -> p ee t b", b=BL))
            emb16 = embp.tile([128, 2, TB], F16)
            nc.vector.tensor_copy(emb16[:], embT_sb[:])

            # ======= Phase B: gates_e = W_ihe @ embT (+bias), fp16 =======
            with tc.tile_pool(name="geps", bufs=1, space="PSUM") as geps:
                for half in range(2):
                    ps_ge = geps.tile([128, 8, 256], F32, tag="ge")
                    for jt8 in range(8):
                        jt = half * 8 + jt8
                        for ee in range(2):
                            nc.tensor.matmul(
                                ps_ge[:, jt8, 0:TB],
                                WieT_sb[:, ee, jt * 128:(jt + 1) * 128],
                                emb16[:, ee, :],
                                start=(ee == 0), stop=(ee == 1))
                    for jt8 in range(8):
                        jt = half * 8 + jt8
                        eng = nc.vector if jt8 % 2 == 0 else nc.scalar
                        if eng is nc.vector:
                            nc.vector.tensor_scalar_add(
                                ge_sb[:, jt, :], ps_ge[:, jt8, 0:TB],
                                bgT_sb[:, jt:jt + 1])
                        else:
                            nc.scalar.activation(
                                ge_sb[:, jt, :], ps_ge[:, jt8, 0:TB],
                                AF.Identity, bias=bgT_sb[:, jt:jt + 1])

        # =========== Phase C: feats_mean + keys + initial state ===========
        with tc.tile_pool(name="inip", bufs=1) as inip, \
             tc.tile_pool(name="inips", bufs=1, space="PSUM") as inips, \
             tc.tile_pool(name="keyps", bufs=2, space="PSUM") as keyps:
            fsum = inip.tile([128, 4, BL], F16)
            for cc in range(4):
                nc.vector.tensor_reduce(
                    fsum[:, cc, :],
                    featsT_sb[:, cc, :].rearrange("p (b k) -> p b k", b=BL),
                    axis=mybir.AxisListType.X, op=OP.add)
            # keys: out keysT [u-tile, (b,k)] ; evac into padded [u, b, KP]
            HHW = 4 * HW  # half the (b,k) range = 784
            for jt in range(4):
                for half in range(2):
                    ps_k = keyps.tile([128, HHW], F32, tag="k")
                    for cc in range(4):
                        for (n0, n1) in [(0, 512), (512, HHW)]:
                            nc.tensor.matmul(
                                ps_k[:, n0:n1],
                                WkT_sb[:, cc, jt * 128:(jt + 1) * 128],
                                featsT_sb[:, cc,
                                          half * HHW + n0:half * HHW + n1],
                                start=(cc == 0), stop=(cc == 3))
                    out_ap = keysT_sb[:, jt, half * 4:(half + 1) * 4, 0:HW]
                    in_ap = ps_k[:].rearrange("p (b k) -> p b k", b=4)
                    if (2 * jt + half) % 2 == 0:
                        nc.vector.tensor_scalar_add(out_ap, in_ap,
                                                    bkT_sb[:, jt:jt + 1])
                    else:
                        nc.scalar.activation(out_ap, in_ap, AF.Identity,
                                             bias=bkT_sb[:, jt:jt + 1])
                    # zero the k-padding (196..256) with a rounding store
                    nc.vector.tensor_scalar_mul(
                        keysT_sb[:, jt, half * 4:(half + 1) * 4, HW:KP],
                        ps_k[:, 0:4 * (KP - HW)]
                        .rearrange("p (b k) -> p b k", b=4),
                        0.0)
            # hid0 / cell0 (M-orientation, f32r)
            ps_i = inips.tile([128, 4, 128], F32, tag="i0")
            ps_c = inips.tile([128, 4, 128], F32, tag="c0")
            for jt in range(4):
                for cc in range(4):
                    nc.tensor.matmul(
                        ps_i[:, jt, 0:BL],
                        Wh0T_sb[:, cc, jt * 128:(jt + 1) * 128],
                        fsum[:, cc, :],
                        start=(cc == 0), stop=(cc == 3))
                    nc.tensor.matmul(
                        ps_c[:, jt, 0:BL],
                        Wc0T_sb[:, cc, jt * 128:(jt + 1) * 128],
                        fsum[:, cc, :],
                        start=(cc == 0), stop=(cc == 3))
            for jt in range(4):
                nc.vector.tensor_scalar_add(hidT_sb[:, jt, :],
                                            ps_i[:, jt, 0:BL],
                                            bh0T_sb[:, jt:jt + 1])
                nc.vector.tensor_scalar_add(cellT_sb[:, jt, :],
                                            ps_c[:, jt, 0:BL],
                                            bc0T_sb[:, jt:jt + 1])
            nc.vector.tensor_copy(xT16[:, 4:8, 0:BL], hidT_sb[:])
            nc.vector.tensor_scalar_mul(
                xT16[:, :, BL:], bh0T_sb[:, 0:1].unsqueeze(-1)
                .to_broadcast([128, 8, 32]), 0.0)

        early.__exit__(None, None, None)

        # prefetch all logits weight tiles (sync queue stays unblocked;
        # loads beyond the buffer count trickle in as logits consumes)
        lwp = ctx.enter_context(tc.tile_pool(name="lw", bufs=46))
        wtiles = []
        for mt in range(VT):
            wt = lwp.tile([128, 10, 128], F16, tag="wt", name=f"wt{mt}")
            nc.sync.dma_start(
                wt[:].rearrange("p kc m -> p (kc m)"), d_WoT.ap()[mt, :, :])
            wtiles.append(wt)

        # ======================= Phase D: the scan =======================
        with tc.tile_pool(name="scps", bufs=1, space="PSUM") as scps, \
             tc.tile_pool(name="scsb", bufs=2) as scsb:
            ps_sc2 = [scps.tile([128, 2, 512], F32, tag=f"sc{i}", name=f"ps_sc{i}")
                      for i in range(2)]
            nc.vector.memset(ps_sc2[0][:], 0.0)
            nc.vector.memset(ps_sc2[1][:], 0.0)
            ps_tr = scps.tile([128, 4, 128], F32)
            ps_aT = scps.tile([128, 4, BL], F32)
            ps_g2 = [scps.tile([128, 2, 16, BL], F32, tag=f"g{i}",
                               name=f"ps_g{i}") for i in range(2)]
            w_sb2 = [scsb.tile([128, 2, KP], F32, tag=f"w{i}", name=f"w_sb{i}")
                     for i in range(2)]
            nc.gpsimd.memset(w_sb2[0][:], 0.0)
            nc.gpsimd.memset(w_sb2[1][:], 0.0)

            for t in range(t_steps):
                ps_sc = ps_sc2[t % 2]
                ps_g = ps_g2[t % 2]

                # 1. scores (col-tiled, f32r): batch b=g*4+c -> psum part 32c
                for g in range(2):
                    for c in range(4):
                        b = g * 4 + c
                        for jj in range(4):
                            nc.tensor.matmul(
                                ps_sc[32 * c:32 * c + 32, g, 0:HW],
                                xT16[:, 4 + jj, b:b + 32],
                                keysT_sb[:, jj, b, 0:HW],
                                start=(jj == 0), stop=(jj == 3),
                                tile_position=(0, 32 * c))

                # 2. gates hid-part (can overlap with attention)
                for jt in range(16):
                    for kc in range(4, 8):
                        nc.tensor.matmul(
                            ps_g[:, 0, jt, :],
                            WahT_sb[:, kc, jt * 128:(jt + 1) * 128],
                            xT16[:, kc, 0:BL],
                            start=(kc == 4), stop=(kc == 7))

                # 3-6. per-group softmax -> transpose -> eT (g0 chain
                # overlaps g1 scores/exp on other engines)
                w_sb = w_sb2[t % 2]
                sume = scsb.tile([128, 2], F32, tag="sume")
                eT16 = scsb.tile([128, 4, 4], F16, tag="eT")
                rinv = scsb.tile([128, 2], F32, tag="rinv")
                for g in range(2):
                    nc.scalar.activation(
                        w_sb[:, g, 0:HW], ps_sc[:, g, 0:HW], AF.Exp,
                        scale=INV_SCALE, accum_out=sume[:, g:g + 1])
                    nc.vector.reciprocal(rinv[:, g:g + 1], sume[:, g:g + 1])
                    nc.vector.tensor_scalar_mul(
                        w_sb[:, g, 0:HW], w_sb[:, g, 0:HW], rinv[:, g:g + 1])
                    for h in range(2):
                        hh = g * 2 + h
                        nc.tensor.transpose(
                            ps_tr[:, hh, :],
                            w_sb[:, g, h * 128:(h + 1) * 128],
                            ident[:])
                    nc.vector.tensor_copy(
                        eT16[:, 2 * g:2 * g + 2, :],
                        ps_tr[:, 2 * g:2 * g + 2, 0:128:32])
                    nc.gpsimd.dma_start(
                        d_attn.ap()[g * 4:(g + 1) * 4, t, :],
                        w_sb[0:128:32, g, 0:HW])

                # 7. attention output aT[c,b] (fp16 feats as weights)
                for b in range(BL):
                    g, c = b // 4, b % 4
                    for cm in range(4):
                        for h in range(2):
                            nc.tensor.matmul(
                                ps_aT[:, cm, b:b + 1],
                                feats16_sb[:, b * 2 + h,
                                           cm * 128:(cm + 1) * 128],
                                eT16[:, g * 2 + h, c:c + 1],
                                start=(h == 0), stop=(h == 1))
                nc.vector.tensor_copy(xT16[:, 0:4, 0:BL], ps_aT[:])
                nc.vector.tensor_copy(outsT16[:, 4:8, t, :], ps_aT[:])

                # 8. gates a-part
                for jt in range(16):
                    for kc in range(0, 4):
                        nc.tensor.matmul(
                            ps_g[:, 1, jt, :],
                            WahT_sb[:, kc, jt * 128:(jt + 1) * 128],
                            xT16[:, kc, 0:BL],
                            start=(kc == 0), stop=(kc == 3))

                # 9. add hid-part + emb-part(with bias) + a-part
                gf = scsb.tile([128, 16, BL], F32, tag="gf")
                nc.vector.tensor_tensor(
                    out=gf[:], in0=ps_g[:, 0, :, :],
                    in1=ge_sb[:, :, t * BL:(t + 1) * BL], op=OP.add)
                nc.vector.tensor_tensor(
                    out=gf[:], in0=ps_g[:, 1, :, :], in1=gf[:], op=OP.add)

                # 10. LSTM pointwise on [128, (16jj, 8b)]
                gff = gf[:].rearrange("p jj b -> p (jj b)")
                pw = scsb.tile([128, 128], F32, tag="pw")
                pwf = pw[:]
                # sigmoid via tanh so the scan only ever needs the
                # exp+tanh ACT table (sigmoid lives in a different LUT set;
                # mixing would cost 2 x 1.28us table reloads per step)
                nc.scalar.activation(pwf[:, 0:96], gff[:, 0:96], AF.Tanh,
                                     scale=0.5)
                nc.vector.tensor_scalar(
                    out=pwf[:, 0:96], in0=pwf[:, 0:96],
                    scalar1=0.5, scalar2=0.5,
                    op0=OP.mult, op1=OP.add)
                nc.scalar.activation(pwf[:, 96:128], gff[:, 96:128], AF.Tanh)
                m1 = scsb.tile([128, 32], F32, tag="m1")
                m2 = scsb.tile([128, 32], F32, tag="m2")
                cellf = cellT_sb[:].rearrange("p jj b -> p (jj b)")
                hidf = hidT_sb[:].rearrange("p jj b -> p (jj b)")
                nc.vector.tensor_tensor(out=m1[:], in0=pwf[:, 32:64],
                                        in1=cellf, op=OP.mult)
                nc.vector.tensor_tensor(out=m2[:], in0=pwf[:, 0:32],
                                        in1=pwf[:, 96:128], op=OP.mult)
                nc.vector.tensor_tensor(out=cellf, in0=m1[:], in1=m2[:],
                                        op=OP.add)
                tc_ = scsb.tile([128, 32], F32, tag="tc")
                nc.scalar.activation(tc_[:], cellf, AF.Tanh)
                nc.vector.tensor_tensor(out=hidf, in0=pwf[:, 64:96],
                                        in1=tc_[:], op=OP.mult)
                nc.vector.tensor_copy(xT16[:, 4:8, 0:BL], hidT_sb[:])
                nc.vector.tensor_copy(outsT16[:, 0:4, t, :], hidT_sb[:])

        # ======================= Phase E: logits =======================
        with tc.tile_pool(name="lo", bufs=2) as lop, \
             tc.tile_pool(name="lps", bufs=4, space="PSUM") as lps:
            GRP = 8
            for mt0 in range(0, VT, GRP):
                n_mt = min(GRP, VT - mt0)
                stage = lop.tile([128, GRP, TB], F32, tag="lo",
                                 name=f"st{mt0}")
                for mi in range(n_mt):
                    mt = mt0 + mi
                    ps_l = lps.tile([128, 256], F32, tag="l",
                                    name=f"psl{mt}")
                    for kc in range(10):
                        nc.tensor.matmul(
                            ps_l[:, 0:TB], wtiles[mt][:, kc, :],
                            outsT16[:, kc, :, :],
                            start=(kc == 0), stop=(kc == 9))
                    if mt % 2 == 0:
                        nc.vector.tensor_scalar_add(
                            stage[:, mi, :], ps_l[:, 0:TB],
                            boT_sb[:, mt:mt + 1])
                    else:
                        nc.scalar.activation(
                            stage[:, mi, :], ps_l[:, 0:TB], AF.Identity,
                            bias=boT_sb[:, mt:mt + 1])
                nc.sync.dma_start(
                    d_logitsT.ap()[mt0:mt0 + n_mt, :, :]
                    .rearrange("m p n -> p m n"),
                    stage[:, 0:n_mt, :])

    nc.compile()
    return nc


def _prep_inputs(image_features, captions_ix, W_h0, b_h0, W_c0, b_c0, emb,
                 W_key, b_key, W_ih, b_ih, W_hh, b_hh, W_out, b_out,
                 t_steps=T):
    """Host-side sharding/layout. Returns list of per-core in_maps."""
    f32 = np.float32
    f16 = np.float16
    img = np.ascontiguousarray(np.asarray(image_features, dtype=f32))
    cap = np.asarray(captions_ix).astype(np.int32)[:, :t_steps]

    WkT = np.ascontiguousarray(np.asarray(W_key, f32).T.astype(f16))
    bkT = np.ascontiguousarray(np.asarray(b_key, f32).reshape(4, 128).T)
    Wh0T = np.ascontiguousarray((np.asarray(W_h0, f32).T / float(HW)).astype(f16))
    bh0T = np.ascontiguousarray(np.asarray(b_h0, f32).reshape(4, 128).T)
    Wc0T = np.ascontiguousarray((np.asarray(W_c0, f32).T / float(HW)).astype(f16))
    bc0T = np.ascontiguousarray(np.asarray(b_c0, f32).reshape(4, 128).T)
    W_ih = np.asarray(W_ih, f32)
    W_hh = np.asarray(W_hh, f32)
    gperm = np.r_[0:2 * U, 3 * U:4 * U, 2 * U:3 * U]   # (i,f,g,o)->(i,f,o,g)
    WieT = np.ascontiguousarray(W_ih[gperm, :E].T.astype(f16))
    WahT = np.ascontiguousarray(
        np.concatenate([W_ih[gperm, E:].T,
                        np.asarray(W_hh, f32)[gperm].T], axis=0).astype(f16))
    bgT = np.ascontiguousarray(
        (np.asarray(b_ih, f32) + np.asarray(b_hh, f32))[gperm]
        .reshape(16, 128).T)
    WoTf = np.zeros((OD, VP), f16)
    WoTf[:, :V] = np.asarray(W_out, f32).T.astype(f16)
    # per-tile layout [VT, 128, (kc, m)]: partition p's whole SBUF row is one
    # contiguous 2.5KB DRAM run per tile
    WoT = np.ascontiguousarray(
        WoTf.reshape(10, 128, VT, 128).transpose(2, 1, 0, 3)
        .reshape(VT, 128, OD))
    boT = np.zeros((VP,), f32)
    boT[:V] = np.asarray(b_out, f32)
    boT = np.ascontiguousarray(boT.reshape(VT, 128).T)
    embf = np.ascontiguousarray(np.asarray(emb, f32))

    in_maps = []
    for ci in range(NCORES):
        sl = slice(ci * BL, (ci + 1) * BL)
        img_l = img[sl]                                   # [BL, C, HW]
        featsT = np.ascontiguousarray(
            img_l.transpose(1, 0, 2).reshape(C, BL * HW).astype(f16))
        fp = np.zeros((BL, KP, C), f32)
        fp[:, :HW, :] = img_l.transpose(0, 2, 1)
        feats16 = np.ascontiguousarray(fp.reshape(BL * KP, C).astype(f16))
        emb_rows = np.ascontiguousarray(
            embf[cap[sl].T.reshape(t_steps * BL)])        # row r = t*BL+b
        in_maps.append({
            "featsT": featsT, "feats16": feats16, "emb": emb_rows,
            "WkT": WkT, "bkT": bkT,
            "Wh0T": Wh0T, "bh0T": bh0T, "Wc0T": Wc0T, "bc0T": bc0T,
            "WieT": WieT, "WahT": WahT, "bgT": bgT,
            "WoT": WoT, "boT": boT,
        })
    return in_maps


def _assemble(results, t_steps=T):
    logits = np.empty((B, t_steps, V), np.float32)
    attn = np.empty((B, t_steps, HW), np.float32)
    for ci, r in enumerate(results):
        lt = r["logitsT"].reshape(VP, t_steps * BL)[:V]   # [V, (t,b)]
        logits[ci * BL:(ci + 1) * BL] = (
            lt.reshape(V, t_steps, BL).transpose(2, 1, 0))
        attn[ci * BL:(ci + 1) * BL] = r["attn"]
    return logits, attn


def kernel(**inputs):
    if "nc" not in _CACHE:
        _CACHE["nc"] = _build(T)
    nc = _CACHE["nc"]
    in_maps = _prep_inputs(**inputs)
    res = run_bass_kernel_spmd(nc, in_maps, core_ids=list(range(NCORES)))
    return _assemble(res.results)


# revision 47
# speedup vs baseline: 1.5222x; 1.0055x over previous
"""CaptionNet Trainium2 kernel (Bass/Tile, 8-core SPMD, batch-sharded).

Strategy:
- Batch-parallel over 8 NeuronCores (8 local batches/core). No collectives;
  the host shards inputs and concatenates outputs.
- LSTM state kept transposed ([feature-part, batch-free]) so pointwise ops
  use all 128 DVE lanes and the recurrent matmuls run weight-stationary.
- Attention scores via 4-way column-tiled PE matmuls (one batch per 32-wide
  column group; fp16 operands — 4-byte dtypes cannot column-tile).
- hid0/cell0 projections use float32r (fp32 bits, 1 cycle/col at N>=256);
  weight-heavy matmuls (gates / attention values / logits) use fp16
  weights+activations (~5e-4 rel err) to halve PE ingest bytes.
- Softmax: exp(scale*x) with fused accumulate sum, no max-subtraction
  (scores are small by construction); 1/sum applied as per-row scalars.
- All 79 logits weight tiles are prefetched on the sync DMA queue during the
  scan (contiguous per-tile DRAM layout, prepared host-side).
"""
import numpy as np

import concourse.bass as bass
import concourse.tile as tile
from concourse import bacc, mybir
from concourse.bass_utils import run_bass_kernel_spmd
from concourse.masks import make_identity
from contextlib import ExitStack

F32 = mybir.dt.float32
F32R = mybir.dt.float32r
F16 = mybir.dt.float16
I32 = mybir.dt.int32
AF = mybir.ActivationFunctionType
OP = mybir.AluOpType

B, T, C, HW, V, E, U = 64, 20, 512, 196, 10000, 256, 512
NCORES = 8
BL = B // NCORES          # 8 batches per core
KP = 256                  # per-batch padded attention length (HW=196 -> 256)
G4 = 4 * U                # 2048 gate width
OD = U + C + E            # 1280 outs feature dim
VT = (V + 127) // 128     # 79 vocab M-tiles
VP = VT * 128             # 10112 padded vocab
INV_SCALE = 1.0 / float(U) ** 0.5

_CACHE = {}


def _build(t_steps=T):
    """Build + compile the per-core Bass program. Returns nc."""
    nc = bacc.Bacc("TRN2", target_bir_lowering=False, debug=False,
                   num_devices=NCORES)
    TB = t_steps * BL

    # ---- DRAM I/O ----
    d_featsT = nc.dram_tensor("featsT", [C, BL * HW], F16, kind="ExternalInput")
    d_feats16 = nc.dram_tensor("feats16", [BL * KP, C], F16, kind="ExternalInput")
    d_emb = nc.dram_tensor("emb", [TB, E], F32, kind="ExternalInput")
    d_WkT = nc.dram_tensor("WkT", [C, U], F16, kind="ExternalInput")
    d_bkT = nc.dram_tensor("bkT", [128, 4], F32, kind="ExternalInput")
    d_Wh0T = nc.dram_tensor("Wh0T", [C, U], F16, kind="ExternalInput")
    d_bh0T = nc.dram_tensor("bh0T", [128, 4], F32, kind="ExternalInput")
    d_Wc0T = nc.dram_tensor("Wc0T", [C, U], F16, kind="ExternalInput")
    d_bc0T = nc.dram_tensor("bc0T", [128, 4], F32, kind="ExternalInput")
    d_WieT = nc.dram_tensor("WieT", [E, G4], F16, kind="ExternalInput")
    d_WahT = nc.dram_tensor("WahT", [C + U, G4], F16, kind="ExternalInput")
    d_bgT = nc.dram_tensor("bgT", [128, 16], F32, kind="ExternalInput")
    d_WoT = nc.dram_tensor("WoT", [VT, 128, OD], F16, kind="ExternalInput")
    d_boT = nc.dram_tensor("boT", [128, VT], F32, kind="ExternalInput")

    d_logitsT = nc.dram_tensor("logitsT", [VT, 128, TB], F32, kind="ExternalOutput")
    d_attn = nc.dram_tensor("attn", [BL, t_steps, HW], F32, kind="ExternalOutput")

    with tile.TileContext(nc) as tc, ExitStack() as ctx:
        ctx.enter_context(nc.allow_low_precision(
            reason="float32r stores feed f32r matmuls by design"))
        per = ctx.enter_context(tc.tile_pool(name="per", bufs=1))

        # ---------- persistent SBUF ----------
        ident = per.tile([128, 128], F32)
        make_identity(nc, ident[:])

        early = tc.tile_pool(name="early", bufs=1)
        earlyp = early.__enter__()
        featsT_sb = earlyp.tile([128, 4, BL * HW], F16)
        for _cc in range(4):
            nc.sync.dma_start(
                featsT_sb[:, _cc, :],
                d_featsT.ap().rearrange("(cc p) n -> p cc n", p=128)[:, _cc, :])
        WkT_sb = earlyp.tile([128, 4, U], F16)
        nc.sync.dma_start(WkT_sb[:],
                          d_WkT.ap().rearrange("(cc p) u -> p cc u", p=128))
        Wh0T_sb = earlyp.tile([128, 4, U], F16)
        nc.sync.dma_start(Wh0T_sb[:],
                          d_Wh0T.ap().rearrange("(cc p) u -> p cc u", p=128))
        Wc0T_sb = earlyp.tile([128, 4, U], F16)
        nc.sync.dma_start(Wc0T_sb[:],
                          d_Wc0T.ap().rearrange("(cc p) u -> p cc u", p=128))
        WieT_sb = earlyp.tile([128, 2, G4], F16)
        nc.sync.dma_start(WieT_sb[:],
                          d_WieT.ap().rearrange("(ee p) j -> p ee j", p=128))
        bkT_sb = per.tile([128, 4], F32)
        nc.sync.dma_start(bkT_sb[:], d_bkT.ap())
        bh0T_sb = per.tile([128, 4], F32)
        nc.sync.dma_start(bh0T_sb[:], d_bh0T.ap())
        bc0T_sb = per.tile([128, 4], F32)
        nc.sync.dma_start(bc0T_sb[:], d_bc0T.ap())
        bgT_sb = per.tile([128, 16], F32)
        nc.sync.dma_start(bgT_sb[:], d_bgT.ap())
        boT_sb = per.tile([128, VT], F32)
        nc.sync.dma_start(boT_sb[:], d_boT.ap())
        feats16_sb = per.tile([128, 2 * BL, C], F16)
        nc.sync.dma_start(feats16_sb[:],
                          d_feats16.ap().rearrange("(bh p) c -> p bh c", p=128))
        WahT_sb = per.tile([128, 8, G4], F16)
        nc.sync.dma_start(WahT_sb[:],
                          d_WahT.ap().rearrange("(kc p) j -> p kc j", p=128))

        keysT_sb = per.tile([128, 4, BL, KP], F16)   # padded [u, b, k]
        ge_sb = per.tile([128, 16, TB], F16)         # emb-part of gates + bias
        embT_sb = per.tile([128, 2, TB], F32)
        outsT16 = per.tile([128, 10, t_steps, BL], F16)

        hidT_sb = per.tile([128, 4, BL], F32)
        cellT_sb = per.tile([128, 4, BL], F32)
        # kc 0-3 = aT, 4-7 = hidT; 40-wide so score matmuls can take 32-col
        # lhsT slices (cols 8..39 zeroed; junk rows land on unused partitions)
        xT16 = per.tile([128, 8, BL + 32], F16)

        # =========== Phase A: embedding gather + renorm + transpose =========
        with tc.tile_pool(name="embp", bufs=1) as embp, \
             tc.tile_pool(name="embps", bufs=1, space="PSUM") as embps:
            segs = [(0, min(128, TB))]
            if TB > 128:
                segs.append((128, TB))
            ps_e = embps.tile([128, 4, 128], F32)
            for si, (r0, r1) in enumerate(segs):
                npart, off = r1 - r0, r0
                g = embp.tile([npart, E], F32, tag=f"eg{si}")
                nc.sync.dma_start(g[:], d_emb.ap()[r0:r1, :])
                sq = embp.tile([npart, E], F32, tag=f"sq{si}")
                n2 = embp.tile([npart, 1], F32, tag=f"n2{si}")
                nc.scalar.activation(sq[:], g[:], AF.Square,
                                     accum_out=n2[:, :1])
                nrm = embp.tile([npart, 1], F32, tag=f"nr{si}")
                nc.scalar.sqrt(nrm[:], n2[:])
                nc.vector.tensor_scalar_max(nrm[:], nrm[:], 1e-12)
                inv = embp.tile([npart, 1], F32, tag=f"iv{si}")
                nc.vector.reciprocal(inv[:], nrm[:])
                nc.vector.tensor_scalar_mul(inv[:], inv[:], 5.0)
                nc.vector.tensor_scalar_min(inv[:], inv[:], 1.0)
                nc.vector.tensor_scalar_mul(g[:], g[:], inv[:, :1])
                # transpose [npart, 256] -> embT [256, npart]
                for ee in range(2):
                    nc.tensor.transpose(
                        ps_e[:, 2 * si + ee, 0:npart],
                        g[:, ee * 128:(ee + 1) * 128],
                        ident[0:npart, 0:npart])
                    nc.vector.tensor_copy(
                        embT_sb[:, ee, off:off + npart],
                        ps_e[:, 2 * si + ee, 0:npart])
            # fp16 copy into outsT16 emb rows (kc 8,9)
            nc.vector.tensor_copy(
                outsT16[:, 8:10, :, :],
                embT_sb[:].rearrange("p ee (t b) -> p ee t b", b=BL))
            emb16 = embp.tile([128, 2, TB], F16)
            nc.vector.tensor_copy(emb16[:], embT_sb[:])

            # ======= Phase B: gates_e = W_ihe @ embT (+bias), fp16 =======
            with tc.tile_pool(name="geps", bufs=1, space="PSUM") as geps:
                for half in range(2):
                    ps_ge = geps.tile([128, 8, 256], F32, tag="ge")
                    for jt8 in range(8):
                        jt = half * 8 + jt8
                        for ee in range(2):
                            nc.tensor.matmul(
                                ps_ge[:, jt8, 0:TB],
                                WieT_sb[:, ee, jt * 128:(jt + 1) * 128],
                                emb16[:, ee, :],
                                start=(ee == 0), stop=(ee == 1))
                    for jt8 in range(8):
                        jt = half * 8 + jt8
                        eng = nc.vector if jt8 % 2 == 0 else nc.scalar
                        if eng is nc.vector:
                            nc.vector.tensor_scalar_add(
                                ge_sb[:, jt, :], ps_ge[:, jt8, 0:TB],
                                bgT_sb[:, jt:jt + 1])
                        else:
                            nc.scalar.activation(
                                ge_sb[:, jt, :], ps_ge[:, jt8, 0:TB],
                                AF.Identity, bias=bgT_sb[:, jt:jt + 1])

        # =========== Phase C: feats_mean + keys + initial state ===========
        with tc.tile_pool(name="inip", bufs=1) as inip, \
             tc.tile_pool(name="inips", bufs=1, space="PSUM") as inips, \
             tc.tile_pool(name="keyps", bufs=2, space="PSUM") as keyps:
            fsum = inip.tile([128, 4, BL], F16)
            for cc in range(4):
                nc.vector.tensor_reduce(
                    fsum[:, cc, :],
                    featsT_sb[:, cc, :].rearrange("p (b k) -> p b k", b=BL),
                    axis=mybir.AxisListType.X, op=OP.add)
            # keys: out keysT [u-tile, (b,k)] ; evac into padded [u, b, KP]
            HHW = 4 * HW  # half the (b,k) range = 784
            for jt in range(4):
                for half in range(2):
                    ps_k = keyps.tile([128, HHW], F32, tag="k")
                    for cc in range(4):
                        for (n0, n1) in [(0, 512), (512, HHW)]:
                            nc.tensor.matmul(
                                ps_k[:, n0:n1],
                                WkT_sb[:, cc, jt * 128:(jt + 1) * 128],
                                featsT_sb[:, cc,
                                          half * HHW + n0:half * HHW + n1],
                                start=(cc == 0), stop=(cc == 3))
                    out_ap = keysT_sb[:, jt, half * 4:(half + 1) * 4, 0:HW]
                    in_ap = ps_k[:].rearrange("p (b k) -> p b k", b=4)
                    if (2 * jt + half) % 2 == 0:
                        nc.vector.tensor_scalar_add(out_ap, in_ap,
                                                    bkT_sb[:, jt:jt + 1])
                    else:
                        nc.scalar.activation(out_ap, in_ap, AF.Identity,
                                             bias=bkT_sb[:, jt:jt + 1])
                    # zero the k-padding (196..256) with a rounding store
                    nc.vector.tensor_scalar_mul(
                        keysT_sb[:, jt, half * 4:(half + 1) * 4, HW:KP],
                        ps_k[:, 0:4 * (KP - HW)]
                        .rearrange("p (b k) -> p b k", b=4),
                        0.0)
            # hid0 / cell0 (M-orientation, f32r)
            ps_i = inips.tile([128, 4, 128], F32, tag="i0")
            ps_c = inips.tile([128, 4, 128], F32, tag="c0")
            for jt in range(4):
                for cc in range(4):
                    nc.tensor.matmul(
                        ps_i[:, jt, 0:BL],
                        Wh0T_sb[:, cc, jt * 128:(jt + 1) * 128],
                        fsum[:, cc, :],
                        start=(cc == 0), stop=(cc == 3))
                    nc.tensor.matmul(
                        ps_c[:, jt, 0:BL],
                        Wc0T_sb[:, cc, jt * 128:(jt + 1) * 128],
                        fsum[:, cc, :],
                        start=(cc == 0), stop=(cc == 3))
            for jt in range(4):
                nc.vector.tensor_scalar_add(hidT_sb[:, jt, :],
                                            ps_i[:, jt, 0:BL],
                                            bh0T_sb[:, jt:jt + 1])
                nc.vector.tensor_scalar_add(cellT_sb[:, jt, :],
                                            ps_c[:, jt, 0:BL],
                                            bc0T_sb[:, jt:jt + 1])
            nc.vector.tensor_copy(xT16[:, 4:8, 0:BL], hidT_sb[:])
            nc.vector.tensor_scalar_mul(
                xT16[:, :, BL:], bh0T_sb[:, 0:1].unsqueeze(-1)
                .to_broadcast([128, 8, 32]), 0.0)

        early.__exit__(None, None, None)

        # prefetch all logits weight tiles (sync queue stays unblocked;
        # loads beyond the buffer count trickle in as logits consumes)
        lwp = ctx.enter_context(tc.tile_pool(name="lw", bufs=48))
        wtiles = []
        for mt in range(VT):
            wt = lwp.tile([128, 10, 128], F16, tag="wt", name=f"wt{mt}")
            nc.sync.dma_start(
                wt[:].rearrange("p kc m -> p (kc m)"), d_WoT.ap()[mt, :, :])
            wtiles.append(wt)

        # ======================= Phase D: the scan =======================
        with tc.tile_pool(name="scps", bufs=1, space="PSUM") as scps, \
             tc.tile_pool(name="scsb", bufs=2) as scsb:
            ps_sc2 = [scps.tile([128, 2, 512], F32, tag=f"sc{i}", name=f"ps_sc{i}")
                      for i in range(2)]
            nc.vector.memset(ps_sc2[0][:], 0.0)
            nc.vector.memset(ps_sc2[1][:], 0.0)
            ps_tr = scps.tile([128, 4, 128], F32)
            ps_aT = scps.tile([128, 4, BL], F32)
            ps_g2 = [scps.tile([128, 2, 16, BL], F32, tag=f"g{i}",
                               name=f"ps_g{i}") for i in range(2)]
            w_sb2 = [scsb.tile([128, 2, KP], F32, tag=f"w{i}", name=f"w_sb{i}")
                     for i in range(2)]
            nc.gpsimd.memset(w_sb2[0][:], 0.0)
            nc.gpsimd.memset(w_sb2[1][:], 0.0)

            for t in range(t_steps):
                ps_sc = ps_sc2[t % 2]
                ps_g = ps_g2[t % 2]

                # 1. scores (col-tiled, f32r): batch b=g*4+c -> psum part 32c
                for g in range(2):
                    for c in range(4):
                        b = g * 4 + c
                        for jj in range(4):
                            nc.tensor.matmul(
                                ps_sc[32 * c:32 * c + 32, g, 0:HW],
                                xT16[:, 4 + jj, b:b + 32],
                                keysT_sb[:, jj, b, 0:HW],
                                start=(jj == 0), stop=(jj == 3),
                                tile_position=(0, 32 * c))

                # 2. gates hid-part (can overlap with attention)
                for jt in range(16):
                    for kc in range(4, 8):
                        nc.tensor.matmul(
                            ps_g[:, 0, jt, :],
                            WahT_sb[:, kc, jt * 128:(jt + 1) * 128],
                            xT16[:, kc, 0:BL],
                            start=(kc == 4), stop=(kc == 7))

                # 3-6. per-group softmax -> transpose -> eT (g0 chain
                # overlaps g1 scores/exp on other engines)
                w_sb = w_sb2[t % 2]
                sume = scsb.tile([128, 2], F32, tag="sume")
                eT16 = scsb.tile([128, 4, 4], F16, tag="eT")
                rinv = scsb.tile([128, 2], F32, tag="rinv")
                for g in range(2):
                    nc.scalar.activation(
                        w_sb[:, g, 0:HW], ps_sc[:, g, 0:HW], AF.Exp,
                        scale=INV_SCALE, accum_out=sume[:, g:g + 1])
                    nc.vector.reciprocal(rinv[:, g:g + 1], sume[:, g:g + 1])
                    nc.vector.tensor_scalar_mul(
                        w_sb[:, g, 0:HW], w_sb[:, g, 0:HW], rinv[:, g:g + 1])
                    for h in range(2):
                        hh = g * 2 + h
                        nc.tensor.transpose(
                            ps_tr[:, hh, :],
                            w_sb[:, g, h * 128:(h + 1) * 128],
                            ident[:])
                    nc.vector.tensor_copy(
                        eT16[:, 2 * g:2 * g + 2, :],
                        ps_tr[:, 2 * g:2 * g + 2, 0:128:32])
                    nc.gpsimd.dma_start(
                        d_attn.ap()[g * 4:(g + 1) * 4, t, :],
                        w_sb[0:128:32, g, 0:HW])

                # 7. attention output aT[c,b] (fp16 feats as weights)
                for b in range(BL):
                    g, c = b // 4, b % 4
                    for cm in range(4):
                        for h in range(2):
                            nc.tensor.matmul(
                                ps_aT[:, cm, b:b + 1],
                                feats16_sb[:, b * 2 + h,
                                           cm * 128:(cm + 1) * 128],
                                eT16[:, g * 2 + h, c:c + 1],
                                start=(h == 0), stop=(h == 1))
                nc.vector.tensor_copy(xT16[:, 0:4, 0:BL], ps_aT[:])
                nc.vector.tensor_copy(outsT16[:, 4:8, t, :], ps_aT[:])

                # 8. gates a-part
                for jt in range(16):
                    for kc in range(0, 4):
                        nc.tensor.matmul(
                            ps_g[:, 1, jt, :],
                            WahT_sb[:, kc, jt * 128:(jt + 1) * 128],
                            xT16[:, kc, 0:BL],
                            start=(kc == 0), stop=(kc == 3))

                # 9. add hid-part + emb-part(with bias) + a-part
                gf = scsb.tile([128, 16, BL], F32, tag="gf")
                nc.vector.tensor_tensor(
                    out=gf[:], in0=ps_g[:, 0, :, :],
                    in1=ge_sb[:, :, t * BL:(t + 1) * BL], op=OP.add)
                nc.vector.tensor_tensor(
                    out=gf[:], in0=ps_g[:, 1, :, :], in1=gf[:], op=OP.add)

                # 10. LSTM pointwise on [128, (16jj, 8b)]
                gff = gf[:].rearrange("p jj b -> p (jj b)")
                pw = scsb.tile([128, 128], F32, tag="pw")
                pwf = pw[:]
                # sigmoid via tanh so the scan only ever needs the
                # exp+tanh ACT table (sigmoid lives in a different LUT set;
                # mixing would cost 2 x 1.28us table reloads per step)
                nc.scalar.activation(pwf[:, 0:96], gff[:, 0:96], AF.Tanh,
                                     scale=0.5)
                nc.vector.tensor_scalar(
                    out=pwf[:, 0:96], in0=pwf[:, 0:96],
                    scalar1=0.5, scalar2=0.5,
                    op0=OP.mult, op1=OP.add)
                nc.scalar.activation(pwf[:, 96:128], gff[:, 96:128], AF.Tanh)
                m1 = scsb.tile([128, 32], F32, tag="m1")
                m2 = scsb.tile([128, 32], F32, tag="m2")
                cellf = cellT_sb[:].rearrange("p jj b -> p (jj b)")
                hidf = hidT_sb[:].rearrange("p jj b -> p (jj b)")
                nc.vector.tensor_tensor(out=m1[:], in0=pwf[:, 32:64],
                                        in1=cellf, op=OP.mult)
                nc.vector.tensor_tensor(out=m2[:], in0=pwf[:, 0:32],
                                        in1=pwf[:, 96:128], op=OP.mult)
                nc.vector.tensor_tensor(out=cellf, in0=m1[:], in1=m2[:],
                                        op=OP.add)
                tc_ = scsb.tile([128, 32], F32, tag="tc")
                nc.scalar.activation(tc_[:], cellf, AF.Tanh)
                nc.vector.tensor_tensor(out=hidf, in0=pwf[:, 64:96],
                                        in1=tc_[:], op=OP.mult)
                nc.vector.tensor_copy(xT16[:, 4:8, 0:BL], hidT_sb[:])
                nc.vector.tensor_copy(outsT16[:, 0:4, t, :], hidT_sb[:])

        # ======================= Phase E: logits =======================
        with tc.tile_pool(name="lo", bufs=2) as lop, \
             tc.tile_pool(name="lps", bufs=4, space="PSUM") as lps:
            GRP = 8
            for mt0 in range(0, VT, GRP):
                n_mt = min(GRP, VT - mt0)
                stage = lop.tile([128, GRP, TB], F32, tag="lo",
                                 name=f"st{mt0}")
                for mi in range(n_mt):
                    mt = mt0 + mi
                    ps_l = lps.tile([128, 256], F32, tag="l",
                                    name=f"psl{mt}")
                    for kc in range(10):
                        nc.tensor.matmul(
                            ps_l[:, 0:TB], wtiles[mt][:, kc, :],
                            outsT16[:, kc, :, :],
                            start=(kc == 0), stop=(kc == 9))
                    if mt % 2 == 0:
                        nc.vector.tensor_scalar_add(
                            stage[:, mi, :], ps_l[:, 0:TB],
                            boT_sb[:, mt:mt + 1])
                    else:
                        nc.scalar.activation(
                            stage[:, mi, :], ps_l[:, 0:TB], AF.Identity,
                            bias=boT_sb[:, mt:mt + 1])
                nc.sync.dma_start(
                    d_logitsT.ap()[mt0:mt0 + n_mt, :, :]
                    .rearrange("m p n -> p m n"),
                    stage[:, 0:n_mt, :])

    nc.compile()
    return nc


def _prep_inputs(image_features, captions_ix, W_h0, b_h0, W_c0, b_c0, emb,
                 W_key, b_key, W_ih, b_ih, W_hh, b_hh, W_out, b_out,
                 t_steps=T):
    """Host-side sharding/layout. Returns list of per-core in_maps."""
    f32 = np.float32
    f16 = np.float16
    img = np.ascontiguousarray(np.asarray(image_features, dtype=f32))
    cap = np.asarray(captions_ix).astype(np.int32)[:, :t_steps]

    WkT = np.ascontiguousarray(np.asarray(W_key, f32).T.astype(f16))
    bkT = np.ascontiguousarray(np.asarray(b_key, f32).reshape(4, 128).T)
    Wh0T = np.ascontiguousarray((np.asarray(W_h0, f32).T / float(HW)).astype(f16))
    bh0T = np.ascontiguousarray(np.asarray(b_h0, f32).reshape(4, 128).T)
    Wc0T = np.ascontiguousarray((np.asarray(W_c0, f32).T / float(HW)).astype(f16))
    bc0T = np.ascontiguousarray(np.asarray(b_c0, f32).reshape(4, 128).T)
    W_ih = np.asarray(W_ih, f32)
    W_hh = np.asarray(W_hh, f32)
    gperm = np.r_[0:2 * U, 3 * U:4 * U, 2 * U:3 * U]   # (i,f,g,o)->(i,f,o,g)
    WieT = np.ascontiguousarray(W_ih[gperm, :E].T.astype(f16))
    WahT = np.ascontiguousarray(
        np.concatenate([W_ih[gperm, E:].T,
                        np.asarray(W_hh, f32)[gperm].T], axis=0).astype(f16))
    bgT = np.ascontiguousarray(
        (np.asarray(b_ih, f32) + np.asarray(b_hh, f32))[gperm]
        .reshape(16, 128).T)
    WoTf = np.zeros((OD, VP), f16)
    WoTf[:, :V] = np.asarray(W_out, f32).T.astype(f16)
    # per-tile layout [VT, 128, (kc, m)]: partition p's whole SBUF row is one
    # contiguous 2.5KB DRAM run per tile
    WoT = np.ascontiguousarray(
        WoTf.reshape(10, 128, VT, 128).transpose(2, 1, 0, 3)
        .reshape(VT, 128, OD))
    boT = np.zeros((VP,), f32)
    boT[:V] = np.asarray(b_out, f32)
    boT = np.ascontiguousarray(boT.reshape(VT, 128).T)
    embf = np.ascontiguousarray(np.asarray(emb, f32))

    in_maps = []
    for ci in range(NCORES):
        sl = slice(ci * BL, (ci + 1) * BL)
        img_l = img[sl]                                   # [BL, C, HW]
        featsT = np.ascontiguousarray(
            img_l.transpose(1, 0, 2).reshape(C, BL * HW).astype(f16))
        fp = np.zeros((BL, KP, C), f32)
        fp[:, :HW, :] = img_l.transpose(0, 2, 1)
        feats16 = np.ascontiguousarray(fp.reshape(BL * KP, C).astype(f16))
        emb_rows = np.ascontiguousarray(
            embf[cap[sl].T.reshape(t_steps * BL)])        # row r = t*BL+b
        in_maps.append({
            "featsT": featsT, "feats16": feats16, "emb": emb_rows,
            "WkT": WkT, "bkT": bkT,
            "Wh0T": Wh0T, "bh0T": bh0T, "Wc0T": Wc0T, "bc0T": bc0T,
            "WieT": WieT, "WahT": WahT, "bgT": bgT,
            "WoT": WoT, "boT": boT,
        })
    return in_maps


def _assemble(results, t_steps=T):
    logits = np.empty((B, t_steps, V), np.float32)
    attn = np.empty((B, t_steps, HW), np.float32)
    for ci, r in enumerate(results):
        lt = r["logitsT"].reshape(VP, t_steps * BL)[:V]   # [V, (t,b)]
        logits[ci * BL:(ci + 1) * BL] = (
            lt.reshape(V, t_steps, BL).transpose(2, 1, 0))
        attn[ci * BL:(ci + 1) * BL] = r["attn"]
    return logits, attn


def kernel(**inputs):
    if "nc" not in _CACHE:
        _CACHE["nc"] = _build(T)
    nc = _CACHE["nc"]
    in_maps = _prep_inputs(**inputs)
    res = run_bass_kernel_spmd(nc, in_maps, core_ids=list(range(NCORES)))
    return _assemble(res.results)
